# revision 1
# baseline (speedup 1.0000x reference)
"""Trainium2 Bass kernel for a pre-norm transformer block (dense_transformer).

Computation (per reference):
    x = x + Attn(LN1(x));  x = x + MLP(LN2(x))
with causal multi-head attention (H=16 heads, D=64) and a 4E ReLU MLP.

Sharding strategy (no collectives):
    8 cores = 2 batches x 4 query-blocks of 512 tokens.  Each core computes
    the full block output for its 512 query rows.  K/V are recomputed per
    core for the causal prefix.  To keep the SPMD program identical across
    cores, the context is ROTATED so the query block always sits in slots
    [T-512, T): slots [0, pad) are zero padding, masked via a per-partition
    additive bias (-1e9) fused into the softmax exp; the causal diagonal
    band mask is a fixed tensor shared by all cores.

Layouts: activations are kept feature-major (E on partitions, tokens on the
free axis) the whole way through, so no on-device transposes are needed.
The host transposes x / assembles the output.  Matmuls run in bf16 with
fp32 PSUM accumulation; LN / softmax / residuals are fp32.  Softmax row
sums come for free from a ones-column appended to V (M=65 matmuls).
"""

from dataclasses import dataclass

import numpy as np
import ml_dtypes

import concourse.bass as bass  # noqa: F401
import concourse.mybir as mybir
import concourse.tile as tile
from concourse import bacc
from concourse import bass_utils

F32 = mybir.dt.float32
BF16 = mybir.dt.bfloat16
AF = mybir.ActivationFunctionType
OP = mybir.AluOpType
NPBF16 = ml_dtypes.bfloat16

P = 128
NEG = -1.0e9


@dataclass(frozen=True)
class Cfg:
    B: int = 2
    T: int = 2048
    E: int = 1024
    H: int = 16
    D: int = 64
    NC: int = 8
    eps: float = 1e-5

    @property
    def CPB(self):
        return self.NC // self.B

    @property
    def Tq(self):
        return self.T // self.CPB

    @property
    def KE(self):
        return self.E // P

    @property
    def TK(self):
        return self.T // P

    @property
    def HP(self):
        return self.H // 2

    @property
    def NB(self):
        return self.Tq // P

    @property
    def F(self):
        return 4 * self.E

    @property
    def KF(self):
        return self.F // P

    @property
    def TCH(self):
        return min(512, self.T)

    @property
    def NQC(self):
        return self.T // self.TCH

    def check(self):
        assert self.D == 64 and self.E == self.H * self.D
        assert self.Tq <= 512 and self.Tq % P == 0
        assert self.T % self.TCH == 0 and self.E % P == 0 and self.F % P == 0
        assert self.NC % self.B == 0 and self.H % 4 == 0 and self.KE % 2 == 0


CFG = Cfg()


class Pools:
    """Tile pools with explicit open/close (LIFO per side, per space)."""

    def __init__(self, tc, prefix=""):
        self.tc = tc
        self.prefix = prefix
        self.live = {}

    def open(self, key, bufs, space=None, side=None):
        kw = dict(name=self.prefix + key, bufs=bufs)
        if space:
            kw["space"] = space
        if side:
            kw["side"] = side
        cm = self.tc.tile_pool(**kw)
        pool = cm.__enter__()
        self.live[key] = cm
        return pool

    def close(self, *keys):
        for key in keys:
            self.live.pop(key).__exit__(None, None, None)

    def close_all(self):
        for key in reversed(list(self.live)):
            self.close(key)


def _emit(tc, c: Cfg, d, reps: int = 1):
    for _rep in range(reps):
        _emit_one(tc, c, d, _rep)


def _emit_one(tc, c: Cfg, d, rep: int):
    nc = tc.nc
    E, T, Tq, H, D = c.E, c.T, c.Tq, c.H, c.D
    KE, TK, HP, NB, KF = c.KE, c.TK, c.HP, c.NB, c.KF
    TCH, NQC = c.TCH, c.NQC
    DQ = D + 1
    SCL = 1.0 / float(np.sqrt(D))

    pp = Pools(tc, prefix=f"r{rep}_")

    # ---------------- constants (whole-kernel lifetime) --------------------
    const = pp.open("const", 1)
    ones_bf = const.tile([P, 1], BF16, name="ones_bf")
    nc.vector.memset(ones_bf[:], 1.0)
    ones_f1 = const.tile([1, P], F32, name="ones_f1")
    nc.vector.memset(ones_f1[:], 1.0)
    onehot = const.tile([P, HP * P], BF16, name="onehot")
    nc.sync.dma_start(onehot[:], d["onehot"])
    colmask = const.tile([P, TK], F32, name="colmask")
    nc.sync.dma_start(colmask[:], d["colmask"])
    band01 = const.tile([P, NB * 4 * Tq], BF16, name="band01")
    nc.sync.dma_start(band01[:], d["band01"])
    gbt = {}
    for nm, cols in [
        ("ln1g", KE), ("ln1b", KE), ("ln2g", KE), ("ln2b", KE),
        ("boc", KE), ("mb1", KF), ("mb2", KE),
    ]:
        gbt[nm] = const.tile([P, cols], F32, name=nm + "_t")
        nc.sync.dma_start(gbt[nm][:], d[nm])

    # ---------------- long-lived activations ------------------------------
    p_xq = pp.open("xqp", 1)
    xq = [p_xq.tile([P, Tq], F32, name=f"xq{e}") for e in range(KE)]

    # ======================================================================
    # Phase 0: load x^T + LayerNorm1 (feature-major, per-column stats)
    # ======================================================================
    p_xn = pp.open("xnp", 1)
    p_xt = pp.open("xtp", 1)
    p_tmp = pp.open("ln_tmp", 4)
    p_rows = pp.open("ln_rows", 1)
    ps_st = pp.open("ln_st", 1, "PSUM")
    ps_bc = pp.open("ln_bc", 2, "PSUM")

    ps_wm = pp.open("warm_ps", 1, "PSUM")
    wmp = ps_wm.tile([1, TCH], F32, name="wmp")
    for _w in range(24):
        nc.tensor.matmul(
            wmp[:], ones_bf[:], band01[:, 0:TCH], start=True, stop=True
        )
    pp.close("warm_ps")

    xt = [p_xt.tile([P, T], F32, name=f"xt{e}") for e in range(KE)]
    xn = [p_xn.tile([P, T], BF16, name=f"xn{e}") for e in range(KE)]
    for e in range(KE):
        nc.sync.dma_start(xt[e][:], d["xt"][e * P : (e + 1) * P, :])
        nc.vector.tensor_copy(xq[e][:], xt[e][:, T - Tq :])

    for ci in range(NQC):
        cs = slice(ci * TCH, (ci + 1) * TCH)
        s1 = ps_st.tile([1, TCH], F32, name="s1")
        s2 = ps_st.tile([1, TCH], F32, name="s2")
        for e in range(KE):
            xbf = p_tmp.tile([P, TCH], BF16, name="xbf")
            nc.vector.tensor_copy(xbf[:], xt[e][:, cs])
            x2 = p_tmp.tile([P, TCH], BF16, name="x2bf")
            nc.scalar.square(x2[:], xt[e][:, cs])
            nc.tensor.matmul(s1[:], ones_bf[:], xbf[:], start=(e == 0), stop=(e == KE - 1))
            nc.tensor.matmul(s2[:], ones_bf[:], x2[:], start=(e == 0), stop=(e == KE - 1))
        mu = p_rows.tile([1, TCH], F32, name="mu")
        nc.vector.tensor_scalar_mul(mu[:], s1[:], 1.0 / E)
        ve = p_rows.tile([1, TCH], F32, name="ve")
        nc.vector.tensor_scalar(ve[:], s2[:], 1.0 / E, c.eps, OP.mult, OP.add)
        mu2 = p_rows.tile([1, TCH], F32, name="mu2")
        nc.vector.tensor_tensor(mu2[:], mu[:], mu[:], OP.mult)
        vee = p_rows.tile([1, TCH], F32, name="vee")
        nc.vector.tensor_tensor(vee[:], ve[:], mu2[:], OP.subtract)
        lv = p_rows.tile([1, TCH], F32, name="lv")
        nc.scalar.activation(lv[:], vee[:], AF.Ln)
        rstd = p_rows.tile([1, TCH], F32, name="rstd")
        nc.scalar.activation(rstd[:], lv[:], AF.Exp, scale=-0.5)

        mub = ps_bc.tile([P, TCH], F32, name="mub")
        nc.tensor.matmul(mub[:], ones_f1[:], mu[:], start=True, stop=True)
        rsb = ps_bc.tile([P, TCH], F32, name="rsb")
        nc.tensor.matmul(rsb[:], ones_f1[:], rstd[:], start=True, stop=True)

        for e in range(KE):
            t1 = p_tmp.tile([P, TCH], F32, name="t1")
            nc.vector.tensor_tensor(t1[:], xt[e][:, cs], mub[:], OP.subtract)
            t2 = p_tmp.tile([P, TCH], F32, name="t2")
            nc.vector.tensor_tensor(t2[:], t1[:], rsb[:], OP.mult)
            nc.vector.tensor_scalar(
                xn[e][:, cs], t2[:],
                gbt["ln1g"][:, e : e + 1], gbt["ln1b"][:, e : e + 1],
                OP.mult, OP.add,
            )
    pp.close("ln_rows", "ln_tmp", "xtp", "ln_bc", "ln_st")

    # ======================================================================
    # Phase 1: QKV projections
    # ======================================================================
    p_wo = pp.open("wop", 1)
    wo_sb = [p_wo.tile([P, E], BF16, name=f"wo{j}") for j in range(HP)]
    for j in range(HP):
        nc.sync.dma_start(wo_sb[j][:], d["wo"][j * P : (j + 1) * P, :])

    p_qt = pp.open("qtp", 1)
    p_kt = pp.open("ktp", 1)
    p_vs = pp.open("vsp", 1)
    p_wcb = pp.open("wcb", 3)
    p_wv = pp.open("wvp", 1)
    ps_qkv = pp.open("qkv_ps", 2, "PSUM")

    qt = [p_qt.tile([P, Tq], BF16, name=f"qt{j}") for j in range(HP)]
    kt = [p_kt.tile([P, T], BF16, name=f"kt{j}") for j in range(HP)]
    vsb = [p_vs.tile([P, H * D], BF16, name=f"vsb{t}") for t in range(TK)]

    def k_proj(j, psum_pool, nm="k_ps"):
        wk_j = p_wcb.tile([P, KE, P], BF16, name="wkcb")
        nc.sync.dma_start(
            wk_j[:],
            d["wk"].rearrange("(e p) m -> p e m", p=P)[:, :, j * P : (j + 1) * P],
        )
        for ci in range(NQC):
            cs = slice(ci * TCH, (ci + 1) * TCH)
            ps = psum_pool.tile([P, TCH], F32, name=nm)
            for e in range(KE):
                nc.tensor.matmul(
                    ps[:], wk_j[:, e, :], xn[e][:, cs],
                    start=(e == 0), stop=(e == KE - 1),
                )
            nc.vector.tensor_copy(kt[j][:, cs], ps[:])

    def q_proj(j):
        wq_j = p_wcb.tile([P, KE, P], BF16, name="wqcb")
        nc.sync.dma_start(
            wq_j[:],
            d["wq"].rearrange("(e p) m -> p e m", p=P)[:, :, j * P : (j + 1) * P],
        )
        ps = ps_qkv.tile([P, Tq], F32, name="q_ps")
        for e in range(KE):
            nc.tensor.matmul(
                ps[:], wq_j[:, e, :], xn[e][:, T - Tq :],
                start=(e == 0), stop=(e == KE - 1),
            )
        nc.any.tensor_copy(qt[j][:], ps[:])

    # Q/K for the first attention group up front so its score/exp stream can
    # start while the remaining projections run; the other K projections are
    # emitted inside the attention loop.
    for j in range(min(2, HP)):
        q_proj(j)
    for j in range(min(2, HP)):
        k_proj(j, ps_qkv)
    for j in range(2, HP):
        q_proj(j)

    wv_sb = [p_wv.tile([P, E], BF16, name=f"wv{e}") for e in range(KE)]
    for e in range(KE):
        nc.sync.dma_start(wv_sb[e][:], d["wv"][e * P : (e + 1) * P, :])
    ECH = min(512, E)
    NEC = E // ECH

    def v_proj(g, psum_pool, nm="v_ps"):
        gs = slice(g * ECH, (g + 1) * ECH)
        for t in range(TK):
            ps = psum_pool.tile([P, ECH], F32, name=nm)
            for e in range(KE):
                nc.tensor.matmul(
                    ps[:], xn[e][:, t * P : (t + 1) * P], wv_sb[e][:, gs],
                    start=(e == 0), stop=(e == KE - 1),
                )
            nc.vector.tensor_copy(vsb[t][:, gs], ps[:])

    # V columns 0:512 (heads 0-7) feed attention groups 0-1; the second
    # chunk is emitted inside the attention loop to overlap the exp stream.
    v_proj(0, ps_qkv)
    pp.close("qkv_ps")

    # ======================================================================
    # Phase 2: attention (4-head groups; row-paired scores, col-paired attnV;
    # remaining K projections interleaved to keep PE fed under the exp stream)
    # ======================================================================
    HG = 4
    NG = H // HG
    GP = HG // 2

    p_ao = pp.open("aop", 1, side="right")
    p_rs = pp.open("rsp", 1)
    p_pr = pp.open("probs", 2)
    p_st2 = pp.open("rstage", 1)
    ps_k2 = pp.open("k2_ps", 1, "PSUM")
    ps_sc = pp.open("sc_ps", 1, "PSUM")
    ps_o = pp.open("o_ps", 1, "PSUM")
    ps_rs = pp.open("rs_ps", 1, "PSUM")

    aop_t = [p_ao.tile([P, Tq], BF16, name=f"aop{j}") for j in range(HP)]
    rs_all = p_rs.tile([P, Tq], F32, name="rs_all")
    nc.vector.memset(rs_all[:], 1.0)
    lrs = p_rs.tile([P, Tq], F32, name="lrs")
    irs_bf = p_rs.tile([P, Tq], BF16, name="irs_bf")
    nc.vector.memset(irs_bf[:], 0.0)

    for g in range(NG):
        opair = [ps_o.tile([P, Tq], F32, name=f"opair{i}") for i in range(GP)]
        rsps = ps_rs.tile([P, Tq], F32, name="rsps")
        for t in range(TK):
            ss = ps_sc.tile([P, HG * Tq], F32, name="ss")
            for i in range(GP):
                j = g * GP + i
                for s in (0, 1):
                    h01 = 2 * i + s
                    nc.tensor.matmul(
                        ss[:, h01 * Tq : (h01 + 1) * Tq],
                        kt[j][s * 64 : (s + 1) * 64, t * P : (t + 1) * P],
                        qt[j][s * 64 : (s + 1) * 64, :],
                        start=True, stop=True,
                        tile_position=(s * 64, 0),
                    )
            pr = p_pr.tile([P, HG * Tq], BF16, name="pr")
            nc.scalar.activation(
                pr[:], ss[:], AF.Exp, bias=colmask[:, t : t + 1], scale=SCL
            )
            bt = t - (TK - NB)
            if bt >= 0:
                nc.vector.tensor_tensor(
                    pr[:], pr[:],
                    band01[:, bt * HG * Tq : (bt + 1) * HG * Tq], OP.mult,
                )
            for i in range(GP):
                j = g * GP + i
                for s in (0, 1):
                    h = 2 * j + s
                    h01 = 2 * i + s
                    nc.tensor.matmul(
                        opair[i][s * 64 : (s + 1) * 64, :],
                        vsb[t][:, h * D : (h + 1) * D],
                        pr[:, h01 * Tq : (h01 + 1) * Tq],
                        start=(t == 0), stop=(t == TK - 1),
                        tile_position=(0, s * 64),
                        skip_group_check=True,
                    )
            for h01 in range(HG):
                nc.tensor.matmul(
                    rsps[32 * h01 : 32 * h01 + 1, :],
                    ones_bf[:],
                    pr[:, h01 * Tq : (h01 + 1) * Tq],
                    start=(t == 0), stop=(t == TK - 1),
                    tile_position=(0, 32 * h01),
                    skip_group_check=True,
                )
        for i in range(GP):
            nc.vector.tensor_copy(aop_t[g * GP + i][:], opair[i][:])
        st = p_st2.tile([P, Tq], F32, name="rstage")
        for h01 in range(HG):
            nc.vector.tensor_copy(
                st[32 * h01 : 32 * h01 + 1, :], rsps[32 * h01 : 32 * h01 + 1, :]
            )
        for h01 in range(HG):
            nc.sync.dma_start(
                rs_all[32 * g + h01 : 32 * g + h01 + 1, :],
                st[32 * h01 : 32 * h01 + 1, :],
            )
        # emit the next group's K projections here: they fill the tensor
        # engine while this group's exp/attnV pipeline drains
        if g + 1 < NG:
            k_proj(2 * (g + 1), ps_k2, nm="kv_ps")
            k_proj(2 * (g + 1) + 1, ps_k2, nm="kv_ps")
        if g == 1 and NEC > 1:
            v_proj(1, ps_k2, nm="kv_ps")

    pp.close("rstage", "probs")
    pp.close("rs_ps", "o_ps", "sc_ps", "k2_ps")

    # softmax denominators (1/rs via exp(-ln)) -> normalize pairs
    p_nb = pp.open("nrm", 2)
    ps_n = pp.open("n_ps", 2, "PSUM")
    nc.scalar.activation(lrs[:], rs_all[:], AF.Ln)
    nc.scalar.activation(irs_bf[:], lrs[:], AF.Exp, scale=-1.0)
    for j in range(HP):
        bb = 64 * (j // 4)  # lhsT base partition must be in {0, 32, 64}
        nb = ps_n.tile([P, Tq], F32, name="nb")
        nc.tensor.matmul(
            nb[:],
            onehot[bb : bb + 64, j * P : (j + 1) * P],
            irs_bf[bb : bb + 64, :],
            start=True, stop=True,
        )
        nbs = p_nb.tile([P, Tq], BF16, name="nbs")
        nc.vector.tensor_copy(nbs[:], nb[:])
        nc.vector.tensor_tensor(aop_t[j][:], aop_t[j][:], nbs[:], OP.mult)
    pp.close("nrm", "n_ps")
    pp.close("rsp", "wvp", "wcb")
    pp.close("vsp", "ktp", "qtp")

    # ======================================================================
    # Phase 3: out-projection + residual -> xres; LayerNorm2 -> xn2
    # ======================================================================
    p_xr = pp.open("xrp", 1)
    p_x2 = pp.open("xn2p", 1)
    ps_ao = pp.open("ao_ps", 2, "PSUM")

    xres = [p_xr.tile([P, Tq], F32, name=f"xres{e}") for e in range(KE)]
    xn2 = [p_x2.tile([P, Tq], BF16, name=f"xn2{e}") for e in range(KE)]

    for e in range(KE):
        ps = ps_ao.tile([P, Tq], F32, name="aops")
        for j in range(HP):
            nc.tensor.matmul(
                ps[:], wo_sb[j][:, e * P : (e + 1) * P], aop_t[j][:],
                start=(j == 0), stop=(j == HP - 1),
            )
        nc.vector.scalar_tensor_tensor(
            xres[e][:], ps[:], gbt["boc"][:, e : e + 1], xq[e][:], OP.add, OP.add
        )
    pp.close("ao_ps", "aop")

    # LayerNorm2 over the Tq query columns
    p_tmp = pp.open("ln2_tmp", 3)
    p_rows = pp.open("ln2_rows", 1)
    ps_st = pp.open("ln2_st", 1, "PSUM")
    ps_bc = pp.open("ln2_bc", 1, "PSUM")
    s1 = ps_st.tile([1, Tq], F32, name="s1b")
    s2 = ps_st.tile([1, Tq], F32, name="s2b")
    for e in range(KE):
        xbf = p_tmp.tile([P, Tq], BF16, name="xbf2")
        nc.vector.tensor_copy(xbf[:], xres[e][:])
        x2 = p_tmp.tile([P, Tq], BF16, name="x2bf2")
        nc.scalar.square(x2[:], xres[e][:])
        nc.tensor.matmul(s1[:], ones_bf[:], xbf[:], start=(e == 0), stop=(e == KE - 1))
        nc.tensor.matmul(s2[:], ones_bf[:], x2[:], start=(e == 0), stop=(e == KE - 1))
    mu = p_rows.tile([1, Tq], F32, name="mu_2")
    nc.vector.tensor_scalar_mul(mu[:], s1[:], 1.0 / E)
    ve = p_rows.tile([1, Tq], F32, name="ve_2")
    nc.vector.tensor_scalar(ve[:], s2[:], 1.0 / E, c.eps, OP.mult, OP.add)
    mu2 = p_rows.tile([1, Tq], F32, name="mu2_2")
    nc.vector.tensor_tensor(mu2[:], mu[:], mu[:], OP.mult)
    vee = p_rows.tile([1, Tq], F32, name="vee_2")
    nc.vector.tensor_tensor(vee[:], ve[:], mu2[:], OP.subtract)
    lv = p_rows.tile([1, Tq], F32, name="lv_2")
    nc.scalar.activation(lv[:], vee[:], AF.Ln)
    rstd = p_rows.tile([1, Tq], F32, name="rstd_2")
    nc.scalar.activation(rstd[:], lv[:], AF.Exp, scale=-0.5)
    mub = ps_bc.tile([P, Tq], F32, name="mub2")
    nc.tensor.matmul(mub[:], ones_f1[:], mu[:], start=True, stop=True)
    rsb = ps_bc.tile([P, Tq], F32, name="rsb2")
    nc.tensor.matmul(rsb[:], ones_f1[:], rstd[:], start=True, stop=True)
    for e in range(KE):
        t1 = p_tmp.tile([P, Tq], F32, name="t1b")
        nc.vector.tensor_tensor(t1[:], xres[e][:], mub[:], OP.subtract)
        t2 = p_tmp.tile([P, Tq], F32, name="t2b")
        nc.vector.tensor_tensor(t2[:], t1[:], rsb[:], OP.mult)
        nc.vector.tensor_scalar(
            xn2[e][:], t2[:],
            gbt["ln2g"][:, e : e + 1], gbt["ln2b"][:, e : e + 1],
            OP.mult, OP.add,
        )
    pp.close("ln2_rows", "ln2_tmp", "ln2_bc", "ln2_st")

    # ======================================================================
    # Phase 4+5: MLP (layer 1 streamed with first-half layer 2, then rest)
    # ======================================================================
    EH = min(KE, 6)  # h2 chunks accumulated under MLP1 (PSUM: 6 + 2 h1 bufs)
    p_h1 = pp.open("h1p", 1, side="right")
    p_w2 = pp.open("w2s", 3)
    p_out = pp.open("outp", 2)
    p_w1 = pp.open("w1s", 3)
    ps_h1 = pp.open("h1_ps", 2, "PSUM")
    ps_h2a = pp.open("h2a_ps", 1, "PSUM")

    h1 = [p_h1.tile([P, Tq], BF16, name=f"h1{f}") for f in range(KF)]
    h2a = [ps_h2a.tile([P, Tq], F32, name=f"h2a{e}") for e in range(EH)]
    for f in range(KF):
        w1f = p_w1.tile([P, KE, P], BF16, name="w1cb")
        nc.sync.dma_start(
            w1f[:],
            d["w1"].rearrange("(e p) m -> p e m", p=P)[:, :, f * P : (f + 1) * P],
        )
        ps = ps_h1.tile([P, Tq], F32, name="h1ps")
        for e in range(KE):
            nc.tensor.matmul(
                ps[:], w1f[:, e, :], xn2[e][:], start=(e == 0), stop=(e == KE - 1)
            )
        nc.scalar.activation(
            h1[f][:], ps[:], AF.Relu, bias=gbt["mb1"][:, f : f + 1], scale=1.0
        )
        w2f = p_w2.tile([P, E], BF16, name="w2sa")
        nc.sync.dma_start(w2f[:], d["w2"][f * P : (f + 1) * P, :])
        for e in range(EH):
            nc.tensor.matmul(
                h2a[e][:], w2f[:, e * P : (e + 1) * P], h1[f][:],
                start=(f == 0), stop=(f == KF - 1),
            )
    for e in range(EH):
        of = p_out.tile([P, Tq], F32, name="outf")
        nc.vector.scalar_tensor_tensor(
            of[:], h2a[e][:], gbt["mb2"][:, e : e + 1], xres[e][:], OP.add, OP.add
        )
        nc.sync.dma_start(d["out_t"][e * P : (e + 1) * P, :], of[:])
    pp.close("w1s", "h2a_ps", "h1_ps")

    if EH < KE:
        ps_h2b = pp.open("h2b_ps", 1, "PSUM")
        h2b = [ps_h2b.tile([P, Tq], F32, name=f"h2b{e}") for e in range(KE - EH)]
        for f in range(KF):
            w2f = p_w2.tile([P, E], BF16, name="w2sb")
            nc.sync.dma_start(w2f[:], d["w2"][f * P : (f + 1) * P, :])
            for i, e in enumerate(range(EH, KE)):
                nc.tensor.matmul(
                    h2b[i][:], w2f[:, e * P : (e + 1) * P], h1[f][:],
                    start=(f == 0), stop=(f == KF - 1),
                )
        for i, e in enumerate(range(EH, KE)):
            of = p_out.tile([P, Tq], F32, name="outf")
            nc.vector.scalar_tensor_tensor(
                of[:], h2b[i][:], gbt["mb2"][:, e : e + 1], xres[e][:], OP.add, OP.add
            )
            nc.sync.dma_start(d["out_t"][e * P : (e + 1) * P, :], of[:])

    pp.close_all()


def build_program(c: Cfg = CFG, reps: int = 1):
    c.check()
    nc = bacc.Bacc(
        "TRN2",
        target_bir_lowering=False,
        debug=False,
        enable_asserts=False,
        num_devices=c.NC,
    )
    d = {}
    d["xt"] = nc.dram_tensor("xt", [c.E, c.T], F32, kind="ExternalInput").ap()
    d["wq"] = nc.dram_tensor("wq", [c.E, c.E], BF16, kind="ExternalInput").ap()
    d["wk"] = nc.dram_tensor("wk", [c.E, c.E], BF16, kind="ExternalInput").ap()
    d["wv"] = nc.dram_tensor("wv", [c.E, c.E], BF16, kind="ExternalInput").ap()
    d["wo"] = nc.dram_tensor("wo", [c.E, c.E], BF16, kind="ExternalInput").ap()
    d["w1"] = nc.dram_tensor("w1", [c.E, c.F], BF16, kind="ExternalInput").ap()
    d["w2"] = nc.dram_tensor("w2", [c.F, c.E], BF16, kind="ExternalInput").ap()
    for nm, cols in [
        ("ln1g", c.KE), ("ln1b", c.KE), ("ln2g", c.KE), ("ln2b", c.KE),
        ("boc", c.KE), ("mb1", c.KF), ("mb2", c.KE),
    ]:
        d[nm] = nc.dram_tensor(nm, [P, cols], F32, kind="ExternalInput").ap()
    d["colmask"] = nc.dram_tensor("colmask", [P, c.TK], F32, kind="ExternalInput").ap()
    d["onehot"] = nc.dram_tensor(
        "onehot", [128, c.HP * 128], BF16, kind="ExternalInput"
    ).ap()
    d["band01"] = nc.dram_tensor(
        "band01", [P, c.NB * 4 * c.Tq], BF16, kind="ExternalInput"
    ).ap()
    d["out_t"] = nc.dram_tensor("out_t", [c.E, c.Tq], F32, kind="ExternalOutput").ap()

    with tile.TileContext(nc) as tc:
        _emit(tc, c, d, reps=reps)
    nc.compile()
    return nc


# --------------------------------------------------------------------------
# host side
# --------------------------------------------------------------------------
def shard_inputs(inputs, c: Cfg = CFG):
    x = np.ascontiguousarray(np.asarray(inputs["x"], np.float32))
    bf = lambda a: np.ascontiguousarray(np.asarray(a, np.float32)).astype(NPBF16)


    chunks = lambda v, k: np.ascontiguousarray(
        np.asarray(v, np.float32).reshape(k, P).T
    )
    com = {
        "wq": bf(inputs["Wq"]),
        "wk": bf(inputs["Wk"]),
        "wv": bf(inputs["Wv"]),
        "wo": bf(inputs["Wo"]),
        "w1": bf(inputs["W1"]),
        "w2": bf(inputs["W2"]),
        "ln1g": chunks(inputs["ln1_g"], c.KE),
        "ln1b": chunks(inputs["ln1_b"], c.KE),
        "ln2g": chunks(inputs["ln2_g"], c.KE),
        "ln2b": chunks(inputs["ln2_b"], c.KE),
        "boc": chunks(inputs["bo"], c.KE),
        "mb1": chunks(inputs["b1"], c.KF),
        "mb2": chunks(inputs["b2"], c.KE),
    }

    p_idx = np.arange(P)[:, None]
    tq_idx = np.arange(c.Tq)[None, :]
    band = np.zeros((P, c.NB * 4 * c.Tq), np.float32)
    for jb in range(c.NB):
        m = (tq_idx >= (jb * P + p_idx)).astype(np.float32)
        for s in range(4):
            band[:, jb * 4 * c.Tq + s * c.Tq : jb * 4 * c.Tq + (s + 1) * c.Tq] = m
    com["band01"] = band.astype(NPBF16)
    oh = np.zeros((P, c.HP * P), np.float32)
    for j in range(c.HP):
        g, i = j // 2, j % 2
        oh[32 * g + 2 * i, j * P : j * P + 64] = 1.0
        oh[32 * g + 2 * i + 1, j * P + 64 : (j + 1) * P] = 1.0
    com["onehot"] = oh.astype(NPBF16)

    slot = np.arange(c.T)
    maps = []
    for core in range(c.NC):
        b, qi = core // c.CPB, core % c.CPB
        qoff = qi * c.Tq
        pad = c.T - qoff - c.Tq
        ctx = np.zeros((c.T, c.E), np.float32)
        ctx[pad:, :] = x[b, : qoff + c.Tq, :]
        colmask = np.ascontiguousarray(
            np.where(slot.reshape(c.TK, P).T < pad, NEG, 0.0).astype(np.float32)
        )
        m = dict(com)
        m["xt"] = np.ascontiguousarray(ctx.T)
        m["colmask"] = colmask
        maps.append(m)
    return maps


def assemble(results, c: Cfg = CFG):
    out = np.empty((c.B, c.T, c.E), np.float32)
    for core in range(c.NC):
        b, qi = core // c.CPB, core % c.CPB
        out[b, qi * c.Tq : (qi + 1) * c.Tq, :] = results[core]["out_t"].T
    return out


_NC_CACHE = {}


def _get_nc(c: Cfg = CFG):
    if c not in _NC_CACHE:
        _NC_CACHE[c] = build_program(c)
    return _NC_CACHE[c]


LAST_RESULT = None


def kernel(**inputs):
    global LAST_RESULT
    c = CFG
    nc = _get_nc(c)
    maps = shard_inputs(inputs, c)
    res = bass_utils.run_bass_kernel_spmd(nc, maps, core_ids=list(range(c.NC)))
    LAST_RESULT = res
    return assemble(res.results, c)



# revision 27
# speedup vs baseline: 1.5579x; 1.5579x over previous
"""Trainium2 Bass kernel for a pre-norm transformer block (dense_transformer).

Computation (per reference):
    x = x + Attn(LN1(x));  x = x + MLP(LN2(x))
with causal multi-head attention (H=16 heads, D=64) and a 4E ReLU MLP.

Sharding: 8 cores = 2 batches x 4 query PHASES.  Core (b, j) owns the 512
query tokens {4r + j}.  The context (all 2048 tokens) is column-PERMUTED
per core so the core's own phase sits last: position 512*i + r holds token
4r + phase_i with phase order (j+1, j+2, j+3, j) mod 4.  With this striping
the causal block structure is identical on every core (SPMD uniform): query
tile m attends context tiles t with t%4 <= m, so upper score tiles are
skipped for real FLOP savings; the diagonal band mask is a per-core input.

Precision: QKV and output projections run in fp8e4 DoubleRow (2x PE perf,
K=256 per instruction) with power-of-2 scales folded into the weights and
descale factors folded into existing copies; LN gains/biases are folded
into the weights host-side.  Scores/attnV/MLP stay bf16 (error budget).

attnV runs "query-major": out[q, d] with a ones column appended to V, so
softmax row-sums accumulate for free in PSUM column 64; normalization is a
per-partition scalar multiply, then a PE transpose returns to feature-major
for the fp8 out-projection.
"""

from dataclasses import dataclass

import numpy as np
import ml_dtypes

import concourse.bass as bass  # noqa: F401
import concourse.mybir as mybir
import concourse.tile as tile
from concourse import bacc
from concourse import bass_utils

F32 = mybir.dt.float32
BF16 = mybir.dt.bfloat16
F8 = mybir.dt.float8e4
AF = mybir.ActivationFunctionType
OP = mybir.AluOpType
DR = mybir.MatmulPerfMode.DoubleRow
NPBF16 = ml_dtypes.bfloat16
NPF8 = ml_dtypes.float8_e4m3

P = 128
SX = 32.0       # fp8 scale on activations
SW = 256.0      # fp8 scale on weights
DS = 1.0 / (SX * SW)


@dataclass(frozen=True)
class Cfg:
    B: int = 2
    T: int = 2048
    E: int = 1024
    H: int = 16
    D: int = 64
    NC: int = 8
    eps: float = 1e-5

    @property
    def CPB(self):
        return self.NC // self.B

    @property
    def Tq(self):
        return self.T // self.CPB

    @property
    def KE(self):
        return self.E // P

    @property
    def TK(self):
        return self.T // P

    @property
    def HP(self):
        return self.H // 2

    @property
    def F(self):
        return 4 * self.E

    @property
    def KF(self):
        return self.F // P

    @property
    def TCH(self):
        return 512

    @property
    def NQC(self):
        return self.T // self.TCH

    @property
    def NM(self):
        return self.Tq // P  # query tiles per core

    def check(self):
        assert self.D == 64 and self.E == self.H * self.D
        assert self.Tq == 512 and self.KE == 8 and self.TK == 16
        assert self.CPB == 4 and self.HP == 8 and self.KF == 32


CFG = Cfg()
DEBUG = False


class Pools:
    def __init__(self, tc, prefix=""):
        self.tc = tc
        self.prefix = prefix
        self.live = {}

    def open(self, key, bufs, space=None, side=None):
        kw = dict(name=self.prefix + key, bufs=bufs)
        if space:
            kw["space"] = space
        if side:
            kw["side"] = side
        cm = self.tc.tile_pool(**kw)
        pool = cm.__enter__()
        self.live[key] = cm
        return pool

    def close(self, *keys):
        for key in keys:
            self.live.pop(key).__exit__(None, None, None)

    def close_all(self):
        for key in reversed(list(self.live)):
            self.close(key)


def _emit(tc, c: Cfg, d):
    nc = tc.nc
    E, T, Tq, H, D = c.E, c.T, c.Tq, c.H, c.D
    KE, TK, HP, KF, NM = c.KE, c.TK, c.HP, c.KF, c.NM
    TCH, NQC = c.TCH, c.NQC
    SCL = 1.0 / float(np.sqrt(D))
    LN32 = float(np.log(SX))

    pp = Pools(tc)

    # ---------------- constants ----------------
    const = pp.open("const", 1)
    ones_bf = const.tile([P, 1], BF16, name="ones_bf")
    nc.vector.memset(ones_bf[:], 1.0)
    ones_f1 = const.tile([1, P], F32, name="ones_f1")
    nc.vector.memset(ones_f1[:], 1.0)
    ln32c = const.tile([P, 1], F32, name="ln32c")
    nc.vector.memset(ln32c[:], LN32)
    ident = const.tile([P, P], BF16, name="ident")
    nc.sync.dma_start(ident[:], d["ident"])
    band = const.tile([P, NQC, 2, P], BF16, name="band")
    nc.sync.dma_start(band[:], d["band"].rearrange("p (i s q) -> p i s q", i=NQC, s=2))
    gbt = {}
    for nm, cols in [("bq", KE), ("bk", KE), ("bv32", KE), ("boc", KE),
                     ("mb1", KF), ("mb2", KE)]:
        gbt[nm] = const.tile([P, cols], F32, name=nm + "_t")
        nc.sync.dma_start(gbt[nm][:], d[nm])

    # ---------------- weights (fp8, resident) ----------------
    p_w8 = pp.open("w8", 1)
    wk8 = p_w8.tile([P, KE, E], F8, name="wk8")
    wo8 = p_w8.tile([P, KE, E], F8, name="wo8")
    for nm, t_ in [("wk8", wk8), ("wo8", wo8)]:
        nc.sync.dma_start(t_[:], d[nm].rearrange("(e p) m -> p e m", p=P))

    # ---------------- warmup (PE p-state ramp; no DMA dependency) ----------
    p_wsb = pp.open("warm_sb", 1, side="right")
    ps_wm = pp.open("warm_ps", 1, "PSUM")
    wsb = p_wsb.tile([P, TCH], BF16, name="wsb")
    nc.vector.memset(wsb[:], 0.0)
    wmp = ps_wm.tile([1, TCH], F32, name="wmp")
    for _w in range(20):
        nc.tensor.matmul(wmp[:], ones_bf[:], wsb[:], start=True, stop=True)
    pp.close("warm_ps", "warm_sb")

    # ---------------- long-lived activations ----------------
    # left stack: const, w8, xqp | w8b, xtp, LN pools (freed) | w2p, phase3/4
    # right stack: aop | xnp, ktp, qtp, vsp (freed after attention), attn pools
    p_xq = pp.open("xqp", 1)
    xq = [p_xq.tile([P, Tq], F32, name=f"xq{e}") for e in range(KE)]
    p_ao = pp.open("aop", 1, side="right")
    aop8 = p_ao.tile([P, HP, Tq], F8, name="aop8")
    p_xn = pp.open("xnp", 1, side="right")
    xn8 = p_xn.tile([P, KE, T], F8, name="xn8")
    p_kt = pp.open("ktp", 1, side="right")
    kt = [p_kt.tile([P, T], BF16, name=f"kt{j}") for j in range(HP)]
    p_qt = pp.open("qtp", 1, side="right")
    qt = [p_qt.tile([P, Tq], BF16, name=f"qt{j}") for j in range(HP)]
    p_vs = pp.open("vsp", 1, side="right")
    vsb = [p_vs.tile([P, H * (D + 1)], BF16, name=f"vsb{t}") for t in range(TK)]

    # ======================================================================
    # Phase 1: x load + LN1 + QKV (chunk-interleaved)
    # ======================================================================
    p_w8b = pp.open("w8b", 1)
    wq8 = p_w8b.tile([P, KE, E], F8, name="wq8")
    wv8 = p_w8b.tile([P, KE, E], F8, name="wv8")
    for nm, t_ in [("wv8", wv8), ("wq8", wq8)]:
        nc.sync.dma_start(t_[:], d[nm].rearrange("(e p) m -> p e m", p=P))

    p_xt = pp.open("xtp", 1)
    xt = [p_xt.tile([P, T], BF16, name=f"xt{e}") for e in range(KE)]
    for e in range(KE):
        nc.sync.dma_start(xt[e][:], d["xt"][e * P : (e + 1) * P, :])

    # ones columns of V (col 64 of each head slot)
    for t in range(TK):
        nc.vector.memset(vsb[t][:, :].rearrange("p (h d) -> p h d", d=D + 1)[:, :, D], 1.0)

    p_tmp = pp.open("ln_tmp", 3)
    p_rows = pp.open("ln_rows", 1)
    ps_st = pp.open("ln_st", 1, "PSUM")
    ps_bc = pp.open("ln_bc", 1, "PSUM")
    ps_qkv = pp.open("qkv_ps", 2, "PSUM")

    def k_proj(j, psum_pool, cis):
        for ci in cis:
            cs = slice(ci * TCH, (ci + 1) * TCH)
            ps = psum_pool.tile([P, TCH], F32, name="qkv")
            for g in range(KE // 2):
                nc.tensor.matmul(
                    ps[:], wk8[:, 2 * g : 2 * g + 2, j * P : (j + 1) * P],
                    xn8[:, 2 * g : 2 * g + 2, cs],
                    start=(g == 0), stop=(g == KE // 2 - 1), perf_mode=DR,
                )
            nc.vector.tensor_scalar(
                kt[j][:, cs], ps[:], DS, gbt["bk"][:, j : j + 1], OP.mult, OP.add
            )

    def v_proj_chunk(ci):
        for tt in range(4 * ci, 4 * ci + 4):
            for hf in range(2):
                ps = ps_qkv.tile([P, KE, D], F32, name="qkv")
                for g in range(KE // 2):
                    nc.tensor.matmul(
                        ps[:], xn8[:, 2 * g : 2 * g + 2, tt * P : (tt + 1) * P],
                        wv8[:, 2 * g : 2 * g + 2, hf * 512 : hf * 512 + 512],
                        start=(g == 0), stop=(g == KE // 2 - 1), perf_mode=DR,
                    )
                dst = vsb[tt][:, hf * 8 * (D + 1) :].rearrange(
                    "p (h d) -> p h d", d=D + 1
                )[:, 0:8, 0:D]
                nc.scalar.activation(dst, ps[:], AF.Copy, scale=DS)

    def q_proj(j):
        ps = ps_qkv.tile([P, Tq], F32, name="qkv")
        for g in range(KE // 2):
            nc.tensor.matmul(
                ps[:], wq8[:, 2 * g : 2 * g + 2, j * P : (j + 1) * P],
                xn8[:, 2 * g : 2 * g + 2, T - Tq :],
                start=(g == 0), stop=(g == KE // 2 - 1), perf_mode=DR,
            )
        nc.vector.tensor_scalar(
            qt[j][:], ps[:], DS, gbt["bq"][:, j : j + 1], OP.mult, OP.add
        )

    for ci in range(NQC):
        cs = slice(ci * TCH, (ci + 1) * TCH)
        s1 = ps_st.tile([1, TCH], F32, name="s1")
        s2 = ps_st.tile([1, TCH], F32, name="s2")
        for e in range(KE):
            x2 = p_tmp.tile([P, TCH], BF16, name="x2bf")
            nc.scalar.square(x2[:], xt[e][:, cs])
            nc.tensor.matmul(s1[:], ones_bf[:], xt[e][:, cs],
                             start=(e == 0), stop=(e == KE - 1))
            nc.tensor.matmul(s2[:], ones_bf[:], x2[:],
                             start=(e == 0), stop=(e == KE - 1))
        mu = p_rows.tile([1, TCH], F32, name="mu")
        nc.vector.tensor_scalar_mul(mu[:], s1[:], 1.0 / E)
        ve = p_rows.tile([1, TCH], F32, name="ve")
        nc.vector.tensor_scalar(ve[:], s2[:], 1.0 / E, c.eps, OP.mult, OP.add)
        mu2 = p_rows.tile([1, TCH], F32, name="mu2")
        nc.vector.tensor_tensor(mu2[:], mu[:], mu[:], OP.mult)
        vee = p_rows.tile([1, TCH], F32, name="vee")
        nc.vector.tensor_tensor(vee[:], ve[:], mu2[:], OP.subtract)
        lv = p_rows.tile([1, TCH], F32, name="lv")
        nc.scalar.activation(lv[:], vee[:], AF.Ln)
        rstd32 = p_rows.tile([1, TCH], F32, name="rstd32")
        nc.scalar.activation(rstd32[:], lv[:], AF.Exp, scale=-0.5,
                             bias=ln32c[0:1, 0:1])

        mub = ps_bc.tile([P, TCH], F32, name="mub")
        nc.tensor.matmul(mub[:], ones_f1[:], mu[:], start=True, stop=True)
        rsb = ps_bc.tile([P, TCH], F32, name="rsb")
        nc.tensor.matmul(rsb[:], ones_f1[:], rstd32[:], start=True, stop=True)

        for e in range(KE):
            t1 = p_tmp.tile([P, TCH], F32, name="t1")
            nc.vector.tensor_tensor(t1[:], xt[e][:, cs], mub[:], OP.subtract)
            nc.vector.tensor_tensor(xn8[:, e, cs], t1[:], rsb[:], OP.mult)

        # residual extraction for the query chunk (last chunk): xq = x + bo
        if ci == NQC - 1:
            for e in range(KE):
                nc.vector.tensor_scalar(
                    xq[e][:], xt[e][:, T - Tq :], gbt["boc"][:, e : e + 1],
                    None, OP.add,
                )

        # interleaved QKV for this chunk
        k_proj(0, ps_qkv, [ci])
        k_proj(1, ps_qkv, [ci])
        v_proj_chunk(ci)
        if ci == NQC - 1:
            for j in range(HP):
                q_proj(j)

    pp.close("qkv_ps", "ln_bc", "ln_st", "ln_rows", "ln_tmp", "xtp", "w8b")

    if DEBUG:
        nc.sync.dma_start(d["dbg_xn"], xn8[:])
        for j in range(HP):
            nc.sync.dma_start(d["dbg_qt"][:, j * Tq : (j + 1) * Tq], qt[j][:])
        for t in range(TK):
            nc.sync.dma_start(
                d["dbg_vs"][:, t * 1040 : (t + 1) * 1040], vsb[t][:]
            )

    # w2 resident load (xt freed now; lands during attention)
    p_w2 = pp.open("w2p", 1)
    w2t = p_w2.tile([P, KF, E], BF16, name="w2t")
    for fq in range(4):
        nc.sync.dma_start(
            w2t[:, 8 * fq : 8 * fq + 8, :],
            d["w2"].rearrange("(f p) m -> p f m", p=P)[:, 8 * fq : 8 * fq + 8, :],
        )

    # ======================================================================
    # Phase 2: attention, head-pair at a time, query-major attnV
    # ======================================================================
    ss_p = pp.open("ss_ps", 2, "PSUM")
    oh_p = pp.open("oh_ps", 1, "PSUM")
    kv2_p = pp.open("kv2_ps", 1, "PSUM")
    tp_p = pp.open("tp_ps", 1, "PSUM")
    p_pr = pp.open("prp", 2, side="right")
    p_tail = pp.open("tailp", 2, side="right")

    for j in range(HP):
        ohq = oh_p.tile([P, 2, 512], F32, name="ohq")  # per s: 4*65 used
        for t in range(TK):
            u = t % 4
            i = t // 4
            ss = ss_p.tile([P, 2, Tq], F32, name="ss")
            for s in (0, 1):
                nc.tensor.matmul(
                    ss[:, s, u * P : Tq],
                    kt[j][s * 64 : (s + 1) * 64, t * P : (t + 1) * P],
                    qt[j][s * 64 : (s + 1) * 64, u * P : Tq],
                    start=True, stop=True, tile_position=(s * 64, 0),
                )
            pr = p_pr.tile([P, 2, Tq], BF16, name="pr")
            nc.scalar.activation(pr[:, :, u * P : Tq], ss[:, :, u * P : Tq],
                                 AF.Exp, scale=SCL)
            nc.vector.tensor_tensor(
                pr[:, :, u * P : (u + 1) * P], pr[:, :, u * P : (u + 1) * P],
                band[:, i, :, :], OP.mult,
            )
            for m in range(u, NM):
                for s in (0, 1):
                    h = 2 * j + s
                    # start=True resets the whole PSUM bank: only the first
                    # matmul into each bank (m==0 at t==0) may set it; later
                    # groups accumulate onto the bank-wide zero.
                    nc.tensor.matmul(
                        ohq[:, s, m * 65 : m * 65 + 65],
                        pr[:, s, m * P : (m + 1) * P],
                        vsb[t][:, h * (D + 1) : (h + 1) * (D + 1)],
                        start=(t == 0 and m == 0), stop=(t == 12 + m),
                        skip_group_check=True,
                    )
        # tail: row-sums -> 32/rs -> normalize -> transpose -> fp8 quantize
        rs_sb = p_tail.tile([P, 2, NM], F32, name="rs_sb")
        for s in (0, 1):
            for m in range(NM):
                nc.vector.tensor_copy(rs_sb[:, s, m : m + 1],
                                      ohq[:, s, m * 65 + 64 : m * 65 + 65])
        lrs = p_tail.tile([P, 2, NM], F32, name="lrs")
        nc.scalar.activation(lrs[:], rs_sb[:], AF.Ln)
        irs = p_tail.tile([P, 2, NM], F32, name="irs")
        nc.scalar.activation(irs[:], lrs[:], AF.Exp, scale=-1.0,
                             bias=ln32c[:, 0:1])
        ptb = p_tail.tile([P, 2, 256], BF16, name="ptb")
        for s in (0, 1):
            for m in range(NM):
                nc.vector.tensor_scalar_mul(
                    ptb[:, s, m * 64 : (m + 1) * 64],
                    ohq[:, s, m * 65 : m * 65 + 64],
                    irs[:, s, m : m + 1],
                )
        for m in range(NM):
            tp = tp_p.tile([P, P], BF16, name="tp")
            nc.tensor.transpose(tp[0:64, :], ptb[:, 0, m * 64 : (m + 1) * 64],
                                ident[:], tile_position=(0, 0))
            nc.tensor.transpose(tp[64:128, :], ptb[:, 1, m * 64 : (m + 1) * 64],
                                ident[:], tile_position=(0, 64))
            nc.vector.tensor_scalar(
                aop8[:, j, m * P : (m + 1) * P], tp[:], 1.0,
                gbt["bv32"][:, j : j + 1], OP.mult, OP.add,
            )
        if DEBUG:
            nc.sync.dma_start(d["dbg_rs"][:, j, :, :], rs_sb[:])
            nc.sync.dma_start(d["dbg_irs"][:, j, :, :], irs[:])
            nc.sync.dma_start(d["dbg_pt"][:, j, :, :], ptb[:])
        if j + 2 < HP:
            k_proj(j + 2, kv2_p, range(NQC))

    if DEBUG:
        nc.sync.dma_start(d["dbg_ao"], aop8[:])
        for j in range(HP):
            nc.sync.dma_start(d["dbg_kt"][:, j * T : (j + 1) * T], kt[j][:])
    pp.close("tailp", "prp", "tp_ps", "kv2_ps", "oh_ps", "ss_ps")
    pp.close("vsp", "qtp", "ktp", "xnp")

    # ======================================================================
    # Phase 3: out-projection (fp8) + residual; LayerNorm2
    # ======================================================================
    p_xr = pp.open("xrp", 1)
    p_x2 = pp.open("xn2p", 1)
    ps_ao = pp.open("ao_ps", 2, "PSUM")
    xres = [p_xr.tile([P, Tq], F32, name=f"xres{e}") for e in range(KE)]
    xn2 = [p_x2.tile([P, Tq], BF16, name=f"xn2{e}") for e in range(KE)]

    for e in range(KE):
        ps = ps_ao.tile([P, Tq], F32, name="aops")
        for g in range(KE // 2):
            nc.tensor.matmul(
                ps[:], wo8[:, 2 * g : 2 * g + 2, e * P : (e + 1) * P],
                aop8[:, 2 * g : 2 * g + 2, :],
                start=(g == 0), stop=(g == KE // 2 - 1), perf_mode=DR,
            )
        nc.vector.scalar_tensor_tensor(
            xres[e][:], ps[:], DS, xq[e][:], OP.mult, OP.add
        )
    pp.close("ao_ps", "aop")

    p_tmp = pp.open("ln2_tmp", 3)
    p_rows = pp.open("ln2_rows", 1)
    ps_st = pp.open("ln2_st", 1, "PSUM")
    ps_bc = pp.open("ln2_bc", 1, "PSUM")
    s1 = ps_st.tile([1, Tq], F32, name="s1b")
    s2 = ps_st.tile([1, Tq], F32, name="s2b")
    for e in range(KE):
        xbf = p_tmp.tile([P, Tq], BF16, name="xbf2")
        nc.vector.tensor_copy(xbf[:], xres[e][:])
        x2 = p_tmp.tile([P, Tq], BF16, name="x2bf2")
        nc.scalar.square(x2[:], xres[e][:])
        nc.tensor.matmul(s1[:], ones_bf[:], xbf[:], start=(e == 0), stop=(e == KE - 1))
        nc.tensor.matmul(s2[:], ones_bf[:], x2[:], start=(e == 0), stop=(e == KE - 1))
    mu = p_rows.tile([1, Tq], F32, name="mu_2")
    nc.vector.tensor_scalar_mul(mu[:], s1[:], 1.0 / E)
    ve = p_rows.tile([1, Tq], F32, name="ve_2")
    nc.vector.tensor_scalar(ve[:], s2[:], 1.0 / E, c.eps, OP.mult, OP.add)
    mu2 = p_rows.tile([1, Tq], F32, name="mu2_2")
    nc.vector.tensor_tensor(mu2[:], mu[:], mu[:], OP.mult)
    vee = p_rows.tile([1, Tq], F32, name="vee_2")
    nc.vector.tensor_tensor(vee[:], ve[:], mu2[:], OP.subtract)
    lv = p_rows.tile([1, Tq], F32, name="lv_2")
    nc.scalar.activation(lv[:], vee[:], AF.Ln)
    rstd = p_rows.tile([1, Tq], F32, name="rstd_2")
    nc.scalar.activation(rstd[:], lv[:], AF.Exp, scale=-0.5)
    mub = ps_bc.tile([P, Tq], F32, name="mub2")
    nc.tensor.matmul(mub[:], ones_f1[:], mu[:], start=True, stop=True)
    rsb = ps_bc.tile([P, Tq], F32, name="rsb2")
    nc.tensor.matmul(rsb[:], ones_f1[:], rstd[:], start=True, stop=True)
    for e in range(KE):
        t1 = p_tmp.tile([P, Tq], F32, name="t1b")
        nc.vector.tensor_tensor(t1[:], xres[e][:], mub[:], OP.subtract)
        nc.vector.tensor_tensor(xn2[e][:], t1[:], rsb[:], OP.mult)
    pp.close("ln2_rows", "ln2_tmp", "ln2_bc", "ln2_st")

    # ======================================================================
    # Phase 4: MLP (bf16), W2 resident
    # ======================================================================
    EH = 6
    p_h1 = pp.open("h1p", 1, side="right")
    p_out = pp.open("outp", 2)
    p_w1 = pp.open("w1s", 3)
    ps_h1 = pp.open("h1_ps", 2, "PSUM")
    ps_h2a = pp.open("h2a_ps", 1, "PSUM")

    h1 = [p_h1.tile([P, Tq], BF16, name=f"h1{f}") for f in range(KF)]
    h2a = [ps_h2a.tile([P, Tq], F32, name=f"h2a{e}") for e in range(EH)]
    for f in range(KF):
        w1f = p_w1.tile([P, KE, P], BF16, name="w1cb")
        nc.sync.dma_start(
            w1f[:],
            d["w1"].rearrange("(e p) m -> p e m", p=P)[:, :, f * P : (f + 1) * P],
        )
        ps = ps_h1.tile([P, Tq], F32, name="h1ps")
        for e in range(KE):
            nc.tensor.matmul(
                ps[:], w1f[:, e, :], xn2[e][:], start=(e == 0), stop=(e == KE - 1)
            )
        nc.scalar.activation(
            h1[f][:], ps[:], AF.Relu, bias=gbt["mb1"][:, f : f + 1], scale=1.0
        )
        for e in range(EH):
            nc.tensor.matmul(
                h2a[e][:], w2t[:, f, e * P : (e + 1) * P], h1[f][:],
                start=(f == 0), stop=(f == KF - 1),
            )
    for e in range(EH):
        of = p_out.tile([P, Tq], F32, name="outf")
        nc.vector.scalar_tensor_tensor(
            of[:], h2a[e][:], gbt["mb2"][:, e : e + 1], xres[e][:], OP.add, OP.add
        )
        nc.sync.dma_start(d["out_t"][e * P : (e + 1) * P, :], of[:])
    pp.close("h2a_ps", "h1_ps")

    ps_h2b = pp.open("h2b_ps", 1, "PSUM")
    h2b = [ps_h2b.tile([P, Tq], F32, name=f"h2b{i}") for i in range(KE - EH)]
    for f in range(KF):
        for i, e in enumerate(range(EH, KE)):
            nc.tensor.matmul(
                h2b[i][:], w2t[:, f, e * P : (e + 1) * P], h1[f][:],
                start=(f == 0), stop=(f == KF - 1),
            )
    for i, e in enumerate(range(EH, KE)):
        of = p_out.tile([P, Tq], F32, name="outf")
        nc.vector.scalar_tensor_tensor(
            of[:], h2b[i][:], gbt["mb2"][:, e : e + 1], xres[e][:], OP.add, OP.add
        )
        nc.sync.dma_start(d["out_t"][e * P : (e + 1) * P, :], of[:])

    pp.close_all()


def build_program(c: Cfg = CFG):
    c.check()
    nc = bacc.Bacc(
        "TRN2",
        target_bir_lowering=False,
        debug=False,
        enable_asserts=False,
        num_devices=c.NC,
    )
    d = {}
    d["xt"] = nc.dram_tensor("xt", [c.E, c.T], BF16, kind="ExternalInput").ap()
    for nm in ("wq8", "wk8", "wv8", "wo8"):
        d[nm] = nc.dram_tensor(nm, [c.E, c.E], F8, kind="ExternalInput").ap()
    d["w1"] = nc.dram_tensor("w1", [c.E, c.F], BF16, kind="ExternalInput").ap()
    d["w2"] = nc.dram_tensor("w2", [c.F, c.E], BF16, kind="ExternalInput").ap()
    for nm, cols in [("bq", c.KE), ("bk", c.KE), ("bv32", c.KE), ("boc", c.KE),
                     ("mb1", c.KF), ("mb2", c.KE)]:
        d[nm] = nc.dram_tensor(nm, [P, cols], F32, kind="ExternalInput").ap()
    d["band"] = nc.dram_tensor("band", [P, c.NQC * 2 * P], BF16,
                               kind="ExternalInput").ap()
    d["ident"] = nc.dram_tensor("ident", [P, P], BF16, kind="ExternalInput").ap()
    d["out_t"] = nc.dram_tensor("out_t", [c.E, c.Tq], F32, kind="ExternalOutput").ap()
    if DEBUG:
        d["dbg_xn"] = nc.dram_tensor("dbg_xn", [P, c.KE, c.T], F8,
                                     kind="ExternalOutput").ap()
        d["dbg_qt"] = nc.dram_tensor("dbg_qt", [P, c.HP * c.Tq], BF16,
                                     kind="ExternalOutput").ap()
        d["dbg_kt"] = nc.dram_tensor("dbg_kt", [P, c.HP * c.T], BF16,
                                     kind="ExternalOutput").ap()
        d["dbg_vs"] = nc.dram_tensor("dbg_vs", [P, c.TK * 1040], BF16,
                                     kind="ExternalOutput").ap()
        d["dbg_ao"] = nc.dram_tensor("dbg_ao", [P, c.HP, c.Tq], F8,
                                     kind="ExternalOutput").ap()
        d["dbg_rs"] = nc.dram_tensor("dbg_rs", [P, c.HP, 2, 4], F32,
                                     kind="ExternalOutput").ap()
        d["dbg_irs"] = nc.dram_tensor("dbg_irs", [P, c.HP, 2, 4], F32,
                                      kind="ExternalOutput").ap()
        d["dbg_pt"] = nc.dram_tensor("dbg_pt", [P, c.HP, 2, 256], BF16,
                                     kind="ExternalOutput").ap()

    with tile.TileContext(nc) as tc:
        _emit(tc, c, d)
    nc.compile()
    return nc


# --------------------------------------------------------------------------
# host side
# --------------------------------------------------------------------------
def shard_inputs(inputs, c: Cfg = CFG):
    f32 = lambda a: np.ascontiguousarray(np.asarray(a, np.float32))
    x = f32(inputs["x"])
    g1, b1n = f32(inputs["ln1_g"]), f32(inputs["ln1_b"])
    g2, b2n = f32(inputs["ln2_g"]), f32(inputs["ln2_b"])
    Wq, Wk, Wv, Wo = (f32(inputs[k]) for k in ("Wq", "Wk", "Wv", "Wo"))
    W1, W2 = f32(inputs["W1"]), f32(inputs["W2"])
    bo, b1, b2 = f32(inputs["bo"]), f32(inputs["b1"]), f32(inputs["b2"])

    q8 = lambda w: np.ascontiguousarray((w * SW)).astype(NPF8)
    bf = lambda w: np.ascontiguousarray(w).astype(NPBF16)
    chunks = lambda v, k: np.ascontiguousarray(v.reshape(k, P).T)

    com = {
        "wq8": q8(g1[:, None] * Wq),
        "wk8": q8(g1[:, None] * Wk),
        "wv8": q8(g1[:, None] * Wv),
        "wo8": q8(Wo),
        "w1": bf(g2[:, None] * W1),
        "w2": bf(W2),
        "bq": chunks(b1n @ Wq, c.KE),
        "bk": chunks(b1n @ Wk, c.KE),
        "bv32": chunks((b1n @ Wv) * SX, c.KE),
        "boc": chunks(bo, c.KE),
        "mb1": chunks(b1 + b2n @ W1, c.KF),
        "mb2": chunks(b2, c.KE),
        "ident": np.eye(P, dtype=np.float32).astype(NPBF16),
    }

    p_idx = np.arange(P)[:, None]
    lq_idx = np.arange(P)[None, :]
    maps = []
    for core in range(c.NC):
        b, j = core // c.CPB, core % c.CPB
        phases = [(j + 1) % 4, (j + 2) % 4, (j + 3) % 4, j]
        ctx = np.empty((c.E, c.T), np.float32)
        for i, ph in enumerate(phases):
            ctx[:, 512 * i : 512 * (i + 1)] = x[b, ph::4, :].T
        band = np.zeros((P, c.NQC, 2, P), np.float32)
        for i, ph in enumerate(phases):
            delta = 1 if ph > j else 0
            m_ = (p_idx <= lq_idx - delta).astype(np.float32)
            band[:, i, 0, :] = m_
            band[:, i, 1, :] = m_
        m = dict(com)
        m["xt"] = bf(ctx)
        m["band"] = band.reshape(P, c.NQC * 2 * P).astype(NPBF16)
        maps.append(m)
    return maps


def assemble(results, c: Cfg = CFG):
    out = np.empty((c.B, c.T, c.E), np.float32)
    for core in range(c.NC):
        b, j = core // c.CPB, core % c.CPB
        out[b, j::4, :] = results[core]["out_t"].T
    return out


_NC_CACHE = {}


def _get_nc(c: Cfg = CFG):
    if c not in _NC_CACHE:
        _NC_CACHE[c] = build_program(c)
    return _NC_CACHE[c]


LAST_RESULT = None


def kernel(**inputs):
    global LAST_RESULT
    c = CFG
    nc = _get_nc(c)
    maps = shard_inputs(inputs, c)
    res = bass_utils.run_bass_kernel_spmd(nc, maps, core_ids=list(range(c.NC)))
    LAST_RESULT = res
    return assemble(res.results, c)


# revision 39
# speedup vs baseline: 1.5938x; 1.0230x over previous
"""Trainium2 Bass kernel for a pre-norm transformer block (dense_transformer).

Computation (per reference):
    x = x + Attn(LN1(x));  x = x + MLP(LN2(x))
with causal multi-head attention (H=16 heads, D=64) and a 4E ReLU MLP.

Sharding: 8 cores = 2 batches x 4 query PHASES.  Core (b, j) owns the 512
query tokens {4r + j}.  The context (all 2048 tokens) is column-PERMUTED
per core so the core's own phase sits last: position 512*i + r holds token
4r + phase_i with phase order (j+1, j+2, j+3, j) mod 4.  With this striping
the causal block structure is identical on every core (SPMD uniform): query
tile m attends context tiles t with t%4 <= m, so upper score tiles are
skipped for real FLOP savings; the diagonal band mask is a per-core input.

Precision: QKV and output projections run in fp8e4 DoubleRow (2x PE perf,
K=256 per instruction) with power-of-2 scales folded into the weights and
descale factors folded into existing copies; LN gains/biases are folded
into the weights host-side.  Scores/attnV/MLP stay bf16 (error budget).

attnV runs "query-major": out[q, d] with a ones column appended to V, so
softmax row-sums accumulate for free in PSUM column 64; normalization is a
per-partition scalar multiply, then a PE transpose returns to feature-major
for the fp8 out-projection.
"""

from dataclasses import dataclass

import numpy as np
import ml_dtypes

import concourse.bass as bass  # noqa: F401
import concourse.mybir as mybir
import concourse.tile as tile
from concourse import bacc
from concourse import bass_utils

F32 = mybir.dt.float32
BF16 = mybir.dt.bfloat16
F8 = mybir.dt.float8e4
AF = mybir.ActivationFunctionType
OP = mybir.AluOpType
DR = mybir.MatmulPerfMode.DoubleRow
NPBF16 = ml_dtypes.bfloat16
NPF8 = ml_dtypes.float8_e4m3

P = 128
SX = 32.0       # fp8 scale on activations
SW = 256.0      # fp8 scale on weights
DS = 1.0 / (SX * SW)


@dataclass(frozen=True)
class Cfg:
    B: int = 2
    T: int = 2048
    E: int = 1024
    H: int = 16
    D: int = 64
    NC: int = 8
    eps: float = 1e-5

    @property
    def CPB(self):
        return self.NC // self.B

    @property
    def Tq(self):
        return self.T // self.CPB

    @property
    def KE(self):
        return self.E // P

    @property
    def TK(self):
        return self.T // P

    @property
    def HP(self):
        return self.H // 2

    @property
    def F(self):
        return 4 * self.E

    @property
    def KF(self):
        return self.F // P

    @property
    def TCH(self):
        return 512

    @property
    def NQC(self):
        return self.T // self.TCH

    @property
    def NM(self):
        return self.Tq // P  # query tiles per core

    def check(self):
        assert self.D == 64 and self.E == self.H * self.D
        assert self.Tq == 512 and self.KE == 8 and self.TK == 16
        assert self.CPB == 4 and self.HP == 8 and self.KF == 32


CFG = Cfg()
DEBUG = False


class Pools:
    def __init__(self, tc, prefix=""):
        self.tc = tc
        self.prefix = prefix
        self.live = {}

    def open(self, key, bufs, space=None, side=None):
        kw = dict(name=self.prefix + key, bufs=bufs)
        if space:
            kw["space"] = space
        if side:
            kw["side"] = side
        cm = self.tc.tile_pool(**kw)
        pool = cm.__enter__()
        self.live[key] = cm
        return pool

    def close(self, *keys):
        for key in keys:
            self.live.pop(key).__exit__(None, None, None)

    def close_all(self):
        for key in reversed(list(self.live)):
            self.close(key)


def _emit(tc, c: Cfg, d):
    nc = tc.nc
    E, T, Tq, H, D = c.E, c.T, c.Tq, c.H, c.D
    KE, TK, HP, KF, NM = c.KE, c.TK, c.HP, c.KF, c.NM
    TCH, NQC = c.TCH, c.NQC
    SCL = 1.0 / float(np.sqrt(D))

    pp = Pools(tc)

    # ---------------- constants ----------------
    const = pp.open("const", 1)
    ones_bf = const.tile([P, 1], BF16, name="ones_bf")
    nc.vector.memset(ones_bf[:], 1.0)
    ones_f1 = const.tile([1, P], F32, name="ones_f1")
    nc.vector.memset(ones_f1[:], 1.0)
    ident = const.tile([P, P], BF16, name="ident")
    nc.sync.dma_start(ident[:], d["ident"])
    band = const.tile([P, NQC, 2, P], BF16, name="band")
    nc.sync.dma_start(band[:], d["band"].rearrange("p (i s q) -> p i s q", i=NQC, s=2))
    gbt = {}
    for nm, cols in [("bq", KE), ("bk", KE), ("bv32", KE), ("boc", KE),
                     ("mb1", KF), ("mb2", KE)]:
        gbt[nm] = const.tile([P, cols], F32, name=nm + "_t")
        nc.sync.dma_start(gbt[nm][:], d[nm])

    # ---------------- weights (fp8, resident) ----------------
    p_w8 = pp.open("w8", 1)
    wk8 = p_w8.tile([P, KE, E], F8, name="wk8")
    wo8 = p_w8.tile([P, KE, E], F8, name="wo8")
    for nm, t_ in [("wk8", wk8), ("wo8", wo8)]:
        nc.sync.dma_start(t_[:], d[nm].rearrange("(e p) m -> p e m", p=P))

    # ---------------- warmup (PE p-state ramp; no DMA dependency) ----------
    p_wsb = pp.open("warm_sb", 1, side="right")
    ps_wm = pp.open("warm_ps", 1, "PSUM")
    wsb = p_wsb.tile([P, TCH], BF16, name="wsb")
    nc.vector.memset(wsb[:], 0.0)
    wmp = ps_wm.tile([1, TCH], F32, name="wmp")
    for _w in range(20):
        nc.tensor.matmul(wmp[:], ones_bf[:], wsb[:], start=True, stop=True)
    pp.close("warm_ps", "warm_sb")

    # ---------------- long-lived activations ----------------
    # left stack: const, w8, xqp | w8b, xtp, LN pools (freed) | w2p, phase3/4
    # right stack: aop | xnp, ktp, qtp, vsp (freed after attention), attn pools
    p_xq = pp.open("xqp", 1)
    xq = [p_xq.tile([P, Tq], F32, name=f"xq{e}") for e in range(KE)]
    p_ao = pp.open("aop", 1, side="right")
    aop8 = p_ao.tile([P, HP, Tq], F8, name="aop8")
    p_xn = pp.open("xnp", 1, side="right")
    xn8 = p_xn.tile([P, KE, T], F8, name="xn8")
    p_kt = pp.open("ktp", 1, side="right")
    kt = [p_kt.tile([P, T], BF16, name=f"kt{j}") for j in range(HP)]
    p_qt = pp.open("qtp", 1, side="right")
    qt = [p_qt.tile([P, Tq], BF16, name=f"qt{j}") for j in range(HP)]
    p_vs = pp.open("vsp", 1, side="right")
    vsb = [p_vs.tile([P, H * (D + 1)], BF16, name=f"vsb{t}") for t in range(TK)]

    # ======================================================================
    # Phase 1: x load + LN1 + QKV (chunk-interleaved)
    # ======================================================================
    p_w8b = pp.open("w8b", 1)
    wq8 = p_w8b.tile([P, KE, E], F8, name="wq8")
    wv8 = p_w8b.tile([P, KE, E], F8, name="wv8")
    for nm, t_ in [("wv8", wv8), ("wq8", wq8)]:
        nc.sync.dma_start(t_[:], d[nm].rearrange("(e p) m -> p e m", p=P))

    p_xt = pp.open("xtp", 1)
    xt = [p_xt.tile([P, T], BF16, name=f"xt{e}") for e in range(KE)]
    for e in range(KE):
        nc.sync.dma_start(xt[e][:], d["xt"][e * P : (e + 1) * P, :])

    # ones columns of V (col 64 of each head slot)
    for t in range(TK):
        nc.vector.memset(vsb[t][:, :].rearrange("p (h d) -> p h d", d=D + 1)[:, :, D], 1.0)

    p_tmp = pp.open("ln_tmp", 3)
    p_rows = pp.open("ln_rows", 1)
    ps_st = pp.open("ln_st", 1, "PSUM")
    ps_bc = pp.open("ln_bc", 1, "PSUM")
    ps_qkv = pp.open("qkv_ps", 2, "PSUM")

    def k_proj(j, psum_pool, cis, nm="qkv", shp=None):
        for ci in cis:
            cs = slice(ci * TCH, (ci + 1) * TCH)
            pst = psum_pool.tile(shp or [P, TCH], F32, name=nm)
            ps = pst[:, 0, :] if shp else pst[:]
            for g in range(KE // 2):
                nc.tensor.matmul(
                    ps, wk8[:, 2 * g : 2 * g + 2, j * P : (j + 1) * P],
                    xn8[:, 2 * g : 2 * g + 2, cs],
                    start=(g == 0), stop=(g == KE // 2 - 1), perf_mode=DR,
                )
            nc.scalar.activation(
                kt[j][:, cs], ps, AF.Identity, bias=gbt["bk"][:, j : j + 1],
                scale=DS,
            )

    def v_proj_chunk(ci):
        for tt in range(4 * ci, 4 * ci + 4):
            for hf in range(2):
                ps = ps_qkv.tile([P, KE, D], F32, name="qkv")
                for g in range(KE // 2):
                    nc.tensor.matmul(
                        ps[:], xn8[:, 2 * g : 2 * g + 2, tt * P : (tt + 1) * P],
                        wv8[:, 2 * g : 2 * g + 2, hf * 512 : hf * 512 + 512],
                        start=(g == 0), stop=(g == KE // 2 - 1), perf_mode=DR,
                    )
                dst = vsb[tt][:, hf * 8 * (D + 1) :].rearrange(
                    "p (h d) -> p h d", d=D + 1
                )[:, 0:8, 0:D]
                nc.vector.tensor_scalar_mul(dst, ps[:], DS)

    def q_proj(j):
        ps = ps_qkv.tile([P, Tq], F32, name="qkv")
        for g in range(KE // 2):
            nc.tensor.matmul(
                ps[:], wq8[:, 2 * g : 2 * g + 2, j * P : (j + 1) * P],
                xn8[:, 2 * g : 2 * g + 2, T - Tq :],
                start=(g == 0), stop=(g == KE // 2 - 1), perf_mode=DR,
            )
        nc.vector.tensor_scalar(
            qt[j][:], ps[:], DS, gbt["bq"][:, j : j + 1], OP.mult, OP.add
        )

    for ci in range(NQC):
        cs = slice(ci * TCH, (ci + 1) * TCH)
        s1 = ps_st.tile([1, TCH], F32, name="s1")
        s2 = ps_st.tile([1, TCH], F32, name="s2")
        for e in range(KE):
            x2 = p_tmp.tile([P, TCH], BF16, name="x2bf")
            nc.scalar.square(x2[:], xt[e][:, cs])
            nc.tensor.matmul(s1[:], ones_bf[:], xt[e][:, cs],
                             start=(e == 0), stop=(e == KE - 1))
            nc.tensor.matmul(s2[:], ones_bf[:], x2[:],
                             start=(e == 0), stop=(e == KE - 1))
        mu = p_rows.tile([1, TCH], F32, name="mu")
        nc.vector.tensor_scalar_mul(mu[:], s1[:], 1.0 / E)
        ve = p_rows.tile([1, TCH], F32, name="ve")
        nc.vector.tensor_scalar(ve[:], s2[:], 1.0 / E, c.eps, OP.mult, OP.add)
        mu2 = p_rows.tile([1, TCH], F32, name="mu2")
        nc.vector.tensor_tensor(mu2[:], mu[:], mu[:], OP.mult)
        vee = p_rows.tile([1, TCH], F32, name="vee")
        nc.vector.tensor_tensor(vee[:], ve[:], mu2[:], OP.subtract)
        # rstd*SX: sqrt(vee/SX^2) on ACT (stays in the sqrt table set),
        # then fast reciprocal on DVE -> SX/sqrt(vee)
        sq = p_rows.tile([1, TCH], F32, name="sq")
        nc.scalar.activation(sq[:], vee[:], AF.Sqrt, scale=1.0 / (SX * SX))
        rstd32 = p_rows.tile([1, TCH], F32, name="rstd32")
        nc.vector.reciprocal_approx_fast(rstd32[:], sq[:])

        mub = ps_bc.tile([P, TCH], F32, name="mub")
        nc.tensor.matmul(mub[:], ones_f1[:], mu[:], start=True, stop=True)
        rsb = ps_bc.tile([P, TCH], F32, name="rsb")
        nc.tensor.matmul(rsb[:], ones_f1[:], rstd32[:], start=True, stop=True)
        mub_sb = p_tmp.tile([P, TCH], BF16, name="mub_sb")
        nc.vector.tensor_copy(mub_sb[:], mub[:])
        rsb_sb = p_tmp.tile([P, TCH], BF16, name="rsb_sb")
        nc.vector.tensor_copy(rsb_sb[:], rsb[:])

        for e in range(KE):
            t1 = p_tmp.tile([P, TCH], BF16, name="t1")
            nc.gpsimd.tensor_tensor(t1[:], xt[e][:, cs], mub_sb[:], OP.subtract)
            nc.vector.tensor_tensor(xn8[:, e, cs], t1[:], rsb_sb[:], OP.mult)

        # residual extraction for the query chunk (last chunk): xq = x + bo
        if ci == NQC - 1:
            for e in range(KE):
                nc.vector.tensor_scalar(
                    xq[e][:], xt[e][:, T - Tq :], gbt["boc"][:, e : e + 1],
                    None, OP.add,
                )

        # interleaved QKV for this chunk
        k_proj(0, ps_qkv, [ci])
        k_proj(1, ps_qkv, [ci])
        v_proj_chunk(ci)
        if ci == NQC - 1:
            for j in range(HP):
                q_proj(j)

    pp.close("qkv_ps", "ln_bc", "ln_st", "ln_rows", "ln_tmp", "xtp", "w8b")

    if DEBUG:
        nc.sync.dma_start(d["dbg_xn"], xn8[:])
        for j in range(HP):
            nc.sync.dma_start(d["dbg_qt"][:, j * Tq : (j + 1) * Tq], qt[j][:])
        for t in range(TK):
            nc.sync.dma_start(
                d["dbg_vs"][:, t * 1040 : (t + 1) * 1040], vsb[t][:]
            )

    # w2 resident load (xt freed now; lands during attention)
    p_w2 = pp.open("w2p", 1)
    w2t = p_w2.tile([P, KF, E], BF16, name="w2t")
    for fq in range(4):
        nc.sync.dma_start(
            w2t[:, 8 * fq : 8 * fq + 8, :],
            d["w2"].rearrange("(f p) m -> p f m", p=P)[:, 8 * fq : 8 * fq + 8, :],
        )

    # ======================================================================
    # Phase 2: attention, head-pair at a time, query-major attnV
    # ======================================================================
    ss_p = pp.open("ss_ps", 2, "PSUM")
    oh_p = pp.open("oh_ps", 1, "PSUM")
    tp_p = pp.open("tp_ps", 1, "PSUM")
    p_pr = pp.open("prp", 2, side="right")
    p_tail = pp.open("tailp", 2, side="right")

    for j in range(HP):
        ohq = oh_p.tile([P, 2, 512], F32, name="ohq")  # per s: 4*65 used
        for t in range(TK):
            u = t % 4
            i = t // 4
            ss = ss_p.tile([P, 2, Tq], F32, name="ss")
            for s in (0, 1):
                nc.tensor.matmul(
                    ss[:, s, u * P : Tq],
                    kt[j][s * 64 : (s + 1) * 64, t * P : (t + 1) * P],
                    qt[j][s * 64 : (s + 1) * 64, u * P : Tq],
                    start=True, stop=True, tile_position=(s * 64, 0),
                )
            pr = p_pr.tile([P, 2, Tq], BF16, name="pr")
            nc.scalar.activation(pr[:, :, u * P : Tq], ss[:, :, u * P : Tq],
                                 AF.Exp, scale=SCL)
            nc.gpsimd.tensor_tensor(
                pr[:, :, u * P : (u + 1) * P], pr[:, :, u * P : (u + 1) * P],
                band[:, i, :, :], OP.mult,
            )
            for m in range(u, NM):
                for s in (0, 1):
                    h = 2 * j + s
                    # start=True resets the whole PSUM bank: only the first
                    # matmul into each bank (m==0 at t==0) may set it; later
                    # groups accumulate onto the bank-wide zero.
                    nc.tensor.matmul(
                        ohq[:, s, m * 65 : m * 65 + 65],
                        pr[:, s, m * P : (m + 1) * P],
                        vsb[t][:, h * (D + 1) : (h + 1) * (D + 1)],
                        start=(t == 0 and m == 0), stop=(t == 12 + m),
                        skip_group_check=True,
                    )
        # next pair's K projection first: PE work to overlap the tail below
        if j + 2 < HP:
            k_proj(j + 2, ss_p, range(NQC), nm="ss", shp=[P, 2, Tq])
        # tail: row-sums -> 1/rs -> normalize -> transpose -> fp8 quantize
        rs_sb = p_tail.tile([P, 2, NM], F32, name="rs_sb")
        for s in (0, 1):
            for m in range(NM):
                nc.vector.tensor_copy(rs_sb[:, s, m : m + 1],
                                      ohq[:, s, m * 65 + 64 : m * 65 + 65])
        irs = p_tail.tile([P, 2, NM], F32, name="irs")
        nc.vector.reciprocal(irs[:], rs_sb[:])
        ptb = p_tail.tile([P, 2, 256], BF16, name="ptb")
        for s in (0, 1):
            for m in range(NM):
                nc.vector.tensor_scalar_mul(
                    ptb[:, s, m * 64 : (m + 1) * 64],
                    ohq[:, s, m * 65 : m * 65 + 64],
                    irs[:, s, m : m + 1],
                )
        for m in range(NM):
            tp = tp_p.tile([P, P], BF16, name="tp")
            nc.tensor.transpose(tp[0:64, :], ptb[:, 0, m * 64 : (m + 1) * 64],
                                ident[:], tile_position=(0, 0))
            nc.tensor.transpose(tp[64:128, :], ptb[:, 1, m * 64 : (m + 1) * 64],
                                ident[:], tile_position=(0, 64))
            nc.vector.tensor_scalar(
                aop8[:, j, m * P : (m + 1) * P], tp[:], SX,
                gbt["bv32"][:, j : j + 1], OP.mult, OP.add,
            )
        if DEBUG:
            nc.sync.dma_start(d["dbg_rs"][:, j, :, :], rs_sb[:])
            nc.sync.dma_start(d["dbg_irs"][:, j, :, :], irs[:])
            nc.sync.dma_start(d["dbg_pt"][:, j, :, :], ptb[:])

    if DEBUG:
        nc.sync.dma_start(d["dbg_ao"], aop8[:])
        for j in range(HP):
            nc.sync.dma_start(d["dbg_kt"][:, j * T : (j + 1) * T], kt[j][:])
    pp.close("tailp", "prp", "tp_ps", "oh_ps", "ss_ps")
    pp.close("vsp", "qtp", "ktp", "xnp")

    # ======================================================================
    # Phase 3: out-projection (fp8) + residual; LayerNorm2
    # ======================================================================
    p_xr = pp.open("xrp", 1)
    p_x2 = pp.open("xn2p", 1)
    ps_ao = pp.open("ao_ps", 2, "PSUM")
    xres = [p_xr.tile([P, Tq], F32, name=f"xres{e}") for e in range(KE)]
    xn2 = [p_x2.tile([P, Tq], BF16, name=f"xn2{e}") for e in range(KE)]

    for e in range(KE):
        ps = ps_ao.tile([P, Tq], F32, name="aops")
        for g in range(KE // 2):
            nc.tensor.matmul(
                ps[:], wo8[:, 2 * g : 2 * g + 2, e * P : (e + 1) * P],
                aop8[:, 2 * g : 2 * g + 2, :],
                start=(g == 0), stop=(g == KE // 2 - 1), perf_mode=DR,
            )
        nc.vector.scalar_tensor_tensor(
            xres[e][:], ps[:], DS, xq[e][:], OP.mult, OP.add
        )
    pp.close("ao_ps", "aop")

    p_tmp = pp.open("ln2_tmp", 3)
    p_rows = pp.open("ln2_rows", 1)
    ps_st = pp.open("ln2_st", 1, "PSUM")
    ps_bc = pp.open("ln2_bc", 1, "PSUM")
    s1 = ps_st.tile([1, Tq], F32, name="s1b")
    s2 = ps_st.tile([1, Tq], F32, name="s2b")
    for e in range(KE):
        xbf = p_tmp.tile([P, Tq], BF16, name="xbf2")
        nc.vector.tensor_copy(xbf[:], xres[e][:])
        x2 = p_tmp.tile([P, Tq], BF16, name="x2bf2")
        nc.scalar.square(x2[:], xres[e][:])
        nc.tensor.matmul(s1[:], ones_bf[:], xbf[:], start=(e == 0), stop=(e == KE - 1))
        nc.tensor.matmul(s2[:], ones_bf[:], x2[:], start=(e == 0), stop=(e == KE - 1))
    mu = p_rows.tile([1, Tq], F32, name="mu_2")
    nc.vector.tensor_scalar_mul(mu[:], s1[:], 1.0 / E)
    ve = p_rows.tile([1, Tq], F32, name="ve_2")
    nc.vector.tensor_scalar(ve[:], s2[:], 1.0 / E, c.eps, OP.mult, OP.add)
    mu2 = p_rows.tile([1, Tq], F32, name="mu2_2")
    nc.vector.tensor_tensor(mu2[:], mu[:], mu[:], OP.mult)
    vee = p_rows.tile([1, Tq], F32, name="vee_2")
    nc.vector.tensor_tensor(vee[:], ve[:], mu2[:], OP.subtract)
    sq2 = p_rows.tile([1, Tq], F32, name="sq_2")
    nc.scalar.activation(sq2[:], vee[:], AF.Sqrt)
    rstd = p_rows.tile([1, Tq], F32, name="rstd_2")
    nc.vector.reciprocal_approx_fast(rstd[:], sq2[:])
    mub = ps_bc.tile([P, Tq], F32, name="mub2")
    nc.tensor.matmul(mub[:], ones_f1[:], mu[:], start=True, stop=True)
    rsb = ps_bc.tile([P, Tq], F32, name="rsb2")
    nc.tensor.matmul(rsb[:], ones_f1[:], rstd[:], start=True, stop=True)
    for e in range(KE):
        t1 = p_tmp.tile([P, Tq], F32, name="t1b")
        nc.vector.tensor_tensor(t1[:], xres[e][:], mub[:], OP.subtract)
        nc.vector.tensor_tensor(xn2[e][:], t1[:], rsb[:], OP.mult)
    pp.close("ln2_rows", "ln2_tmp", "ln2_bc", "ln2_st")

    # ======================================================================
    # Phase 4: MLP (bf16), W2 resident
    # ======================================================================
    EH = 6
    p_h1 = pp.open("h1p", 1, side="right")
    p_out = pp.open("outp", 2)
    p_w1 = pp.open("w1s", 3)
    ps_h1 = pp.open("h1_ps", 2, "PSUM")
    ps_h2a = pp.open("h2a_ps", 1, "PSUM")

    h1 = [p_h1.tile([P, Tq], BF16, name=f"h1{f}") for f in range(KF)]
    h2a = [ps_h2a.tile([P, Tq], F32, name=f"h2a{e}") for e in range(EH)]
    for f in range(KF):
        w1f = p_w1.tile([P, KE, P], BF16, name="w1cb")
        nc.sync.dma_start(
            w1f[:],
            d["w1"].rearrange("(e p) m -> p e m", p=P)[:, :, f * P : (f + 1) * P],
        )
        ps = ps_h1.tile([P, Tq], F32, name="h1ps")
        for e in range(KE):
            nc.tensor.matmul(
                ps[:], w1f[:, e, :], xn2[e][:], start=(e == 0), stop=(e == KE - 1)
            )
        nc.scalar.activation(
            h1[f][:], ps[:], AF.Relu, bias=gbt["mb1"][:, f : f + 1], scale=1.0
        )
        for e in range(EH):
            nc.tensor.matmul(
                h2a[e][:], w2t[:, f, e * P : (e + 1) * P], h1[f][:],
                start=(f == 0), stop=(f == KF - 1),
            )
    for e in range(EH):
        of = p_out.tile([P, Tq], F32, name="outf")
        nc.vector.scalar_tensor_tensor(
            of[:], h2a[e][:], gbt["mb2"][:, e : e + 1], xres[e][:], OP.add, OP.add
        )
        nc.sync.dma_start(d["out_t"][e * P : (e + 1) * P, :], of[:])
    pp.close("h2a_ps", "h1_ps")

    ps_h2b = pp.open("h2b_ps", 1, "PSUM")
    h2b = [ps_h2b.tile([P, Tq], F32, name=f"h2b{i}") for i in range(KE - EH)]
    for f in range(KF):
        for i, e in enumerate(range(EH, KE)):
            nc.tensor.matmul(
                h2b[i][:], w2t[:, f, e * P : (e + 1) * P], h1[f][:],
                start=(f == 0), stop=(f == KF - 1),
            )
    for i, e in enumerate(range(EH, KE)):
        of = p_out.tile([P, Tq], F32, name="outf")
        nc.vector.scalar_tensor_tensor(
            of[:], h2b[i][:], gbt["mb2"][:, e : e + 1], xres[e][:], OP.add, OP.add
        )
        nc.sync.dma_start(d["out_t"][e * P : (e + 1) * P, :], of[:])

    pp.close_all()


def build_program(c: Cfg = CFG):
    c.check()
    nc = bacc.Bacc(
        "TRN2",
        target_bir_lowering=False,
        debug=False,
        enable_asserts=False,
        num_devices=c.NC,
    )
    d = {}
    d["xt"] = nc.dram_tensor("xt", [c.E, c.T], BF16, kind="ExternalInput").ap()
    for nm in ("wq8", "wk8", "wv8", "wo8"):
        d[nm] = nc.dram_tensor(nm, [c.E, c.E], F8, kind="ExternalInput").ap()
    d["w1"] = nc.dram_tensor("w1", [c.E, c.F], BF16, kind="ExternalInput").ap()
    d["w2"] = nc.dram_tensor("w2", [c.F, c.E], BF16, kind="ExternalInput").ap()
    for nm, cols in [("bq", c.KE), ("bk", c.KE), ("bv32", c.KE), ("boc", c.KE),
                     ("mb1", c.KF), ("mb2", c.KE)]:
        d[nm] = nc.dram_tensor(nm, [P, cols], F32, kind="ExternalInput").ap()
    d["band"] = nc.dram_tensor("band", [P, c.NQC * 2 * P], BF16,
                               kind="ExternalInput").ap()
    d["ident"] = nc.dram_tensor("ident", [P, P], BF16, kind="ExternalInput").ap()
    d["out_t"] = nc.dram_tensor("out_t", [c.E, c.Tq], F32, kind="ExternalOutput").ap()
    if DEBUG:
        d["dbg_xn"] = nc.dram_tensor("dbg_xn", [P, c.KE, c.T], F8,
                                     kind="ExternalOutput").ap()
        d["dbg_qt"] = nc.dram_tensor("dbg_qt", [P, c.HP * c.Tq], BF16,
                                     kind="ExternalOutput").ap()
        d["dbg_kt"] = nc.dram_tensor("dbg_kt", [P, c.HP * c.T], BF16,
                                     kind="ExternalOutput").ap()
        d["dbg_vs"] = nc.dram_tensor("dbg_vs", [P, c.TK * 1040], BF16,
                                     kind="ExternalOutput").ap()
        d["dbg_ao"] = nc.dram_tensor("dbg_ao", [P, c.HP, c.Tq], F8,
                                     kind="ExternalOutput").ap()
        d["dbg_rs"] = nc.dram_tensor("dbg_rs", [P, c.HP, 2, 4], F32,
                                     kind="ExternalOutput").ap()
        d["dbg_irs"] = nc.dram_tensor("dbg_irs", [P, c.HP, 2, 4], F32,
                                      kind="ExternalOutput").ap()
        d["dbg_pt"] = nc.dram_tensor("dbg_pt", [P, c.HP, 2, 256], BF16,
                                     kind="ExternalOutput").ap()

    with tile.TileContext(nc) as tc:
        _emit(tc, c, d)
    nc.compile()
    return nc


# --------------------------------------------------------------------------
# host side
# --------------------------------------------------------------------------
def shard_inputs(inputs, c: Cfg = CFG):
    f32 = lambda a: np.ascontiguousarray(np.asarray(a, np.float32))
    x = f32(inputs["x"])
    g1, b1n = f32(inputs["ln1_g"]), f32(inputs["ln1_b"])
    g2, b2n = f32(inputs["ln2_g"]), f32(inputs["ln2_b"])
    Wq, Wk, Wv, Wo = (f32(inputs[k]) for k in ("Wq", "Wk", "Wv", "Wo"))
    W1, W2 = f32(inputs["W1"]), f32(inputs["W2"])
    bo, b1, b2 = f32(inputs["bo"]), f32(inputs["b1"]), f32(inputs["b2"])

    q8 = lambda w: np.ascontiguousarray((w * SW)).astype(NPF8)
    bf = lambda w: np.ascontiguousarray(w).astype(NPBF16)
    chunks = lambda v, k: np.ascontiguousarray(v.reshape(k, P).T)

    com = {
        "wq8": q8(g1[:, None] * Wq),
        "wk8": q8(g1[:, None] * Wk),
        "wv8": q8(g1[:, None] * Wv),
        "wo8": q8(Wo),
        "w1": bf(g2[:, None] * W1),
        "w2": bf(W2),
        "bq": chunks(b1n @ Wq, c.KE),
        "bk": chunks(b1n @ Wk, c.KE),
        "bv32": chunks((b1n @ Wv) * SX, c.KE),
        "boc": chunks(bo, c.KE),
        "mb1": chunks(b1 + b2n @ W1, c.KF),
        "mb2": chunks(b2, c.KE),
        "ident": np.eye(P, dtype=np.float32).astype(NPBF16),
    }

    p_idx = np.arange(P)[:, None]
    lq_idx = np.arange(P)[None, :]
    maps = []
    for core in range(c.NC):
        b, j = core // c.CPB, core % c.CPB
        phases = [(j + 1) % 4, (j + 2) % 4, (j + 3) % 4, j]
        ctx = np.empty((c.E, c.T), np.float32)
        for i, ph in enumerate(phases):
            ctx[:, 512 * i : 512 * (i + 1)] = x[b, ph::4, :].T
        band = np.zeros((P, c.NQC, 2, P), np.float32)
        for i, ph in enumerate(phases):
            delta = 1 if ph > j else 0
            m_ = (p_idx <= lq_idx - delta).astype(np.float32)
            band[:, i, 0, :] = m_
            band[:, i, 1, :] = m_
        m = dict(com)
        m["xt"] = bf(ctx)
        m["band"] = band.reshape(P, c.NQC * 2 * P).astype(NPBF16)
        maps.append(m)
    return maps


def assemble(results, c: Cfg = CFG):
    out = np.empty((c.B, c.T, c.E), np.float32)
    for core in range(c.NC):
        b, j = core // c.CPB, core % c.CPB
        out[b, j::4, :] = results[core]["out_t"].T
    return out


_NC_CACHE = {}


def _get_nc(c: Cfg = CFG):
    if c not in _NC_CACHE:
        _NC_CACHE[c] = build_program(c)
    return _NC_CACHE[c]


LAST_RESULT = None


def kernel(**inputs):
    global LAST_RESULT
    c = CFG
    nc = _get_nc(c)
    maps = shard_inputs(inputs, c)
    res = bass_utils.run_bass_kernel_spmd(nc, maps, core_ids=list(range(c.NC)))
    LAST_RESULT = res
    return assemble(res.results, c)


# revision 52
# speedup vs baseline: 1.7429x; 1.0936x over previous
"""Trainium2 Bass kernel for a pre-norm transformer block (dense_transformer).

Computation (per reference):
    x = x + Attn(LN1(x));  x = x + MLP(LN2(x))
with causal multi-head attention (H=16 heads, D=64) and a 4E ReLU MLP.

Sharding: 8 cores = 2 batches x 4 query PHASES.  Core (b, j) owns the 512
query tokens {4r + j}.  The context (all 2048 tokens) is column-PERMUTED
per core so the core's own phase sits last: position 512*i + r holds token
4r + phase_i with phase order (j+1, j+2, j+3, j) mod 4.  With this striping
the causal block structure is identical on every core (SPMD uniform): query
tile m attends context tiles t with t%4 <= m, so upper score tiles are
skipped for real FLOP savings; the diagonal band mask is a per-core input.

Precision: QKV and output projections run in fp8e4 DoubleRow (2x PE perf,
K=256 per instruction) with power-of-2 scales folded into the weights and
descale factors folded into existing copies; LN gains/biases are folded
into the weights host-side.  Scores/attnV/MLP stay bf16 (error budget).

attnV runs "query-major": out[q, d] with a ones column appended to V, so
softmax row-sums accumulate for free in PSUM column 64; normalization is a
per-partition scalar multiply, then a PE transpose returns to feature-major
for the fp8 out-projection.
"""

from dataclasses import dataclass

import numpy as np
import ml_dtypes

import concourse.bass as bass  # noqa: F401
import concourse.mybir as mybir
import concourse.tile as tile
from concourse import bacc
from concourse import bass_utils

F32 = mybir.dt.float32
BF16 = mybir.dt.bfloat16
F8 = mybir.dt.float8e4
AF = mybir.ActivationFunctionType
OP = mybir.AluOpType
DR = mybir.MatmulPerfMode.DoubleRow
NPBF16 = ml_dtypes.bfloat16
NPF8 = ml_dtypes.float8_e4m3

P = 128
SX = 32.0       # fp8 scale on activations
SW = 256.0      # fp8 scale on weights
DS = 1.0 / (SX * SW)


@dataclass(frozen=True)
class Cfg:
    B: int = 2
    T: int = 2048
    E: int = 1024
    H: int = 16
    D: int = 64
    NC: int = 8
    eps: float = 1e-5

    @property
    def CPB(self):
        return self.NC // self.B

    @property
    def Tq(self):
        return self.T // self.CPB

    @property
    def KE(self):
        return self.E // P

    @property
    def TK(self):
        return self.T // P

    @property
    def HP(self):
        return self.H // 2

    @property
    def F(self):
        return 4 * self.E

    @property
    def KF(self):
        return self.F // P

    @property
    def TCH(self):
        return 512

    @property
    def NQC(self):
        return self.T // self.TCH

    @property
    def NM(self):
        return self.Tq // P  # query tiles per core

    def check(self):
        assert self.D == 64 and self.E == self.H * self.D
        assert self.Tq == 512 and self.KE == 8 and self.TK == 16
        assert self.CPB == 4 and self.HP == 8 and self.KF == 32


CFG = Cfg()
DEBUG = False


class Pools:
    def __init__(self, tc, prefix=""):
        self.tc = tc
        self.prefix = prefix
        self.live = {}

    def open(self, key, bufs, space=None, side=None):
        kw = dict(name=self.prefix + key, bufs=bufs)
        if space:
            kw["space"] = space
        if side:
            kw["side"] = side
        cm = self.tc.tile_pool(**kw)
        pool = cm.__enter__()
        self.live[key] = cm
        return pool

    def close(self, *keys):
        for key in keys:
            self.live.pop(key).__exit__(None, None, None)

    def close_all(self):
        for key in reversed(list(self.live)):
            self.close(key)


def _emit(tc, c: Cfg, d):
    nc = tc.nc
    E, T, Tq, H, D = c.E, c.T, c.Tq, c.H, c.D
    KE, TK, HP, KF, NM = c.KE, c.TK, c.HP, c.KF, c.NM
    TCH, NQC = c.TCH, c.NQC
    SCL = 1.0 / float(np.sqrt(D))

    pp = Pools(tc)

    # ---------------- constants ----------------
    const = pp.open("const", 1)
    ones_bf = const.tile([P, 1], BF16, name="ones_bf")
    nc.vector.memset(ones_bf[:], 1.0)
    ones_f1 = const.tile([1, P], F32, name="ones_f1")
    nc.vector.memset(ones_f1[:], 1.0)
    ident = const.tile([P, P], BF16, name="ident")
    band = const.tile([P, NQC, 2, P], BF16, name="band")
    gbt = {}
    for nm, cols in [("bq", KE), ("bk", KE), ("bv32", KE), ("boc", KE),
                     ("mb1", KF)]:
        gbt[nm] = const.tile([P, cols], F32, name=nm + "_t")
        nc.sync.dma_start(gbt[nm][:], d[nm])

    # ---------------- weights (fp8, resident) ----------------
    p_w8 = pp.open("w8", 1)
    wk8 = p_w8.tile([P, KE, E], F8, name="wk8")
    wo8 = p_w8.tile([P, KE, E], F8, name="wo8")

    # ---------------- warmup (PE p-state ramp; no DMA dependency) ----------
    p_wsb = pp.open("warm_sb", 1, side="right")
    ps_wm = pp.open("warm_ps", 1, "PSUM")
    wsb = p_wsb.tile([P, TCH], BF16, name="wsb")
    nc.vector.memset(wsb[:], 0.0)
    wmp = ps_wm.tile([1, TCH], F32, name="wmp")
    for _w in range(8):
        nc.tensor.matmul(wmp[:], ones_bf[:], wsb[:], start=True, stop=True)
    pp.close("warm_ps", "warm_sb")

    # ---------------- long-lived activations ----------------
    # left stack: const, w8, xqp | w8b, xtp, LN pools (freed) | w2p, phase3/4
    # right stack: aop | xnp, ktp, qtp, vsp (freed after attention), attn pools
    p_xq = pp.open("xqp", 1)
    xq = [p_xq.tile([P, Tq], F32, name=f"xq{e}") for e in range(KE)]
    p_ao = pp.open("aop", 1, side="right")
    aop8 = p_ao.tile([P, HP, Tq], F8, name="aop8")
    p_xn = pp.open("xnp", 1, side="right")
    xn8 = p_xn.tile([P, KE, T], F8, name="xn8")
    p_kt = pp.open("ktp", 1, side="right")
    kt = [p_kt.tile([P, T], BF16, name=f"kt{j}") for j in range(HP)]
    p_qt = pp.open("qtp", 1, side="right")
    qt = [p_qt.tile([P, Tq], BF16, name=f"qt{j}") for j in range(HP)]
    p_vs = pp.open("vsp", 1, side="right")
    vsb = [p_vs.tile([P, H * (D + 1)], BF16, name=f"vsb{t}") for t in range(TK)]

    # ======================================================================
    # Phase 1: x load + LN1 + QKV (chunk-interleaved)
    # ======================================================================
    p_w8b = pp.open("w8b", 1)
    wq8 = p_w8b.tile([P, KE, E], F8, name="wq8")
    wv8 = p_w8b.tile([P, KE, E], F8, name="wv8")

    p_xt = pp.open("xtp", 1)
    xt = [p_xt.tile([P, T], BF16, name=f"xt{e}") for e in range(KE)]
    # DMA priority order: xt (stats start on it), then K/V weights (used from
    # chunk 0), then Q (chunk 3) and O/ident/band (attention phase).
    for e in range(KE):
        nc.sync.dma_start(xt[e][:], d["xt"][e * P : (e + 1) * P, :])
    for nm, t_ in [("wv8", wv8), ("wk8", wk8), ("wq8", wq8), ("wo8", wo8)]:
        nc.sync.dma_start(t_[:], d[nm].rearrange("(e p) m -> p e m", p=P))
    nc.sync.dma_start(ident[:], d["ident"])
    nc.sync.dma_start(band[:], d["band"].rearrange("p (i s q) -> p i s q",
                                                   i=NQC, s=2))

    # ones columns of V (col 64 of each head slot)
    for t in range(TK):
        nc.vector.memset(vsb[t][:, :].rearrange("p (h d) -> p h d", d=D + 1)[:, :, D], 1.0)

    p_tmp = pp.open("ln_tmp", 3)
    p_rows = pp.open("ln_rows", 1)
    ps_st = pp.open("ln_st", 2, "PSUM")
    ps_bc = pp.open("ln_bc", 1, "PSUM")
    ps_qkv = pp.open("qkv_ps", 2, "PSUM")

    def k_proj(j, psum_pool, cis, nm="qkv", shp=None):
        for ci in cis:
            cs = slice(ci * TCH, (ci + 1) * TCH)
            pst = psum_pool.tile(shp or [P, TCH], F32, name=nm)
            ps = pst[:, 0, :] if shp else pst[:]
            for g in range(KE // 2):
                nc.tensor.matmul(
                    ps, wk8[:, 2 * g : 2 * g + 2, j * P : (j + 1) * P],
                    xn8[:, 2 * g : 2 * g + 2, cs],
                    start=(g == 0), stop=(g == KE // 2 - 1), perf_mode=DR,
                )
            nc.scalar.activation(
                kt[j][:, cs], ps, AF.Identity, bias=gbt["bk"][:, j : j + 1],
                scale=DS,
            )

    def v_proj_chunk(ci):
        for tt in range(4 * ci, 4 * ci + 4):
            for hf in range(2):
                ps = ps_qkv.tile([P, KE, D], F32, name="qkv")
                for g in range(KE // 2):
                    nc.tensor.matmul(
                        ps[:], xn8[:, 2 * g : 2 * g + 2, tt * P : (tt + 1) * P],
                        wv8[:, 2 * g : 2 * g + 2, hf * 512 : hf * 512 + 512],
                        start=(g == 0), stop=(g == KE // 2 - 1), perf_mode=DR,
                    )
                dst = vsb[tt][:, hf * 8 * (D + 1) :].rearrange(
                    "p (h d) -> p h d", d=D + 1
                )[:, 0:8, 0:D]
                nc.vector.tensor_scalar_mul(dst, ps[:], DS)

    def q_proj(j):
        ps = ps_qkv.tile([P, Tq], F32, name="qkv")
        for g in range(KE // 2):
            nc.tensor.matmul(
                ps[:], wq8[:, 2 * g : 2 * g + 2, j * P : (j + 1) * P],
                xn8[:, 2 * g : 2 * g + 2, T - Tq :],
                start=(g == 0), stop=(g == KE // 2 - 1), perf_mode=DR,
            )
        nc.vector.tensor_scalar(
            qt[j][:], ps[:], DS, gbt["bq"][:, j : j + 1], OP.mult, OP.add
        )

    def ln_stats(ci):
        """Stats matmuls + row math + broadcast + SBUF stage for chunk ci.
        Returns (mub_sb, rsb_sb)."""
        cs = slice(ci * TCH, (ci + 1) * TCH)
        s1 = ps_st.tile([1, TCH], F32, name="s1")
        s2 = ps_st.tile([1, TCH], F32, name="s2")
        for e in range(KE):
            x2 = p_tmp.tile([P, TCH], BF16, name="x2bf")
            nc.scalar.square(x2[:], xt[e][:, cs])
            nc.tensor.matmul(s1[:], ones_bf[:], xt[e][:, cs],
                             start=(e == 0), stop=(e == KE - 1))
            nc.tensor.matmul(s2[:], ones_bf[:], x2[:],
                             start=(e == 0), stop=(e == KE - 1))
        mu = p_rows.tile([1, TCH], F32, name="mu")
        nc.vector.tensor_scalar_mul(mu[:], s1[:], 1.0 / E)
        ve = p_rows.tile([1, TCH], F32, name="ve")
        nc.vector.tensor_scalar(ve[:], s2[:], 1.0 / E, c.eps, OP.mult, OP.add)
        mu2 = p_rows.tile([1, TCH], F32, name="mu2")
        nc.vector.tensor_tensor(mu2[:], mu[:], mu[:], OP.mult)
        vee = p_rows.tile([1, TCH], F32, name="vee")
        nc.vector.tensor_tensor(vee[:], ve[:], mu2[:], OP.subtract)
        # rstd*SX: sqrt(vee/SX^2) on ACT (stays in the sqrt table set),
        # then fast reciprocal on DVE -> SX/sqrt(vee)
        sq = p_rows.tile([1, TCH], F32, name="sq")
        nc.scalar.activation(sq[:], vee[:], AF.Sqrt, scale=1.0 / (SX * SX))
        rstd32 = p_rows.tile([1, TCH], F32, name="rstd32")
        nc.vector.reciprocal_approx_fast(rstd32[:], sq[:])

        mub = ps_bc.tile([P, TCH], F32, name="mub")
        nc.tensor.matmul(mub[:], ones_f1[:], mu[:], start=True, stop=True)
        rsb = ps_bc.tile([P, TCH], F32, name="rsb")
        nc.tensor.matmul(rsb[:], ones_f1[:], rstd32[:], start=True, stop=True)
        mub_sb = p_tmp.tile([P, TCH], BF16, name="mub_sb")
        nc.vector.tensor_copy(mub_sb[:], mub[:])
        rsb_sb = p_tmp.tile([P, TCH], BF16, name="rsb_sb")
        nc.vector.tensor_copy(rsb_sb[:], rsb[:])
        return mub_sb, rsb_sb

    # software-pipelined: stats(ci+1) emitted before projections(ci) so the
    # PE has stat matmuls to chew while DVE/Pool normalize chunk ci
    stg = ln_stats(0)
    for ci in range(NQC):
        cs = slice(ci * TCH, (ci + 1) * TCH)
        mub_sb, rsb_sb = stg
        for e in range(KE):
            t1 = p_tmp.tile([P, TCH], BF16, name="t1")
            nc.gpsimd.tensor_tensor(t1[:], xt[e][:, cs], mub_sb[:], OP.subtract)
            nc.vector.tensor_tensor(xn8[:, e, cs], t1[:], rsb_sb[:], OP.mult)
        if ci + 1 < NQC:
            stg = ln_stats(ci + 1)

        # residual extraction for the query chunk (last chunk): xq = x + bo
        if ci == NQC - 1:
            for e in range(KE):
                nc.vector.tensor_scalar(
                    xq[e][:], xt[e][:, T - Tq :], gbt["boc"][:, e : e + 1],
                    None, OP.add,
                )

        # interleaved QKV for this chunk
        k_proj(0, ps_qkv, [ci])
        k_proj(1, ps_qkv, [ci])
        v_proj_chunk(ci)
        if ci == NQC - 1:
            for j in range(HP):
                q_proj(j)

    pp.close("qkv_ps", "ln_bc", "ln_st", "ln_rows", "ln_tmp", "xtp", "w8b")

    if DEBUG:
        nc.sync.dma_start(d["dbg_xn"], xn8[:])
        for j in range(HP):
            nc.sync.dma_start(d["dbg_qt"][:, j * Tq : (j + 1) * Tq], qt[j][:])
        for t in range(TK):
            nc.sync.dma_start(
                d["dbg_vs"][:, t * 1040 : (t + 1) * 1040], vsb[t][:]
            )

    # w2 resident load (xt freed now; lands during attention)
    p_w2 = pp.open("w2p", 1)
    w2a = p_w2.tile([P, KF, E], F8, name="w2a")
    w2b = p_w2.tile([P, KF, E], F8, name="w2b")
    mb2dr = p_w2.tile([P, 2, E], F8, name="mb2dr")
    ones8 = p_w2.tile([P, 2, Tq], F8, name="ones8")
    nc.vector.memset(ones8[:], 1.0)
    nc.sync.dma_start(mb2dr[:], d["mb2dr"].rearrange("p (k m) -> p k m", k=2))
    for nm, t_ in (("w2a", w2a), ("w2b", w2b)):
        for fq in range(4):
            nc.sync.dma_start(
                t_[:, 8 * fq : 8 * fq + 8, :],
                d[nm].rearrange("(f p) m -> p f m", p=P)[:, 8 * fq : 8 * fq + 8, :],
            )

    # ======================================================================
    # Phase 2: attention, head-pair at a time, query-major attnV
    # ======================================================================
    ss_p = pp.open("ss_ps", 2, "PSUM")
    oh_p = pp.open("oh_ps", 1, "PSUM")
    tp_p = pp.open("tp_ps", 1, "PSUM")
    p_pr = pp.open("prp", 2, side="right")
    p_tail = pp.open("tailp", 2, side="right")

    for j in range(HP):
        ohq = oh_p.tile([P, 2, 512], F32, name="ohq")  # per s: 4*65 used
        for t in range(TK):
            u = t % 4
            i = t // 4
            ss = ss_p.tile([P, 2, Tq], F32, name="ss")
            for s in (0, 1):
                nc.tensor.matmul(
                    ss[:, s, u * P : Tq],
                    kt[j][s * 64 : (s + 1) * 64, t * P : (t + 1) * P],
                    qt[j][s * 64 : (s + 1) * 64, u * P : Tq],
                    start=True, stop=True, tile_position=(s * 64, 0),
                )
            pr = p_pr.tile([P, 2, Tq], BF16, name="pr")
            nc.scalar.activation(pr[:, :, u * P : Tq], ss[:, :, u * P : Tq],
                                 AF.Exp, scale=SCL)
            nc.gpsimd.tensor_tensor(
                pr[:, :, u * P : (u + 1) * P], pr[:, :, u * P : (u + 1) * P],
                band[:, i, :, :], OP.mult,
            )
            for m in range(u, NM):
                for s in (0, 1):
                    h = 2 * j + s
                    # start=True resets the whole PSUM bank: only the first
                    # matmul into each bank (m==0 at t==0) may set it; later
                    # groups accumulate onto the bank-wide zero.
                    nc.tensor.matmul(
                        ohq[:, s, m * 65 : m * 65 + 65],
                        pr[:, s, m * P : (m + 1) * P],
                        vsb[t][:, h * (D + 1) : (h + 1) * (D + 1)],
                        start=(t == 0 and m == 0), stop=(t == 12 + m),
                        skip_group_check=True,
                    )
        # next pair's K projection first: PE work to overlap the tail below
        if j + 2 < HP:
            k_proj(j + 2, ss_p, range(NQC), nm="ss", shp=[P, 2, Tq])
        # tail: row-sums -> 1/rs -> normalize -> transpose -> fp8 quantize
        rs_sb = p_tail.tile([P, 2, NM], F32, name="rs_sb")
        for s in (0, 1):
            for m in range(NM):
                nc.vector.tensor_copy(rs_sb[:, s, m : m + 1],
                                      ohq[:, s, m * 65 + 64 : m * 65 + 65])
        irs = p_tail.tile([P, 2, NM], F32, name="irs")
        nc.vector.reciprocal(irs[:], rs_sb[:])
        ptb = p_tail.tile([P, 2, 256], BF16, name="ptb")
        for s in (0, 1):
            for m in range(NM):
                nc.vector.tensor_scalar_mul(
                    ptb[:, s, m * 64 : (m + 1) * 64],
                    ohq[:, s, m * 65 : m * 65 + 64],
                    irs[:, s, m : m + 1],
                )
        for m in range(NM):
            tp = tp_p.tile([P, P], BF16, name="tp")
            nc.tensor.transpose(tp[0:64, :], ptb[:, 0, m * 64 : (m + 1) * 64],
                                ident[:], tile_position=(0, 0))
            nc.tensor.transpose(tp[64:128, :], ptb[:, 1, m * 64 : (m + 1) * 64],
                                ident[:], tile_position=(0, 64))
            nc.vector.tensor_scalar(
                aop8[:, j, m * P : (m + 1) * P], tp[:], SX,
                gbt["bv32"][:, j : j + 1], OP.mult, OP.add,
            )
        if DEBUG:
            nc.sync.dma_start(d["dbg_rs"][:, j, :, :], rs_sb[:])
            nc.sync.dma_start(d["dbg_irs"][:, j, :, :], irs[:])
            nc.sync.dma_start(d["dbg_pt"][:, j, :, :], ptb[:])

    if DEBUG:
        nc.sync.dma_start(d["dbg_ao"], aop8[:])
        for j in range(HP):
            nc.sync.dma_start(d["dbg_kt"][:, j * T : (j + 1) * T], kt[j][:])
    pp.close("tailp", "prp", "tp_ps", "oh_ps", "ss_ps")
    pp.close("vsp", "qtp", "ktp", "xnp")

    # ======================================================================
    # Phase 3: out-projection (fp8) + residual; LayerNorm2
    # ======================================================================
    p_xr = pp.open("xrp", 1)
    p_x2 = pp.open("xn2p", 1)
    ps_ao = pp.open("ao_ps", 2, "PSUM")
    xres = [p_xr.tile([P, Tq], F32, name=f"xres{e}") for e in range(KE)]
    x2h = p_x2.tile([P, KE, Tq], F8, name="x2h")
    x2l = p_x2.tile([P, KE, Tq], F8, name="x2l")

    for e in range(KE):
        ps = ps_ao.tile([P, Tq], F32, name="aops")
        for g in range(KE // 2):
            nc.tensor.matmul(
                ps[:], wo8[:, 2 * g : 2 * g + 2, e * P : (e + 1) * P],
                aop8[:, 2 * g : 2 * g + 2, :],
                start=(g == 0), stop=(g == KE // 2 - 1), perf_mode=DR,
            )
        nc.vector.scalar_tensor_tensor(
            xres[e][:], ps[:], DS, xq[e][:], OP.mult, OP.add
        )
    pp.close("ao_ps", "aop")

    p_tmp = pp.open("ln2_tmp", 3)
    p_rows = pp.open("ln2_rows", 1)
    ps_st = pp.open("ln2_st", 1, "PSUM")
    ps_bc = pp.open("ln2_bc", 1, "PSUM")
    s1 = ps_st.tile([1, Tq], F32, name="s1b")
    s2 = ps_st.tile([1, Tq], F32, name="s2b")
    for e in range(KE):
        xbf = p_tmp.tile([P, Tq], BF16, name="xbf2")
        nc.vector.tensor_copy(xbf[:], xres[e][:])
        x2 = p_tmp.tile([P, Tq], BF16, name="x2bf2")
        nc.scalar.square(x2[:], xres[e][:])
        nc.tensor.matmul(s1[:], ones_bf[:], xbf[:], start=(e == 0), stop=(e == KE - 1))
        nc.tensor.matmul(s2[:], ones_bf[:], x2[:], start=(e == 0), stop=(e == KE - 1))
    mu = p_rows.tile([1, Tq], F32, name="mu_2")
    nc.vector.tensor_scalar_mul(mu[:], s1[:], 1.0 / E)
    ve = p_rows.tile([1, Tq], F32, name="ve_2")
    nc.vector.tensor_scalar(ve[:], s2[:], 1.0 / E, c.eps, OP.mult, OP.add)
    mu2 = p_rows.tile([1, Tq], F32, name="mu2_2")
    nc.vector.tensor_tensor(mu2[:], mu[:], mu[:], OP.mult)
    vee = p_rows.tile([1, Tq], F32, name="vee_2")
    nc.vector.tensor_tensor(vee[:], ve[:], mu2[:], OP.subtract)
    sq2 = p_rows.tile([1, Tq], F32, name="sq_2")
    nc.scalar.activation(sq2[:], vee[:], AF.Sqrt, scale=1.0 / (SX * SX))
    rstd = p_rows.tile([1, Tq], F32, name="rstd_2")
    nc.vector.reciprocal_approx_fast(rstd[:], sq2[:])
    mub = ps_bc.tile([P, Tq], F32, name="mub2")
    nc.tensor.matmul(mub[:], ones_f1[:], mu[:], start=True, stop=True)
    rsb = ps_bc.tile([P, Tq], F32, name="rsb2")
    nc.tensor.matmul(rsb[:], ones_f1[:], rstd[:], start=True, stop=True)
    # xn2 (scaled by SX) split into fp8 head + fp8 residual for DoubleRow MLP1
    for e in range(KE):
        t1 = p_tmp.tile([P, Tq], F32, name="t1b")
        nc.vector.tensor_tensor(t1[:], xres[e][:], mub[:], OP.subtract)
        xn2bf = p_tmp.tile([P, Tq], BF16, name="xn2bf")
        nc.vector.tensor_tensor(xn2bf[:], t1[:], rsb[:], OP.mult)
        nc.vector.tensor_copy(x2h[:, e, :], xn2bf[:])
        nc.vector.tensor_tensor(x2l[:, e, :], xn2bf[:], x2h[:, e, :], OP.subtract)
    pp.close("ln2_rows", "ln2_tmp", "ln2_bc", "ln2_st")

    # ======================================================================
    # Phase 4: MLP in split-fp8 DoubleRow: X~H+L, W~A+B (fp8 residuals);
    # X@W ~ H@A + H@B + L@A at 0.75x the bf16 PE cost, ~bf16 accuracy.
    # ======================================================================
    EH = 6
    p_h1 = pp.open("h1p", 1, side="right")
    p_hbf = pp.open("h1bfp", 3, side="right")
    p_out = pp.open("outp", 2)
    p_w1 = pp.open("w1s", 3)
    ps_h1 = pp.open("h1_ps", 2, "PSUM")
    ps_h2a = pp.open("h2a_ps", 1, "PSUM")

    h1h = p_h1.tile([P, KF, Tq], F8, name="h1h")
    h1l = p_h1.tile([P, KF, Tq], F8, name="h1l")
    h2a = [ps_h2a.tile([P, Tq], F32, name=f"h2a{e}") for e in range(EH)]

    def mlp2_acc(g, es, h2ps, first, last):
        for i, e in enumerate(es):
            for wt, ht in ((w2a, h1h), (w2b, h1h), (w2a, h1l)):
                fst = first and wt is w2a and ht is h1h
                lst = last and wt is w2a and ht is h1l
                nc.tensor.matmul(
                    h2ps[i][:], wt[:, 2 * g : 2 * g + 2, e * P : (e + 1) * P],
                    ht[:, 2 * g : 2 * g + 2, :],
                    start=fst, stop=False, perf_mode=DR, skip_group_check=True,
                )
            if last:
                # += b2*8192 via ones rhs, then stop the group
                nc.tensor.matmul(
                    h2ps[i][:], mb2dr[:, :, e * P : (e + 1) * P], ones8[:],
                    start=False, stop=True, perf_mode=DR, skip_group_check=True,
                )

    for f in range(KF):
        w1af = p_w1.tile([P, KE, P], F8, name="w1a")
        w1bf = p_w1.tile([P, KE, P], F8, name="w1b")
        nc.sync.dma_start(
            w1af[:],
            d["w1a"].rearrange("(e p) m -> p e m", p=P)[:, :, f * P : (f + 1) * P],
        )
        nc.sync.dma_start(
            w1bf[:],
            d["w1b"].rearrange("(e p) m -> p e m", p=P)[:, :, f * P : (f + 1) * P],
        )
        ps = ps_h1.tile([P, Tq], F32, name="h1ps")
        for g in range(KE // 2):
            for wt, xt_ in ((w1af, x2h), (w1bf, x2h), (w1af, x2l)):
                nc.tensor.matmul(
                    ps[:], wt[:, 2 * g : 2 * g + 2, :],
                    xt_[:, 2 * g : 2 * g + 2, :],
                    start=(g == 0 and wt is w1af and xt_ is x2h),
                    stop=(g == KE // 2 - 1 and wt is w1af and xt_ is x2l),
                    perf_mode=DR,
                )
        h1bf = p_hbf.tile([P, Tq], BF16, name="h1bf")
        nc.scalar.activation(
            h1bf[:], ps[:], AF.Relu, bias=gbt["mb1"][:, f : f + 1], scale=DS
        )
        nc.vector.tensor_scalar_mul(h1h[:, f, :], h1bf[:], SX)
        nc.vector.scalar_tensor_tensor(
            h1l[:, f, :], h1bf[:], SX, h1h[:, f, :], OP.mult, OP.subtract
        )
        if f % 2 == 1:
            mlp2_acc(f // 2, range(EH), h2a, first=(f == 1), last=(f == KF - 1))
    for e in range(EH):
        of = p_out.tile([P, Tq], F32, name="outf")
        nc.vector.scalar_tensor_tensor(
            of[:], h2a[e][:], DS, xres[e][:], OP.mult, OP.add
        )
        nc.sync.dma_start(d["out_t"][e * P : (e + 1) * P, :], of[:])
    pp.close("h2a_ps", "h1_ps")

    ps_h2b = pp.open("h2b_ps", 1, "PSUM")
    h2b = [ps_h2b.tile([P, Tq], F32, name=f"h2b{i}") for i in range(KE - EH)]
    for g in range(KF // 2):
        mlp2_acc(g, range(EH, KE), h2b, first=(g == 0), last=(g == KF // 2 - 1))
    for i, e in enumerate(range(EH, KE)):
        of = p_out.tile([P, Tq], F32, name="outf")
        nc.vector.scalar_tensor_tensor(
            of[:], h2b[i][:], DS, xres[e][:], OP.mult, OP.add
        )
        nc.sync.dma_start(d["out_t"][e * P : (e + 1) * P, :], of[:])

    pp.close_all()


def build_program(c: Cfg = CFG):
    c.check()
    nc = bacc.Bacc(
        "TRN2",
        target_bir_lowering=False,
        debug=False,
        enable_asserts=False,
        num_devices=c.NC,
    )
    d = {}
    d["xt"] = nc.dram_tensor("xt", [c.E, c.T], BF16, kind="ExternalInput").ap()
    for nm in ("wq8", "wk8", "wv8", "wo8"):
        d[nm] = nc.dram_tensor(nm, [c.E, c.E], F8, kind="ExternalInput").ap()
    d["w1a"] = nc.dram_tensor("w1a", [c.E, c.F], F8, kind="ExternalInput").ap()
    d["w1b"] = nc.dram_tensor("w1b", [c.E, c.F], F8, kind="ExternalInput").ap()
    d["w2a"] = nc.dram_tensor("w2a", [c.F, c.E], F8, kind="ExternalInput").ap()
    d["w2b"] = nc.dram_tensor("w2b", [c.F, c.E], F8, kind="ExternalInput").ap()
    d["mb2dr"] = nc.dram_tensor("mb2dr", [P, 2 * c.E], F8, kind="ExternalInput").ap()
    for nm, cols in [("bq", c.KE), ("bk", c.KE), ("bv32", c.KE), ("boc", c.KE),
                     ("mb1", c.KF)]:
        d[nm] = nc.dram_tensor(nm, [P, cols], F32, kind="ExternalInput").ap()
    d["band"] = nc.dram_tensor("band", [P, c.NQC * 2 * P], BF16,
                               kind="ExternalInput").ap()
    d["ident"] = nc.dram_tensor("ident", [P, P], BF16, kind="ExternalInput").ap()
    d["out_t"] = nc.dram_tensor("out_t", [c.E, c.Tq], F32, kind="ExternalOutput").ap()
    if DEBUG:
        d["dbg_xn"] = nc.dram_tensor("dbg_xn", [P, c.KE, c.T], F8,
                                     kind="ExternalOutput").ap()
        d["dbg_qt"] = nc.dram_tensor("dbg_qt", [P, c.HP * c.Tq], BF16,
                                     kind="ExternalOutput").ap()
        d["dbg_kt"] = nc.dram_tensor("dbg_kt", [P, c.HP * c.T], BF16,
                                     kind="ExternalOutput").ap()
        d["dbg_vs"] = nc.dram_tensor("dbg_vs", [P, c.TK * 1040], BF16,
                                     kind="ExternalOutput").ap()
        d["dbg_ao"] = nc.dram_tensor("dbg_ao", [P, c.HP, c.Tq], F8,
                                     kind="ExternalOutput").ap()
        d["dbg_rs"] = nc.dram_tensor("dbg_rs", [P, c.HP, 2, 4], F32,
                                     kind="ExternalOutput").ap()
        d["dbg_irs"] = nc.dram_tensor("dbg_irs", [P, c.HP, 2, 4], F32,
                                      kind="ExternalOutput").ap()
        d["dbg_pt"] = nc.dram_tensor("dbg_pt", [P, c.HP, 2, 256], BF16,
                                     kind="ExternalOutput").ap()

    with tile.TileContext(nc) as tc:
        _emit(tc, c, d)
    nc.compile()
    return nc


# --------------------------------------------------------------------------
# host side
# --------------------------------------------------------------------------
def shard_inputs(inputs, c: Cfg = CFG):
    f32 = lambda a: np.ascontiguousarray(np.asarray(a, np.float32))
    x = f32(inputs["x"])
    g1, b1n = f32(inputs["ln1_g"]), f32(inputs["ln1_b"])
    g2, b2n = f32(inputs["ln2_g"]), f32(inputs["ln2_b"])
    Wq, Wk, Wv, Wo = (f32(inputs[k]) for k in ("Wq", "Wk", "Wv", "Wo"))
    W1, W2 = f32(inputs["W1"]), f32(inputs["W2"])
    bo, b1, b2 = f32(inputs["bo"]), f32(inputs["b1"]), f32(inputs["b2"])

    q8 = lambda w: np.ascontiguousarray((w * SW)).astype(NPF8)
    bf = lambda w: np.ascontiguousarray(w).astype(NPBF16)
    chunks = lambda v, k: np.ascontiguousarray(v.reshape(k, P).T)

    def split8(w):
        ws = np.ascontiguousarray(w * SW)
        a = ws.astype(NPF8)
        b = (ws - a.astype(np.float32)).astype(NPF8)
        return a, b

    w1a, w1b = split8(g2[:, None] * W1)
    w2a, w2b = split8(W2)
    com = {
        "wq8": q8(g1[:, None] * Wq),
        "wk8": q8(g1[:, None] * Wk),
        "wv8": q8(g1[:, None] * Wv),
        "wo8": q8(Wo),
        "w1a": w1a, "w1b": w1b,
        "w2a": w2a, "w2b": w2b,
        "mb2dr": np.broadcast_to(
            (b2 * SX).astype(NPF8), (P, 2, c.E)
        ).reshape(P, 2 * c.E).copy(),
        "bq": chunks(b1n @ Wq, c.KE),
        "bk": chunks(b1n @ Wk, c.KE),
        "bv32": chunks((b1n @ Wv) * SX, c.KE),
        "boc": chunks(bo, c.KE),
        "mb1": chunks(b1 + b2n @ W1, c.KF),
        "ident": np.eye(P, dtype=np.float32).astype(NPBF16),
    }

    p_idx = np.arange(P)[:, None]
    lq_idx = np.arange(P)[None, :]
    maps = []
    for core in range(c.NC):
        b, j = core // c.CPB, core % c.CPB
        phases = [(j + 1) % 4, (j + 2) % 4, (j + 3) % 4, j]
        ctx = np.empty((c.E, c.T), np.float32)
        for i, ph in enumerate(phases):
            ctx[:, 512 * i : 512 * (i + 1)] = x[b, ph::4, :].T
        band = np.zeros((P, c.NQC, 2, P), np.float32)
        for i, ph in enumerate(phases):
            delta = 1 if ph > j else 0
            m_ = (p_idx <= lq_idx - delta).astype(np.float32)
            band[:, i, 0, :] = m_
            band[:, i, 1, :] = m_
        m = dict(com)
        m["xt"] = bf(ctx)
        m["band"] = band.reshape(P, c.NQC * 2 * P).astype(NPBF16)
        maps.append(m)
    return maps


def assemble(results, c: Cfg = CFG):
    out = np.empty((c.B, c.T, c.E), np.float32)
    for core in range(c.NC):
        b, j = core // c.CPB, core % c.CPB
        out[b, j::4, :] = results[core]["out_t"].T
    return out


_NC_CACHE = {}


def _get_nc(c: Cfg = CFG):
    if c not in _NC_CACHE:
        _NC_CACHE[c] = build_program(c)
    return _NC_CACHE[c]


LAST_RESULT = None


def kernel(**inputs):
    global LAST_RESULT
    c = CFG
    nc = _get_nc(c)
    maps = shard_inputs(inputs, c)
    res = bass_utils.run_bass_kernel_spmd(nc, maps, core_ids=list(range(c.NC)))
    LAST_RESULT = res
    return assemble(res.results, c)


# revision 59
# speedup vs baseline: 1.8176x; 1.0429x over previous
"""Trainium2 Bass kernel for a pre-norm transformer block (dense_transformer).

Computation (per reference):
    x = x + Attn(LN1(x));  x = x + MLP(LN2(x))
with causal multi-head attention (H=16 heads, D=64) and a 4E ReLU MLP.

Sharding: 8 cores = 2 batches x 4 query PHASES.  Core (b, j) owns the 512
query tokens {4r + j}.  The context (all 2048 tokens) is column-PERMUTED
per core so the core's own phase sits last: position 512*i + r holds token
4r + phase_i with phase order (j+1, j+2, j+3, j) mod 4.  With this striping
the causal block structure is identical on every core (SPMD uniform): query
tile m attends context tiles t with t%4 <= m, so upper score tiles are
skipped for real FLOP savings; the diagonal band mask is a per-core input.

Precision: QKV and output projections run in fp8e4 DoubleRow (2x PE perf,
K=256 per instruction) with power-of-2 scales folded into the weights and
descale factors folded into existing copies; LN gains/biases are folded
into the weights host-side.  Scores/attnV/MLP stay bf16 (error budget).

attnV runs "query-major": out[q, d] with a ones column appended to V, so
softmax row-sums accumulate for free in PSUM column 64; normalization is a
per-partition scalar multiply, then a PE transpose returns to feature-major
for the fp8 out-projection.
"""

from dataclasses import dataclass

import numpy as np
import ml_dtypes

import concourse.bass as bass  # noqa: F401
import concourse.mybir as mybir
import concourse.tile as tile
from concourse import bacc
from concourse import bass_utils

F32 = mybir.dt.float32
BF16 = mybir.dt.bfloat16
F8 = mybir.dt.float8e4
AF = mybir.ActivationFunctionType
OP = mybir.AluOpType
DR = mybir.MatmulPerfMode.DoubleRow
NPBF16 = ml_dtypes.bfloat16
NPF8 = ml_dtypes.float8_e4m3

P = 128
SX = 32.0       # fp8 scale on activations
SW = 256.0      # fp8 scale on weights
DS = 1.0 / (SX * SW)


@dataclass(frozen=True)
class Cfg:
    B: int = 2
    T: int = 2048
    E: int = 1024
    H: int = 16
    D: int = 64
    NC: int = 8
    eps: float = 1e-5

    @property
    def CPB(self):
        return self.NC // self.B

    @property
    def Tq(self):
        return self.T // self.CPB

    @property
    def KE(self):
        return self.E // P

    @property
    def TK(self):
        return self.T // P

    @property
    def HP(self):
        return self.H // 2

    @property
    def F(self):
        return 4 * self.E

    @property
    def KF(self):
        return self.F // P

    @property
    def TCH(self):
        return 512

    @property
    def NQC(self):
        return self.T // self.TCH

    @property
    def NM(self):
        return self.Tq // P  # query tiles per core

    def check(self):
        assert self.D == 64 and self.E == self.H * self.D
        assert self.Tq == 512 and self.KE == 8 and self.TK == 16
        assert self.CPB == 4 and self.HP == 8 and self.KF == 32


CFG = Cfg()
DEBUG = False


class Pools:
    def __init__(self, tc, prefix=""):
        self.tc = tc
        self.prefix = prefix
        self.live = {}

    def open(self, key, bufs, space=None, side=None):
        kw = dict(name=self.prefix + key, bufs=bufs)
        if space:
            kw["space"] = space
        if side:
            kw["side"] = side
        cm = self.tc.tile_pool(**kw)
        pool = cm.__enter__()
        self.live[key] = cm
        return pool

    def close(self, *keys):
        for key in keys:
            self.live.pop(key).__exit__(None, None, None)

    def close_all(self):
        for key in reversed(list(self.live)):
            self.close(key)


def _emit(tc, c: Cfg, d):
    nc = tc.nc
    E, T, Tq, H, D = c.E, c.T, c.Tq, c.H, c.D
    KE, TK, HP, KF, NM = c.KE, c.TK, c.HP, c.KF, c.NM
    TCH, NQC = c.TCH, c.NQC
    SCL = 1.0 / float(np.sqrt(D))

    pp = Pools(tc)

    # ---------------- constants ----------------
    const = pp.open("const", 1)
    ones_bf = const.tile([P, 1], BF16, name="ones_bf")
    nc.vector.memset(ones_bf[:], 1.0)
    ones_f1 = const.tile([1, P], F32, name="ones_f1")
    nc.vector.memset(ones_f1[:], 1.0)
    ident = const.tile([P, P], BF16, name="ident")
    band = const.tile([P, NQC, 2, P], BF16, name="band")
    gbt = {}
    for nm, cols in [("bq", KE), ("bk", KE), ("bv32", KE), ("boc", KE),
                     ("mb1", KF)]:
        gbt[nm] = const.tile([P, cols], F32, name=nm + "_t")
        nc.sync.dma_start(gbt[nm][:], d[nm])

    # ---------------- weights (fp8, resident) ----------------
    p_w8 = pp.open("w8", 1)
    wk8 = p_w8.tile([P, KE, E], F8, name="wk8")
    wo8 = p_w8.tile([P, KE, E], F8, name="wo8")

    # ---------------- warmup (PE p-state ramp; no DMA dependency) ----------
    p_wsb = pp.open("warm_sb", 1, side="right")
    ps_wm = pp.open("warm_ps", 1, "PSUM")
    wsb = p_wsb.tile([P, TCH], BF16, name="wsb")
    nc.vector.memset(wsb[:], 0.0)
    wmp = ps_wm.tile([1, TCH], F32, name="wmp")
    for _w in range(8):
        nc.tensor.matmul(wmp[:], ones_bf[:], wsb[:], start=True, stop=True)
    pp.close("warm_ps", "warm_sb")

    # ---------------- long-lived activations ----------------
    # left stack: const, w8, xqp | w8b, xtp, LN pools (freed) | w2p, phase3/4
    # right stack: aop | xnp, ktp, qtp, vsp (freed after attention), attn pools
    p_xq = pp.open("xqp", 1)
    xq = [p_xq.tile([P, Tq], F32, name=f"xq{e}") for e in range(KE)]
    p_ao = pp.open("aop", 1, side="right")
    aop8 = p_ao.tile([P, HP, Tq], F8, name="aop8")
    p_xn = pp.open("xnp", 1, side="right")
    xn8 = p_xn.tile([P, KE, T], F8, name="xn8")
    p_kt = pp.open("ktp", 1, side="right")
    kt = [p_kt.tile([P, T], BF16, name=f"kt{j}") for j in range(HP)]
    p_qt = pp.open("qtp", 1, side="right")
    qt = [p_qt.tile([P, Tq], BF16, name=f"qt{j}") for j in range(HP)]
    p_vs = pp.open("vsp", 1, side="right")
    vsb = [p_vs.tile([P, H * (D + 1)], BF16, name=f"vsb{t}") for t in range(TK)]

    # ======================================================================
    # Phase 1: x load + LN1 + QKV (chunk-interleaved)
    # ======================================================================
    p_w8b = pp.open("w8b", 1)
    wq8 = p_w8b.tile([P, KE, E], F8, name="wq8")
    wv8 = p_w8b.tile([P, KE, E], F8, name="wv8")

    p_xt = pp.open("xtp", 1)
    xt = [p_xt.tile([P, T], BF16, name=f"xt{e}") for e in range(KE)]
    # DMA priority order: xt (stats start on it), then K/V weights (used from
    # chunk 0), then Q (chunk 3) and O/ident/band (attention phase).
    for e in range(KE):
        nc.sync.dma_start(xt[e][:], d["xt"][e * P : (e + 1) * P, :])
    for nm, t_ in [("wv8", wv8), ("wk8", wk8), ("wq8", wq8), ("wo8", wo8)]:
        nc.sync.dma_start(t_[:], d[nm].rearrange("(e p) m -> p e m", p=P))
    nc.sync.dma_start(ident[:], d["ident"])
    nc.sync.dma_start(band[:], d["band"].rearrange("p (i s q) -> p i s q",
                                                   i=NQC, s=2))

    # ones columns of V (col 64 of each head slot)
    for t in range(TK):
        nc.vector.memset(vsb[t][:, :].rearrange("p (h d) -> p h d", d=D + 1)[:, :, D], 1.0)

    p_tmp = pp.open("ln_tmp", 3)
    p_rows = pp.open("ln_rows", 1)
    ps_st = pp.open("ln_st", 2, "PSUM")
    ps_bc = pp.open("ln_bc", 1, "PSUM")
    ps_qkv = pp.open("qkv_ps", 2, "PSUM")

    def k_proj(j, psum_pool, cis, nm="qkv", shp=None, eng="act"):
        for ci in cis:
            cs = slice(ci * TCH, (ci + 1) * TCH)
            pst = psum_pool.tile(shp or [P, TCH], F32, name=nm)
            ps = pst[:, 0, :] if shp else pst[:]
            for g in range(KE // 2):
                nc.tensor.matmul(
                    ps, wk8[:, 2 * g : 2 * g + 2, j * P : (j + 1) * P],
                    xn8[:, 2 * g : 2 * g + 2, cs],
                    start=(g == 0), stop=(g == KE // 2 - 1), perf_mode=DR,
                )
            if eng == "act":
                nc.scalar.activation(
                    kt[j][:, cs], ps, AF.Identity,
                    bias=gbt["bk"][:, j : j + 1], scale=DS,
                )
            else:
                nc.vector.tensor_scalar(
                    kt[j][:, cs], ps, DS, gbt["bk"][:, j : j + 1],
                    OP.mult, OP.add,
                )

    def v_proj_chunk(ci):
        for tt in range(4 * ci, 4 * ci + 4):
            for hf in range(2):
                ps = ps_qkv.tile([P, KE, D], F32, name="qkv")
                for g in range(KE // 2):
                    nc.tensor.matmul(
                        ps[:], xn8[:, 2 * g : 2 * g + 2, tt * P : (tt + 1) * P],
                        wv8[:, 2 * g : 2 * g + 2, hf * 512 : hf * 512 + 512],
                        start=(g == 0), stop=(g == KE // 2 - 1), perf_mode=DR,
                    )
                dst = vsb[tt][:, hf * 8 * (D + 1) :].rearrange(
                    "p (h d) -> p h d", d=D + 1
                )[:, 0:8, 0:D]
                if tt % 2 == 0:
                    nc.scalar.activation(dst, ps[:], AF.Copy, scale=DS)
                else:
                    nc.vector.tensor_scalar_mul(dst, ps[:], DS)

    def q_proj(j):
        ps = ps_qkv.tile([P, Tq], F32, name="qkv")
        for g in range(KE // 2):
            nc.tensor.matmul(
                ps[:], wq8[:, 2 * g : 2 * g + 2, j * P : (j + 1) * P],
                xn8[:, 2 * g : 2 * g + 2, T - Tq :],
                start=(g == 0), stop=(g == KE // 2 - 1), perf_mode=DR,
            )
        nc.vector.tensor_scalar(
            qt[j][:], ps[:], DS, gbt["bq"][:, j : j + 1], OP.mult, OP.add
        )

    def ln_stats(ci):
        """Stats matmuls + row math + broadcast + SBUF stage for chunk ci.
        Returns (mub_sb, rsb_sb)."""
        cs = slice(ci * TCH, (ci + 1) * TCH)
        s1 = ps_st.tile([1, TCH], F32, name="s1")
        s2 = ps_st.tile([1, TCH], F32, name="s2")
        for e in range(KE):
            x2 = p_tmp.tile([P, TCH], BF16, name="x2bf")
            nc.scalar.square(x2[:], xt[e][:, cs])
            nc.tensor.matmul(s1[:], ones_bf[:], xt[e][:, cs],
                             start=(e == 0), stop=(e == KE - 1))
            nc.tensor.matmul(s2[:], ones_bf[:], x2[:],
                             start=(e == 0), stop=(e == KE - 1))
        mu = p_rows.tile([1, TCH], F32, name="mu")
        nc.vector.tensor_scalar_mul(mu[:], s1[:], 1.0 / E)
        ve = p_rows.tile([1, TCH], F32, name="ve")
        nc.vector.tensor_scalar(ve[:], s2[:], 1.0 / E, c.eps, OP.mult, OP.add)
        mu2 = p_rows.tile([1, TCH], F32, name="mu2")
        nc.vector.tensor_tensor(mu2[:], mu[:], mu[:], OP.mult)
        vee = p_rows.tile([1, TCH], F32, name="vee")
        nc.vector.tensor_tensor(vee[:], ve[:], mu2[:], OP.subtract)
        # rstd*SX: sqrt(vee/SX^2) on ACT (stays in the sqrt table set),
        # then fast reciprocal on DVE -> SX/sqrt(vee)
        sq = p_rows.tile([1, TCH], F32, name="sq")
        nc.scalar.activation(sq[:], vee[:], AF.Sqrt, scale=1.0 / (SX * SX))
        rstd32 = p_rows.tile([1, TCH], F32, name="rstd32")
        nc.vector.reciprocal_approx_fast(rstd32[:], sq[:])

        mub = ps_bc.tile([P, TCH], F32, name="mub")
        nc.tensor.matmul(mub[:], ones_f1[:], mu[:], start=True, stop=True)
        rsb = ps_bc.tile([P, TCH], F32, name="rsb")
        nc.tensor.matmul(rsb[:], ones_f1[:], rstd32[:], start=True, stop=True)
        mub_sb = p_tmp.tile([P, TCH], BF16, name="mub_sb")
        nc.vector.tensor_copy(mub_sb[:], mub[:])
        rsb_sb = p_tmp.tile([P, TCH], BF16, name="rsb_sb")
        nc.vector.tensor_copy(rsb_sb[:], rsb[:])
        return mub_sb, rsb_sb

    # software-pipelined: stats(ci+1) emitted before projections(ci) so the
    # PE has stat matmuls to chew while DVE/Pool normalize chunk ci
    stg = ln_stats(0)
    for ci in range(NQC):
        cs = slice(ci * TCH, (ci + 1) * TCH)
        mub_sb, rsb_sb = stg
        for e in range(KE):
            t1 = p_tmp.tile([P, TCH], BF16, name="t1")
            nc.gpsimd.tensor_tensor(t1[:], xt[e][:, cs], mub_sb[:], OP.subtract)
            nc.vector.tensor_tensor(xn8[:, e, cs], t1[:], rsb_sb[:], OP.mult)
        if ci + 1 < NQC:
            stg = ln_stats(ci + 1)

        # residual extraction for the query chunk (last chunk): xq = x + bo
        if ci == NQC - 1:
            for e in range(KE):
                nc.vector.tensor_scalar(
                    xq[e][:], xt[e][:, T - Tq :], gbt["boc"][:, e : e + 1],
                    None, OP.add,
                )

        # interleaved QKV for this chunk
        k_proj(0, ps_qkv, [ci])
        k_proj(1, ps_qkv, [ci])
        v_proj_chunk(ci)
        if ci == NQC - 1:
            for j in range(HP):
                q_proj(j)

    pp.close("qkv_ps", "ln_bc", "ln_st", "ln_rows", "ln_tmp", "xtp", "w8b")

    if DEBUG:
        nc.sync.dma_start(d["dbg_xn"], xn8[:])
        for j in range(HP):
            nc.sync.dma_start(d["dbg_qt"][:, j * Tq : (j + 1) * Tq], qt[j][:])
        for t in range(TK):
            nc.sync.dma_start(
                d["dbg_vs"][:, t * 1040 : (t + 1) * 1040], vsb[t][:]
            )

    # w2 resident load (xt freed now; lands during attention)
    p_w2 = pp.open("w2p", 1)
    w2a = p_w2.tile([P, KF, E], F8, name="w2a")
    w2b = p_w2.tile([P, KF, E], F8, name="w2b")
    mb2dr = p_w2.tile([P, 2, E], F8, name="mb2dr")
    ones8 = p_w2.tile([P, 2, Tq], F8, name="ones8")
    nc.vector.memset(ones8[:], 1.0)
    nc.sync.dma_start(mb2dr[:], d["mb2dr"].rearrange("p (k m) -> p k m", k=2))
    for nm, t_ in (("w2a", w2a), ("w2b", w2b)):
        for fq in range(4):
            nc.sync.dma_start(
                t_[:, 8 * fq : 8 * fq + 8, :],
                d[nm].rearrange("(f p) m -> p f m", p=P)[:, 8 * fq : 8 * fq + 8, :],
            )

    # ======================================================================
    # Phase 2: attention, head-pair at a time, query-major attnV
    # ======================================================================
    ss_p = pp.open("ss_ps", 2, "PSUM")
    oh_p = pp.open("oh_ps", 1, "PSUM")
    tp_p = pp.open("tp_ps", 1, "PSUM")
    p_pr = pp.open("prp", 2, side="right")
    p_tail = pp.open("tailp", 2, side="right")

    for j in range(HP):
        ohq = oh_p.tile([P, 2, 512], F32, name="ohq")  # per s: 4*65 used
        for t in range(TK):
            u = t % 4
            i = t // 4
            ss = ss_p.tile([P, 2, Tq], F32, name="ss")
            for s in (0, 1):
                nc.tensor.matmul(
                    ss[:, s, u * P : Tq],
                    kt[j][s * 64 : (s + 1) * 64, t * P : (t + 1) * P],
                    qt[j][s * 64 : (s + 1) * 64, u * P : Tq],
                    start=True, stop=True, tile_position=(s * 64, 0),
                )
            pr = p_pr.tile([P, 2, Tq], BF16, name="pr")
            nc.scalar.activation(pr[:, :, u * P : Tq], ss[:, :, u * P : Tq],
                                 AF.Exp, scale=SCL)
            nc.gpsimd.tensor_tensor(
                pr[:, :, u * P : (u + 1) * P], pr[:, :, u * P : (u + 1) * P],
                band[:, i, :, :], OP.mult,
            )
            for m in range(u, NM):
                for s in (0, 1):
                    h = 2 * j + s
                    # start=True resets the whole PSUM bank: only the first
                    # matmul into each bank (m==0 at t==0) may set it; later
                    # groups accumulate onto the bank-wide zero.
                    nc.tensor.matmul(
                        ohq[:, s, m * 65 : m * 65 + 65],
                        pr[:, s, m * P : (m + 1) * P],
                        vsb[t][:, h * (D + 1) : (h + 1) * (D + 1)],
                        start=(t == 0 and m == 0), stop=(t == 12 + m),
                        skip_group_check=True,
                    )
        # next pair's K projection first: PE work to overlap the tail below
        if j + 2 < HP:
            k_proj(j + 2, ss_p, range(NQC), nm="ss", shp=[P, 2, Tq], eng="dve")
        # tail: row-sums -> 1/rs -> normalize -> transpose -> fp8 quantize
        rs_sb = p_tail.tile([P, 2, NM], F32, name="rs_sb")
        for s in (0, 1):
            for m in range(NM):
                nc.vector.tensor_copy(rs_sb[:, s, m : m + 1],
                                      ohq[:, s, m * 65 + 64 : m * 65 + 65])
        irs = p_tail.tile([P, 2, NM], F32, name="irs")
        nc.vector.reciprocal(irs[:], rs_sb[:])
        ptb = p_tail.tile([P, 2, 256], BF16, name="ptb")
        for s in (0, 1):
            for m in range(NM):
                nc.vector.tensor_scalar_mul(
                    ptb[:, s, m * 64 : (m + 1) * 64],
                    ohq[:, s, m * 65 : m * 65 + 64],
                    irs[:, s, m : m + 1],
                )
        for m in range(NM):
            tp = tp_p.tile([P, P], BF16, name="tp")
            nc.tensor.transpose(tp[0:64, :], ptb[:, 0, m * 64 : (m + 1) * 64],
                                ident[:], tile_position=(0, 0))
            nc.tensor.transpose(tp[64:128, :], ptb[:, 1, m * 64 : (m + 1) * 64],
                                ident[:], tile_position=(0, 64))
            nc.scalar.activation(
                aop8[:, j, m * P : (m + 1) * P], tp[:], AF.Identity,
                bias=gbt["bv32"][:, j : j + 1], scale=SX,
            )
        if DEBUG:
            nc.sync.dma_start(d["dbg_rs"][:, j, :, :], rs_sb[:])
            nc.sync.dma_start(d["dbg_irs"][:, j, :, :], irs[:])
            nc.sync.dma_start(d["dbg_pt"][:, j, :, :], ptb[:])

    if DEBUG:
        nc.sync.dma_start(d["dbg_ao"], aop8[:])
        for j in range(HP):
            nc.sync.dma_start(d["dbg_kt"][:, j * T : (j + 1) * T], kt[j][:])
    pp.close("tailp", "prp", "tp_ps", "oh_ps", "ss_ps")
    pp.close("vsp", "qtp", "ktp", "xnp")

    # ======================================================================
    # Phase 3: out-projection (fp8) + residual; LayerNorm2
    # ======================================================================
    p_xr = pp.open("xrp", 1)
    p_x2 = pp.open("xn2p", 1)
    ps_ao = pp.open("ao_ps", 2, "PSUM")
    xres = [p_xr.tile([P, Tq], F32, name=f"xres{e}") for e in range(KE)]
    x2h = p_x2.tile([P, KE, Tq], F8, name="x2h")
    x2l = p_x2.tile([P, KE, Tq], F8, name="x2l")

    for e in range(KE):
        ps = ps_ao.tile([P, Tq], F32, name="aops")
        for g in range(KE // 2):
            nc.tensor.matmul(
                ps[:], wo8[:, 2 * g : 2 * g + 2, e * P : (e + 1) * P],
                aop8[:, 2 * g : 2 * g + 2, :],
                start=(g == 0), stop=(g == KE // 2 - 1), perf_mode=DR,
            )
        nc.vector.scalar_tensor_tensor(
            xres[e][:], ps[:], DS, xq[e][:], OP.mult, OP.add
        )
    pp.close("ao_ps", "aop")

    p_tmp = pp.open("ln2_tmp", 3)
    p_rows = pp.open("ln2_rows", 1)
    ps_st = pp.open("ln2_st", 1, "PSUM")
    ps_bc = pp.open("ln2_bc", 1, "PSUM")
    s1 = ps_st.tile([1, Tq], F32, name="s1b")
    s2 = ps_st.tile([1, Tq], F32, name="s2b")
    for e in range(KE):
        xbf = p_tmp.tile([P, Tq], BF16, name="xbf2")
        nc.scalar.activation(xbf[:], xres[e][:], AF.Copy)
        x2 = p_tmp.tile([P, Tq], BF16, name="x2bf2")
        nc.vector.tensor_tensor(x2[:], xbf[:], xbf[:], OP.mult)
        nc.tensor.matmul(s1[:], ones_bf[:], xbf[:], start=(e == 0), stop=(e == KE - 1))
        nc.tensor.matmul(s2[:], ones_bf[:], x2[:], start=(e == 0), stop=(e == KE - 1))
    mu = p_rows.tile([1, Tq], F32, name="mu_2")
    nc.vector.tensor_scalar_mul(mu[:], s1[:], 1.0 / E)
    ve = p_rows.tile([1, Tq], F32, name="ve_2")
    nc.vector.tensor_scalar(ve[:], s2[:], 1.0 / E, c.eps, OP.mult, OP.add)
    mu2 = p_rows.tile([1, Tq], F32, name="mu2_2")
    nc.vector.tensor_tensor(mu2[:], mu[:], mu[:], OP.mult)
    vee = p_rows.tile([1, Tq], F32, name="vee_2")
    nc.vector.tensor_tensor(vee[:], ve[:], mu2[:], OP.subtract)
    sq2 = p_rows.tile([1, Tq], F32, name="sq_2")
    nc.scalar.activation(sq2[:], vee[:], AF.Sqrt, scale=1.0 / (SX * SX))
    rstd = p_rows.tile([1, Tq], F32, name="rstd_2")
    nc.vector.reciprocal_approx_fast(rstd[:], sq2[:])
    mub = ps_bc.tile([P, Tq], F32, name="mub2")
    nc.tensor.matmul(mub[:], ones_f1[:], mu[:], start=True, stop=True)
    rsb = ps_bc.tile([P, Tq], F32, name="rsb2")
    nc.tensor.matmul(rsb[:], ones_f1[:], rstd[:], start=True, stop=True)
    mub_sb = p_tmp.tile([P, Tq], BF16, name="mub2_sb")
    nc.vector.tensor_copy(mub_sb[:], mub[:])
    rsb_sb = p_tmp.tile([P, Tq], BF16, name="rsb2_sb")
    nc.vector.tensor_copy(rsb_sb[:], rsb[:])
    # xn2 (scaled by SX) split into fp8 head + fp8 residual for DoubleRow MLP1
    for e in range(KE):
        t1 = p_tmp.tile([P, Tq], BF16, name="t1b")
        nc.gpsimd.tensor_tensor(t1[:], xres[e][:], mub_sb[:], OP.subtract)
        xn2bf = p_tmp.tile([P, Tq], BF16, name="xn2bf")
        nc.vector.tensor_tensor(xn2bf[:], t1[:], rsb_sb[:], OP.mult)
        nc.scalar.activation(x2h[:, e, :], xn2bf[:], AF.Copy)
        nc.vector.tensor_tensor(x2l[:, e, :], xn2bf[:], x2h[:, e, :], OP.subtract)
    pp.close("ln2_rows", "ln2_tmp", "ln2_bc", "ln2_st")

    # ======================================================================
    # Phase 4: MLP in split-fp8 DoubleRow: X~H+L, W~A+B (fp8 residuals);
    # X@W ~ H@A + H@B + L@A at 0.75x the bf16 PE cost, ~bf16 accuracy.
    # ======================================================================
    EH = 6
    p_h1 = pp.open("h1p", 1, side="right")
    p_hbf = pp.open("h1bfp", 3, side="right")
    p_out = pp.open("outp", 2)
    p_w1 = pp.open("w1s", 3)
    ps_h1 = pp.open("h1_ps", 2, "PSUM")
    ps_h2a = pp.open("h2a_ps", 1, "PSUM")

    h1h = p_h1.tile([P, KF, Tq], F8, name="h1h")
    h1l = p_h1.tile([P, KF, Tq], F8, name="h1l")
    h2a = [ps_h2a.tile([P, Tq], F32, name=f"h2a{e}") for e in range(EH)]

    def mlp2_acc(g, es, h2ps, first, last):
        for i, e in enumerate(es):
            for wt, ht in ((w2a, h1h), (w2b, h1h), (w2a, h1l)):
                fst = first and wt is w2a and ht is h1h
                lst = last and wt is w2a and ht is h1l
                nc.tensor.matmul(
                    h2ps[i][:], wt[:, 2 * g : 2 * g + 2, e * P : (e + 1) * P],
                    ht[:, 2 * g : 2 * g + 2, :],
                    start=fst, stop=False, perf_mode=DR, skip_group_check=True,
                )
            if last:
                # += b2*8192 via ones rhs, then stop the group
                nc.tensor.matmul(
                    h2ps[i][:], mb2dr[:, :, e * P : (e + 1) * P], ones8[:],
                    start=False, stop=True, perf_mode=DR, skip_group_check=True,
                )

    for f in range(KF):
        w1af = p_w1.tile([P, KE, P], F8, name="w1a")
        w1bf = p_w1.tile([P, KE, P], F8, name="w1b")
        nc.sync.dma_start(
            w1af[:],
            d["w1a"].rearrange("(e p) m -> p e m", p=P)[:, :, f * P : (f + 1) * P],
        )
        nc.sync.dma_start(
            w1bf[:],
            d["w1b"].rearrange("(e p) m -> p e m", p=P)[:, :, f * P : (f + 1) * P],
        )
        ps = ps_h1.tile([P, Tq], F32, name="h1ps")
        for g in range(KE // 2):
            for wt, xt_ in ((w1af, x2h), (w1bf, x2h), (w1af, x2l)):
                nc.tensor.matmul(
                    ps[:], wt[:, 2 * g : 2 * g + 2, :],
                    xt_[:, 2 * g : 2 * g + 2, :],
                    start=(g == 0 and wt is w1af and xt_ is x2h),
                    stop=(g == KE // 2 - 1 and wt is w1af and xt_ is x2l),
                    perf_mode=DR,
                )
        h1bf = p_hbf.tile([P, Tq], BF16, name="h1bf")
        nc.scalar.activation(
            h1bf[:], ps[:], AF.Relu, bias=gbt["mb1"][:, f : f + 1], scale=DS
        )
        nc.vector.tensor_scalar_mul(h1h[:, f, :], h1bf[:], SX)
        nc.vector.scalar_tensor_tensor(
            h1l[:, f, :], h1bf[:], SX, h1h[:, f, :], OP.mult, OP.subtract
        )
        if f % 2 == 1:
            mlp2_acc(f // 2, range(EH), h2a, first=(f == 1), last=(f == KF - 1))
    for e in range(EH):
        of = p_out.tile([P, Tq], F32, name="outf")
        nc.vector.scalar_tensor_tensor(
            of[:], h2a[e][:], DS, xres[e][:], OP.mult, OP.add
        )
        nc.sync.dma_start(d["out_t"][e * P : (e + 1) * P, :], of[:])
    pp.close("h2a_ps", "h1_ps")

    ps_h2b = pp.open("h2b_ps", 1, "PSUM")
    h2b = [ps_h2b.tile([P, Tq], F32, name=f"h2b{i}") for i in range(KE - EH)]
    for g in range(KF // 2):
        mlp2_acc(g, range(EH, KE), h2b, first=(g == 0), last=(g == KF // 2 - 1))
    for i, e in enumerate(range(EH, KE)):
        of = p_out.tile([P, Tq], F32, name="outf")
        nc.vector.scalar_tensor_tensor(
            of[:], h2b[i][:], DS, xres[e][:], OP.mult, OP.add
        )
        nc.sync.dma_start(d["out_t"][e * P : (e + 1) * P, :], of[:])

    pp.close_all()


def build_program(c: Cfg = CFG):
    c.check()
    nc = bacc.Bacc(
        "TRN2",
        target_bir_lowering=False,
        debug=False,
        enable_asserts=False,
        num_devices=c.NC,
    )
    d = {}
    d["xt"] = nc.dram_tensor("xt", [c.E, c.T], BF16, kind="ExternalInput").ap()
    for nm in ("wq8", "wk8", "wv8", "wo8"):
        d[nm] = nc.dram_tensor(nm, [c.E, c.E], F8, kind="ExternalInput").ap()
    d["w1a"] = nc.dram_tensor("w1a", [c.E, c.F], F8, kind="ExternalInput").ap()
    d["w1b"] = nc.dram_tensor("w1b", [c.E, c.F], F8, kind="ExternalInput").ap()
    d["w2a"] = nc.dram_tensor("w2a", [c.F, c.E], F8, kind="ExternalInput").ap()
    d["w2b"] = nc.dram_tensor("w2b", [c.F, c.E], F8, kind="ExternalInput").ap()
    d["mb2dr"] = nc.dram_tensor("mb2dr", [P, 2 * c.E], F8, kind="ExternalInput").ap()
    for nm, cols in [("bq", c.KE), ("bk", c.KE), ("bv32", c.KE), ("boc", c.KE),
                     ("mb1", c.KF)]:
        d[nm] = nc.dram_tensor(nm, [P, cols], F32, kind="ExternalInput").ap()
    d["band"] = nc.dram_tensor("band", [P, c.NQC * 2 * P], BF16,
                               kind="ExternalInput").ap()
    d["ident"] = nc.dram_tensor("ident", [P, P], BF16, kind="ExternalInput").ap()
    d["out_t"] = nc.dram_tensor("out_t", [c.E, c.Tq], F32, kind="ExternalOutput").ap()
    if DEBUG:
        d["dbg_xn"] = nc.dram_tensor("dbg_xn", [P, c.KE, c.T], F8,
                                     kind="ExternalOutput").ap()
        d["dbg_qt"] = nc.dram_tensor("dbg_qt", [P, c.HP * c.Tq], BF16,
                                     kind="ExternalOutput").ap()
        d["dbg_kt"] = nc.dram_tensor("dbg_kt", [P, c.HP * c.T], BF16,
                                     kind="ExternalOutput").ap()
        d["dbg_vs"] = nc.dram_tensor("dbg_vs", [P, c.TK * 1040], BF16,
                                     kind="ExternalOutput").ap()
        d["dbg_ao"] = nc.dram_tensor("dbg_ao", [P, c.HP, c.Tq], F8,
                                     kind="ExternalOutput").ap()
        d["dbg_rs"] = nc.dram_tensor("dbg_rs", [P, c.HP, 2, 4], F32,
                                     kind="ExternalOutput").ap()
        d["dbg_irs"] = nc.dram_tensor("dbg_irs", [P, c.HP, 2, 4], F32,
                                      kind="ExternalOutput").ap()
        d["dbg_pt"] = nc.dram_tensor("dbg_pt", [P, c.HP, 2, 256], BF16,
                                     kind="ExternalOutput").ap()

    with tile.TileContext(nc) as tc:
        _emit(tc, c, d)
    nc.compile()
    return nc


# --------------------------------------------------------------------------
# host side
# --------------------------------------------------------------------------
def shard_inputs(inputs, c: Cfg = CFG):
    f32 = lambda a: np.ascontiguousarray(np.asarray(a, np.float32))
    x = f32(inputs["x"])
    g1, b1n = f32(inputs["ln1_g"]), f32(inputs["ln1_b"])
    g2, b2n = f32(inputs["ln2_g"]), f32(inputs["ln2_b"])
    Wq, Wk, Wv, Wo = (f32(inputs[k]) for k in ("Wq", "Wk", "Wv", "Wo"))
    W1, W2 = f32(inputs["W1"]), f32(inputs["W2"])
    bo, b1, b2 = f32(inputs["bo"]), f32(inputs["b1"]), f32(inputs["b2"])

    q8 = lambda w: np.ascontiguousarray((w * SW)).astype(NPF8)
    bf = lambda w: np.ascontiguousarray(w).astype(NPBF16)
    chunks = lambda v, k: np.ascontiguousarray(v.reshape(k, P).T)

    def split8(w):
        ws = np.ascontiguousarray(w * SW)
        a = ws.astype(NPF8)
        b = (ws - a.astype(np.float32)).astype(NPF8)
        return a, b

    w1a, w1b = split8(g2[:, None] * W1)
    w2a, w2b = split8(W2)
    com = {
        "wq8": q8(g1[:, None] * Wq),
        "wk8": q8(g1[:, None] * Wk),
        "wv8": q8(g1[:, None] * Wv),
        "wo8": q8(Wo),
        "w1a": w1a, "w1b": w1b,
        "w2a": w2a, "w2b": w2b,
        "mb2dr": np.broadcast_to(
            (b2 * SX).astype(NPF8), (P, 2, c.E)
        ).reshape(P, 2 * c.E).copy(),
        "bq": chunks(b1n @ Wq, c.KE),
        "bk": chunks(b1n @ Wk, c.KE),
        "bv32": chunks((b1n @ Wv) * SX, c.KE),
        "boc": chunks(bo, c.KE),
        "mb1": chunks(b1 + b2n @ W1, c.KF),
        "ident": np.eye(P, dtype=np.float32).astype(NPBF16),
    }

    p_idx = np.arange(P)[:, None]
    lq_idx = np.arange(P)[None, :]
    maps = []
    for core in range(c.NC):
        b, j = core // c.CPB, core % c.CPB
        phases = [(j + 1) % 4, (j + 2) % 4, (j + 3) % 4, j]
        ctx = np.empty((c.E, c.T), np.float32)
        for i, ph in enumerate(phases):
            ctx[:, 512 * i : 512 * (i + 1)] = x[b, ph::4, :].T
        band = np.zeros((P, c.NQC, 2, P), np.float32)
        for i, ph in enumerate(phases):
            delta = 1 if ph > j else 0
            m_ = (p_idx <= lq_idx - delta).astype(np.float32)
            band[:, i, 0, :] = m_
            band[:, i, 1, :] = m_
        m = dict(com)
        m["xt"] = bf(ctx)
        m["band"] = band.reshape(P, c.NQC * 2 * P).astype(NPBF16)
        maps.append(m)
    return maps


def assemble(results, c: Cfg = CFG):
    out = np.empty((c.B, c.T, c.E), np.float32)
    for core in range(c.NC):
        b, j = core // c.CPB, core % c.CPB
        out[b, j::4, :] = results[core]["out_t"].T
    return out


_NC_CACHE = {}


def _get_nc(c: Cfg = CFG):
    if c not in _NC_CACHE:
        _NC_CACHE[c] = build_program(c)
    return _NC_CACHE[c]


LAST_RESULT = None


def kernel(**inputs):
    global LAST_RESULT
    c = CFG
    nc = _get_nc(c)
    maps = shard_inputs(inputs, c)
    res = bass_utils.run_bass_kernel_spmd(nc, maps, core_ids=list(range(c.NC)))
    LAST_RESULT = res
    return assemble(res.results, c)


# revision 67
# speedup vs baseline: 1.8400x; 1.0124x over previous
"""Trainium2 Bass kernel for a pre-norm transformer block (dense_transformer).

Computation (per reference):
    x = x + Attn(LN1(x));  x = x + MLP(LN2(x))
with causal multi-head attention (H=16 heads, D=64) and a 4E ReLU MLP.

Sharding: 8 cores = 2 batches x 4 query PHASES.  Core (b, j) owns the 512
query tokens {4r + j}.  The context (all 2048 tokens) is column-PERMUTED
per core so the core's own phase sits last: position 512*i + r holds token
4r + phase_i with phase order (j+1, j+2, j+3, j) mod 4.  With this striping
the causal block structure is identical on every core (SPMD uniform): query
tile m attends context tiles t with t%4 <= m, so upper score tiles are
skipped for real FLOP savings; the diagonal band mask is a per-core input.

Precision: QKV and output projections run in fp8e4 DoubleRow (2x PE perf,
K=256 per instruction) with power-of-2 scales folded into the weights and
descale factors folded into existing copies; LN gains/biases are folded
into the weights host-side.  Scores/attnV/MLP stay bf16 (error budget).

attnV runs "query-major": out[q, d] with a ones column appended to V, so
softmax row-sums accumulate for free in PSUM column 64; normalization is a
per-partition scalar multiply, then a PE transpose returns to feature-major
for the fp8 out-projection.
"""

from dataclasses import dataclass

import numpy as np
import ml_dtypes

import concourse.bass as bass  # noqa: F401
import concourse.mybir as mybir
import concourse.tile as tile
from concourse import bacc
from concourse import bass_utils

F32 = mybir.dt.float32
BF16 = mybir.dt.bfloat16
F8 = mybir.dt.float8e4
AF = mybir.ActivationFunctionType
OP = mybir.AluOpType
DR = mybir.MatmulPerfMode.DoubleRow
NPBF16 = ml_dtypes.bfloat16
NPF8 = ml_dtypes.float8_e4m3

P = 128
SX = 32.0       # fp8 scale on activations
SW = 256.0      # fp8 scale on weights
DS = 1.0 / (SX * SW)


@dataclass(frozen=True)
class Cfg:
    B: int = 2
    T: int = 2048
    E: int = 1024
    H: int = 16
    D: int = 64
    NC: int = 8
    eps: float = 1e-5

    @property
    def CPB(self):
        return self.NC // self.B

    @property
    def Tq(self):
        return self.T // self.CPB

    @property
    def KE(self):
        return self.E // P

    @property
    def TK(self):
        return self.T // P

    @property
    def HP(self):
        return self.H // 2

    @property
    def F(self):
        return 4 * self.E

    @property
    def KF(self):
        return self.F // P

    @property
    def TCH(self):
        return 512

    @property
    def NQC(self):
        return self.T // self.TCH

    @property
    def NM(self):
        return self.Tq // P  # query tiles per core

    def check(self):
        assert self.D == 64 and self.E == self.H * self.D
        assert self.Tq == 512 and self.KE == 8 and self.TK == 16
        assert self.CPB == 4 and self.HP == 8 and self.KF == 32


CFG = Cfg()
DEBUG = False


class Pools:
    def __init__(self, tc, prefix=""):
        self.tc = tc
        self.prefix = prefix
        self.live = {}

    def open(self, key, bufs, space=None, side=None):
        kw = dict(name=self.prefix + key, bufs=bufs)
        if space:
            kw["space"] = space
        if side:
            kw["side"] = side
        cm = self.tc.tile_pool(**kw)
        pool = cm.__enter__()
        self.live[key] = cm
        return pool

    def close(self, *keys):
        for key in keys:
            self.live.pop(key).__exit__(None, None, None)

    def close_all(self):
        for key in reversed(list(self.live)):
            self.close(key)


def _emit(tc, c: Cfg, d):
    nc = tc.nc
    E, T, Tq, H, D = c.E, c.T, c.Tq, c.H, c.D
    KE, TK, HP, KF, NM = c.KE, c.TK, c.HP, c.KF, c.NM
    TCH, NQC = c.TCH, c.NQC
    SCL = 1.0 / float(np.sqrt(D))

    pp = Pools(tc)

    # ---------------- constants ----------------
    const = pp.open("const", 1)
    ones_bf = const.tile([P, 1], BF16, name="ones_bf")
    nc.vector.memset(ones_bf[:], 1.0)
    ones_f1 = const.tile([1, P], F32, name="ones_f1")
    nc.vector.memset(ones_f1[:], 1.0)
    ident = const.tile([P, P], BF16, name="ident")
    band = const.tile([P, NQC, 2, P], BF16, name="band")
    gbt = {}
    for nm, cols in [("bq", KE), ("bk", KE), ("bv32", KE), ("boc", KE),
                     ("mb1", KF)]:
        gbt[nm] = const.tile([P, cols], F32, name=nm + "_t")
        nc.sync.dma_start(gbt[nm][:], d[nm])

    # ---------------- weights (fp8, resident) ----------------
    p_w8 = pp.open("w8", 1)
    wk8 = p_w8.tile([P, KE, E], F8, name="wk8")
    wo8 = p_w8.tile([P, KE, E], F8, name="wo8")

    # ---------------- warmup (PE p-state ramp; no DMA dependency) ----------
    p_wsb = pp.open("warm_sb", 1, side="right")
    ps_wm = pp.open("warm_ps", 1, "PSUM")
    wsb = p_wsb.tile([P, TCH], BF16, name="wsb")
    nc.vector.memset(wsb[:], 0.0)
    wmp = ps_wm.tile([1, TCH], F32, name="wmp")
    for _w in range(8):
        nc.tensor.matmul(wmp[:], ones_bf[:], wsb[:], start=True, stop=True)
    pp.close("warm_ps", "warm_sb")

    # ---------------- long-lived activations ----------------
    # left stack: const, w8, xqp | w8b, xtp, LN pools (freed) | w2p, phase3/4
    # right stack: aop | xnp, ktp, qtp, vsp (freed after attention), attn pools
    p_xq = pp.open("xqp", 1)
    xq = [p_xq.tile([P, Tq], F32, name=f"xq{e}") for e in range(KE)]
    p_ao = pp.open("aop", 1, side="right")
    aop8 = p_ao.tile([P, HP, Tq], F8, name="aop8")
    p_xn = pp.open("xnp", 1, side="right")
    xn8 = p_xn.tile([P, KE, T], F8, name="xn8")
    p_kt = pp.open("ktp", 1, side="right")
    kt = [p_kt.tile([P, T], BF16, name=f"kt{j}") for j in range(HP)]
    p_qt = pp.open("qtp", 1, side="right")
    qt = [p_qt.tile([P, Tq], BF16, name=f"qt{j}") for j in range(HP)]
    p_vs = pp.open("vsp", 1, side="right")
    vsb = [p_vs.tile([P, H * (D + 1)], BF16, name=f"vsb{t}") for t in range(TK)]

    # ======================================================================
    # Phase 1: x load + LN1 + QKV (chunk-interleaved)
    # ======================================================================
    p_w8b = pp.open("w8b", 1)
    wq8 = p_w8b.tile([P, KE, E], F8, name="wq8")
    wv8 = p_w8b.tile([P, KE, E], F8, name="wv8")

    p_xt = pp.open("xtp", 1)
    xt = [p_xt.tile([P, T], BF16, name=f"xt{e}") for e in range(KE)]
    # DMA priority order: xt (stats start on it), then K/V weights (used from
    # chunk 0), then Q (chunk 3) and O/ident/band (attention phase).
    for e in range(KE):
        nc.sync.dma_start(xt[e][:], d["xt"][e * P : (e + 1) * P, :])
    for nm, t_ in [("wv8", wv8), ("wk8", wk8), ("wq8", wq8), ("wo8", wo8)]:
        nc.sync.dma_start(t_[:], d[nm].rearrange("(e p) m -> p e m", p=P))
    nc.sync.dma_start(ident[:], d["ident"])
    nc.sync.dma_start(band[:], d["band"].rearrange("p (i s q) -> p i s q",
                                                   i=NQC, s=2))

    # ones columns of V (col 64 of each head slot)
    for t in range(TK):
        nc.vector.memset(vsb[t][:, :].rearrange("p (h d) -> p h d", d=D + 1)[:, :, D], 1.0)

    p_tmp = pp.open("ln_tmp", 3)
    p_rows = pp.open("ln_rows", 1)
    ps_st = pp.open("ln_st", 2, "PSUM")
    ps_bc = pp.open("ln_bc", 1, "PSUM")
    ps_qkv = pp.open("qkv_ps", 2, "PSUM")

    def k_proj(j, psum_pool, cis, nm="qkv", shp=None, eng="act"):
        for ci in cis:
            cs = slice(ci * TCH, (ci + 1) * TCH)
            pst = psum_pool.tile(shp or [P, TCH], F32, name=nm)
            ps = pst[:, 0, :] if shp else pst[:]
            for g in range(KE // 2):
                nc.tensor.matmul(
                    ps, wk8[:, 2 * g : 2 * g + 2, j * P : (j + 1) * P],
                    xn8[:, 2 * g : 2 * g + 2, cs],
                    start=(g == 0), stop=(g == KE // 2 - 1), perf_mode=DR,
                )
            if eng == "act":
                nc.scalar.activation(
                    kt[j][:, cs], ps, AF.Identity,
                    bias=gbt["bk"][:, j : j + 1], scale=DS,
                )
            else:
                nc.vector.tensor_scalar(
                    kt[j][:, cs], ps, DS, gbt["bk"][:, j : j + 1],
                    OP.mult, OP.add,
                )

    def v_proj_chunk(ci):
        for tt in range(4 * ci, 4 * ci + 4):
            for hf in range(2):
                ps = ps_qkv.tile([P, KE, D], F32, name="qkv")
                for g in range(KE // 2):
                    nc.tensor.matmul(
                        ps[:], xn8[:, 2 * g : 2 * g + 2, tt * P : (tt + 1) * P],
                        wv8[:, 2 * g : 2 * g + 2, hf * 512 : hf * 512 + 512],
                        start=(g == 0), stop=(g == KE // 2 - 1), perf_mode=DR,
                    )
                dst = vsb[tt][:, hf * 8 * (D + 1) :].rearrange(
                    "p (h d) -> p h d", d=D + 1
                )[:, 0:8, 0:D]
                if tt % 2 == 0:
                    nc.scalar.activation(dst, ps[:], AF.Copy, scale=DS)
                else:
                    nc.vector.tensor_scalar_mul(dst, ps[:], DS)

    def q_proj(j):
        ps = ps_qkv.tile([P, Tq], F32, name="qkv")
        for g in range(KE // 2):
            nc.tensor.matmul(
                ps[:], wq8[:, 2 * g : 2 * g + 2, j * P : (j + 1) * P],
                xn8[:, 2 * g : 2 * g + 2, T - Tq :],
                start=(g == 0), stop=(g == KE // 2 - 1), perf_mode=DR,
            )
        nc.vector.tensor_scalar(
            qt[j][:], ps[:], DS, gbt["bq"][:, j : j + 1], OP.mult, OP.add
        )

    def ln_stats(ci):
        """Stats matmuls + row math + broadcast + SBUF stage for chunk ci.
        Returns (mub_sb, rsb_sb)."""
        cs = slice(ci * TCH, (ci + 1) * TCH)
        s1 = ps_st.tile([1, TCH], F32, name="s1")
        s2 = ps_st.tile([1, TCH], F32, name="s2")
        for e in range(KE):
            x2 = p_tmp.tile([P, TCH], BF16, name="x2bf")
            nc.scalar.square(x2[:], xt[e][:, cs])
            nc.tensor.matmul(s1[:], ones_bf[:], xt[e][:, cs],
                             start=(e == 0), stop=(e == KE - 1))
            nc.tensor.matmul(s2[:], ones_bf[:], x2[:],
                             start=(e == 0), stop=(e == KE - 1))
        mu = p_rows.tile([1, TCH], F32, name="mu")
        nc.vector.tensor_scalar_mul(mu[:], s1[:], 1.0 / E)
        ve = p_rows.tile([1, TCH], F32, name="ve")
        nc.vector.tensor_scalar(ve[:], s2[:], 1.0 / E, c.eps, OP.mult, OP.add)
        mu2 = p_rows.tile([1, TCH], F32, name="mu2")
        nc.vector.tensor_tensor(mu2[:], mu[:], mu[:], OP.mult)
        vee = p_rows.tile([1, TCH], F32, name="vee")
        nc.vector.tensor_tensor(vee[:], ve[:], mu2[:], OP.subtract)
        # rstd*SX: sqrt(vee/SX^2) on ACT (stays in the sqrt table set),
        # then fast reciprocal on DVE -> SX/sqrt(vee)
        sq = p_rows.tile([1, TCH], F32, name="sq")
        nc.scalar.activation(sq[:], vee[:], AF.Sqrt, scale=1.0 / (SX * SX))
        rstd32 = p_rows.tile([1, TCH], F32, name="rstd32")
        nc.vector.reciprocal_approx_fast(rstd32[:], sq[:])

        mub = ps_bc.tile([P, TCH], F32, name="mub")
        nc.tensor.matmul(mub[:], ones_f1[:], mu[:], start=True, stop=True)
        rsb = ps_bc.tile([P, TCH], F32, name="rsb")
        nc.tensor.matmul(rsb[:], ones_f1[:], rstd32[:], start=True, stop=True)
        mub_sb = p_tmp.tile([P, TCH], BF16, name="mub_sb")
        nc.vector.tensor_copy(mub_sb[:], mub[:])
        rsb_sb = p_tmp.tile([P, TCH], BF16, name="rsb_sb")
        nc.vector.tensor_copy(rsb_sb[:], rsb[:])
        return mub_sb, rsb_sb

    # software-pipelined: stats(ci+1) emitted before projections(ci) so the
    # PE has stat matmuls to chew while DVE/Pool normalize chunk ci
    stg = ln_stats(0)
    for ci in range(NQC):
        cs = slice(ci * TCH, (ci + 1) * TCH)
        mub_sb, rsb_sb = stg
        for e in range(KE):
            t1 = p_tmp.tile([P, TCH], BF16, name="t1")
            sub_eng = nc.gpsimd if e % 8 < 5 else nc.vector
            sub_eng.tensor_tensor(t1[:], xt[e][:, cs], mub_sb[:], OP.subtract)
            nc.vector.tensor_tensor(xn8[:, e, cs], t1[:], rsb_sb[:], OP.mult)
        if ci + 1 < NQC:
            stg = ln_stats(ci + 1)

        # residual extraction for the query chunk (last chunk): xq = x + bo
        if ci == NQC - 1:
            for e in range(KE):
                nc.vector.tensor_scalar(
                    xq[e][:], xt[e][:, T - Tq :], gbt["boc"][:, e : e + 1],
                    None, OP.add,
                )

        # interleaved QKV for this chunk
        k_proj(0, ps_qkv, [ci])
        k_proj(1, ps_qkv, [ci])
        v_proj_chunk(ci)
        if ci == NQC - 1:
            for j in range(HP):
                q_proj(j)

    pp.close("qkv_ps", "ln_bc", "ln_st", "ln_rows", "ln_tmp", "xtp", "w8b")

    if DEBUG:
        nc.sync.dma_start(d["dbg_xn"], xn8[:])
        for j in range(HP):
            nc.sync.dma_start(d["dbg_qt"][:, j * Tq : (j + 1) * Tq], qt[j][:])
        for t in range(TK):
            nc.sync.dma_start(
                d["dbg_vs"][:, t * 1040 : (t + 1) * 1040], vsb[t][:]
            )

    # w2 resident load (xt freed now; lands during attention)
    p_w2 = pp.open("w2p", 1)
    w2a = p_w2.tile([P, KF, E], F8, name="w2a")
    w2b = p_w2.tile([P, KF, E], F8, name="w2b")
    mb2dr = p_w2.tile([P, 2, E], F8, name="mb2dr")
    ones8 = p_w2.tile([P, 2, Tq], F8, name="ones8")
    nc.vector.memset(ones8[:], 1.0)
    nc.sync.dma_start(mb2dr[:], d["mb2dr"].rearrange("p (k m) -> p k m", k=2))
    for nm, t_ in (("w2a", w2a), ("w2b", w2b)):
        for fq in range(4):
            nc.sync.dma_start(
                t_[:, 8 * fq : 8 * fq + 8, :],
                d[nm].rearrange("(f p) m -> p f m", p=P)[:, 8 * fq : 8 * fq + 8, :],
            )

    # ======================================================================
    # Phase 2: attention, head-pair at a time, query-major attnV
    # ======================================================================
    ss_p = pp.open("ss_ps", 2, "PSUM")
    oh_p = pp.open("oh_ps", 1, "PSUM")
    kv2_p = pp.open("kv2_ps", 1, "PSUM")
    tp_p = pp.open("tp_ps", 1, "PSUM")
    p_pr = pp.open("prp", 2, side="right")
    p_tail = pp.open("tailp", 2, side="right")

    for j in range(HP):
        ohq = oh_p.tile([P, 2, 512], F32, name="ohq")  # per s: 4*65 used
        for t in range(TK):
            u = t % 4
            i = t // 4
            # filler: K projection for pair j+2, one chunk per phase block,
            # slotted into the t-loop so PE bubbles absorb the psum ping-pong
            if u == 2 and j + 2 < HP:
                k_proj(j + 2, kv2_p, [i], eng="dve")
            ss = ss_p.tile([P, 2, Tq], F32, name="ss")
            for s in (0, 1):
                nc.tensor.matmul(
                    ss[:, s, u * P : Tq],
                    kt[j][s * 64 : (s + 1) * 64, t * P : (t + 1) * P],
                    qt[j][s * 64 : (s + 1) * 64, u * P : Tq],
                    start=True, stop=True, tile_position=(s * 64, 0),
                )
            pr = p_pr.tile([P, 2, Tq], BF16, name="pr")
            nc.scalar.activation(pr[:, :, u * P : Tq], ss[:, :, u * P : Tq],
                                 AF.Exp, scale=SCL)
            nc.gpsimd.tensor_tensor(
                pr[:, :, u * P : (u + 1) * P], pr[:, :, u * P : (u + 1) * P],
                band[:, i, :, :], OP.mult,
            )
            for m in range(u, NM):
                for s in (0, 1):
                    h = 2 * j + s
                    # start=True resets the whole PSUM bank: only the first
                    # matmul into each bank (m==0 at t==0) may set it; later
                    # groups accumulate onto the bank-wide zero.
                    nc.tensor.matmul(
                        ohq[:, s, m * 65 : m * 65 + 65],
                        pr[:, s, m * P : (m + 1) * P],
                        vsb[t][:, h * (D + 1) : (h + 1) * (D + 1)],
                        start=(t == 0 and m == 0), stop=(t == 12 + m),
                        skip_group_check=True,
                    )
        # tail: row-sums -> 1/rs -> normalize -> transpose -> fp8 quantize
        rs_sb = p_tail.tile([P, 2, NM], F32, name="rs_sb")
        for s in (0, 1):
            for m in range(NM):
                nc.vector.tensor_copy(rs_sb[:, s, m : m + 1],
                                      ohq[:, s, m * 65 + 64 : m * 65 + 65])
        irs = p_tail.tile([P, 2, NM], F32, name="irs")
        nc.vector.reciprocal(irs[:], rs_sb[:])
        ptb = p_tail.tile([P, 2, 256], BF16, name="ptb")
        for s in (0, 1):
            for m in range(NM):
                nc.vector.tensor_scalar_mul(
                    ptb[:, s, m * 64 : (m + 1) * 64],
                    ohq[:, s, m * 65 : m * 65 + 64],
                    irs[:, s, m : m + 1],
                )
        for m in range(NM):
            tp = tp_p.tile([P, P], BF16, name="tp")
            nc.tensor.transpose(tp[0:64, :], ptb[:, 0, m * 64 : (m + 1) * 64],
                                ident[:], tile_position=(0, 0))
            nc.tensor.transpose(tp[64:128, :], ptb[:, 1, m * 64 : (m + 1) * 64],
                                ident[:], tile_position=(0, 64))
            nc.vector.tensor_scalar(
                aop8[:, j, m * P : (m + 1) * P], tp[:], SX,
                gbt["bv32"][:, j : j + 1], OP.mult, OP.add,
            )
        if DEBUG:
            nc.sync.dma_start(d["dbg_rs"][:, j, :, :], rs_sb[:])
            nc.sync.dma_start(d["dbg_irs"][:, j, :, :], irs[:])
            nc.sync.dma_start(d["dbg_pt"][:, j, :, :], ptb[:])

    if DEBUG:
        nc.sync.dma_start(d["dbg_ao"], aop8[:])
        for j in range(HP):
            nc.sync.dma_start(d["dbg_kt"][:, j * T : (j + 1) * T], kt[j][:])
    pp.close("tailp", "prp", "tp_ps", "kv2_ps", "oh_ps", "ss_ps")
    pp.close("vsp", "qtp", "ktp", "xnp")

    # ======================================================================
    # Phase 3: out-projection (fp8) + residual; LayerNorm2
    # ======================================================================
    p_xr = pp.open("xrp", 1)
    p_x2 = pp.open("xn2p", 1)
    ps_ao = pp.open("ao_ps", 2, "PSUM")
    xres = [p_xr.tile([P, Tq], F32, name=f"xres{e}") for e in range(KE)]
    x2h = p_x2.tile([P, KE, Tq], F8, name="x2h")
    x2l = p_x2.tile([P, KE, Tq], F8, name="x2l")

    for e in range(KE):
        ps = ps_ao.tile([P, Tq], F32, name="aops")
        for g in range(KE // 2):
            nc.tensor.matmul(
                ps[:], wo8[:, 2 * g : 2 * g + 2, e * P : (e + 1) * P],
                aop8[:, 2 * g : 2 * g + 2, :],
                start=(g == 0), stop=(g == KE // 2 - 1), perf_mode=DR,
            )
        nc.vector.scalar_tensor_tensor(
            xres[e][:], ps[:], DS, xq[e][:], OP.mult, OP.add
        )
    pp.close("ao_ps", "aop")

    p_tmp = pp.open("ln2_tmp", 3)
    p_rows = pp.open("ln2_rows", 1)
    ps_st = pp.open("ln2_st", 1, "PSUM")
    ps_bc = pp.open("ln2_bc", 1, "PSUM")
    s1 = ps_st.tile([1, Tq], F32, name="s1b")
    s2 = ps_st.tile([1, Tq], F32, name="s2b")
    for e in range(KE):
        xbf = p_tmp.tile([P, Tq], BF16, name="xbf2")
        nc.scalar.activation(xbf[:], xres[e][:], AF.Copy)
        x2 = p_tmp.tile([P, Tq], BF16, name="x2bf2")
        nc.gpsimd.tensor_tensor(x2[:], xbf[:], xbf[:], OP.mult)
        nc.tensor.matmul(s1[:], ones_bf[:], xbf[:], start=(e == 0), stop=(e == KE - 1))
        nc.tensor.matmul(s2[:], ones_bf[:], x2[:], start=(e == 0), stop=(e == KE - 1))
    mu = p_rows.tile([1, Tq], F32, name="mu_2")
    nc.vector.tensor_scalar_mul(mu[:], s1[:], 1.0 / E)
    ve = p_rows.tile([1, Tq], F32, name="ve_2")
    nc.vector.tensor_scalar(ve[:], s2[:], 1.0 / E, c.eps, OP.mult, OP.add)
    mu2 = p_rows.tile([1, Tq], F32, name="mu2_2")
    nc.vector.tensor_tensor(mu2[:], mu[:], mu[:], OP.mult)
    vee = p_rows.tile([1, Tq], F32, name="vee_2")
    nc.vector.tensor_tensor(vee[:], ve[:], mu2[:], OP.subtract)
    sq2 = p_rows.tile([1, Tq], F32, name="sq_2")
    nc.scalar.activation(sq2[:], vee[:], AF.Sqrt, scale=1.0 / (SX * SX))
    rstd = p_rows.tile([1, Tq], F32, name="rstd_2")
    nc.vector.reciprocal_approx_fast(rstd[:], sq2[:])
    mub = ps_bc.tile([P, Tq], F32, name="mub2")
    nc.tensor.matmul(mub[:], ones_f1[:], mu[:], start=True, stop=True)
    rsb = ps_bc.tile([P, Tq], F32, name="rsb2")
    nc.tensor.matmul(rsb[:], ones_f1[:], rstd[:], start=True, stop=True)
    mub_sb = p_tmp.tile([P, Tq], BF16, name="mub2_sb")
    nc.vector.tensor_copy(mub_sb[:], mub[:])
    rsb_sb = p_tmp.tile([P, Tq], BF16, name="rsb2_sb")
    nc.vector.tensor_copy(rsb_sb[:], rsb[:])
    # xn2 (scaled by SX) split into fp8 head + fp8 residual for DoubleRow MLP1
    for e in range(KE):
        t1 = p_tmp.tile([P, Tq], BF16, name="t1b")
        nc.gpsimd.tensor_tensor(t1[:], xres[e][:], mub_sb[:], OP.subtract)
        xn2bf = p_tmp.tile([P, Tq], BF16, name="xn2bf")
        nc.vector.tensor_tensor(xn2bf[:], t1[:], rsb_sb[:], OP.mult)
        nc.scalar.activation(x2h[:, e, :], xn2bf[:], AF.Copy)
        nc.vector.tensor_tensor(x2l[:, e, :], xn2bf[:], x2h[:, e, :], OP.subtract)
    pp.close("ln2_rows", "ln2_tmp", "ln2_bc", "ln2_st")

    # ======================================================================
    # Phase 4: MLP in split-fp8 DoubleRow: X~H+L, W~A+B (fp8 residuals);
    # X@W ~ H@A + H@B + L@A at 0.75x the bf16 PE cost, ~bf16 accuracy.
    # ======================================================================
    EH = 6
    p_h1 = pp.open("h1p", 1, side="right")
    p_hbf = pp.open("h1bfp", 3, side="right")
    p_out = pp.open("outp", 2)
    p_w1 = pp.open("w1s", 3)
    ps_h1 = pp.open("h1_ps", 2, "PSUM")
    ps_h2a = pp.open("h2a_ps", 1, "PSUM")

    h1h = p_h1.tile([P, KF, Tq], F8, name="h1h")
    h1l = p_h1.tile([P, KF, Tq], F8, name="h1l")
    h2a = [ps_h2a.tile([P, Tq], F32, name=f"h2a{e}") for e in range(EH)]

    def mlp2_acc(g, es, h2ps, first, last):
        for i, e in enumerate(es):
            for wt, ht in ((w2a, h1h), (w2b, h1h), (w2a, h1l)):
                fst = first and wt is w2a and ht is h1h
                lst = last and wt is w2a and ht is h1l
                nc.tensor.matmul(
                    h2ps[i][:], wt[:, 2 * g : 2 * g + 2, e * P : (e + 1) * P],
                    ht[:, 2 * g : 2 * g + 2, :],
                    start=fst, stop=False, perf_mode=DR, skip_group_check=True,
                )
            if last:
                # += b2*8192 via ones rhs, then stop the group
                nc.tensor.matmul(
                    h2ps[i][:], mb2dr[:, :, e * P : (e + 1) * P], ones8[:],
                    start=False, stop=True, perf_mode=DR, skip_group_check=True,
                )

    for f in range(KF):
        w1af = p_w1.tile([P, KE, P], F8, name="w1a")
        w1bf = p_w1.tile([P, KE, P], F8, name="w1b")
        nc.sync.dma_start(
            w1af[:],
            d["w1a"].rearrange("(e p) m -> p e m", p=P)[:, :, f * P : (f + 1) * P],
        )
        nc.sync.dma_start(
            w1bf[:],
            d["w1b"].rearrange("(e p) m -> p e m", p=P)[:, :, f * P : (f + 1) * P],
        )
        ps = ps_h1.tile([P, Tq], F32, name="h1ps")
        for g in range(KE // 2):
            for wt, xt_ in ((w1af, x2h), (w1bf, x2h), (w1af, x2l)):
                nc.tensor.matmul(
                    ps[:], wt[:, 2 * g : 2 * g + 2, :],
                    xt_[:, 2 * g : 2 * g + 2, :],
                    start=(g == 0 and wt is w1af and xt_ is x2h),
                    stop=(g == KE // 2 - 1 and wt is w1af and xt_ is x2l),
                    perf_mode=DR,
                )
        h1bf = p_hbf.tile([P, Tq], BF16, name="h1bf")
        nc.scalar.activation(
            h1bf[:], ps[:], AF.Relu, bias=gbt["mb1"][:, f : f + 1], scale=DS
        )
        nc.vector.tensor_scalar_mul(h1h[:, f, :], h1bf[:], SX)
        nc.vector.scalar_tensor_tensor(
            h1l[:, f, :], h1bf[:], SX, h1h[:, f, :], OP.mult, OP.subtract
        )
        if f % 2 == 1:
            mlp2_acc(f // 2, range(EH), h2a, first=(f == 1), last=(f == KF - 1))
    for e in range(EH):
        of = p_out.tile([P, Tq], F32, name="outf")
        nc.vector.scalar_tensor_tensor(
            of[:], h2a[e][:], DS, xres[e][:], OP.mult, OP.add
        )
        nc.sync.dma_start(d["out_t"][e * P : (e + 1) * P, :], of[:])
    pp.close("h2a_ps", "h1_ps")

    ps_h2b = pp.open("h2b_ps", 1, "PSUM")
    h2b = [ps_h2b.tile([P, Tq], F32, name=f"h2b{i}") for i in range(KE - EH)]
    for g in range(KF // 2):
        mlp2_acc(g, range(EH, KE), h2b, first=(g == 0), last=(g == KF // 2 - 1))
    for i, e in enumerate(range(EH, KE)):
        of = p_out.tile([P, Tq], F32, name="outf")
        nc.vector.scalar_tensor_tensor(
            of[:], h2b[i][:], DS, xres[e][:], OP.mult, OP.add
        )
        nc.sync.dma_start(d["out_t"][e * P : (e + 1) * P, :], of[:])

    pp.close_all()


def build_program(c: Cfg = CFG):
    c.check()
    nc = bacc.Bacc(
        "TRN2",
        target_bir_lowering=False,
        debug=False,
        enable_asserts=False,
        num_devices=c.NC,
    )
    d = {}
    d["xt"] = nc.dram_tensor("xt", [c.E, c.T], BF16, kind="ExternalInput").ap()
    for nm in ("wq8", "wk8", "wv8", "wo8"):
        d[nm] = nc.dram_tensor(nm, [c.E, c.E], F8, kind="ExternalInput").ap()
    d["w1a"] = nc.dram_tensor("w1a", [c.E, c.F], F8, kind="ExternalInput").ap()
    d["w1b"] = nc.dram_tensor("w1b", [c.E, c.F], F8, kind="ExternalInput").ap()
    d["w2a"] = nc.dram_tensor("w2a", [c.F, c.E], F8, kind="ExternalInput").ap()
    d["w2b"] = nc.dram_tensor("w2b", [c.F, c.E], F8, kind="ExternalInput").ap()
    d["mb2dr"] = nc.dram_tensor("mb2dr", [P, 2 * c.E], F8, kind="ExternalInput").ap()
    for nm, cols in [("bq", c.KE), ("bk", c.KE), ("bv32", c.KE), ("boc", c.KE),
                     ("mb1", c.KF)]:
        d[nm] = nc.dram_tensor(nm, [P, cols], F32, kind="ExternalInput").ap()
    d["band"] = nc.dram_tensor("band", [P, c.NQC * 2 * P], BF16,
                               kind="ExternalInput").ap()
    d["ident"] = nc.dram_tensor("ident", [P, P], BF16, kind="ExternalInput").ap()
    d["out_t"] = nc.dram_tensor("out_t", [c.E, c.Tq], F32, kind="ExternalOutput").ap()
    if DEBUG:
        d["dbg_xn"] = nc.dram_tensor("dbg_xn", [P, c.KE, c.T], F8,
                                     kind="ExternalOutput").ap()
        d["dbg_qt"] = nc.dram_tensor("dbg_qt", [P, c.HP * c.Tq], BF16,
                                     kind="ExternalOutput").ap()
        d["dbg_kt"] = nc.dram_tensor("dbg_kt", [P, c.HP * c.T], BF16,
                                     kind="ExternalOutput").ap()
        d["dbg_vs"] = nc.dram_tensor("dbg_vs", [P, c.TK * 1040], BF16,
                                     kind="ExternalOutput").ap()
        d["dbg_ao"] = nc.dram_tensor("dbg_ao", [P, c.HP, c.Tq], F8,
                                     kind="ExternalOutput").ap()
        d["dbg_rs"] = nc.dram_tensor("dbg_rs", [P, c.HP, 2, 4], F32,
                                     kind="ExternalOutput").ap()
        d["dbg_irs"] = nc.dram_tensor("dbg_irs", [P, c.HP, 2, 4], F32,
                                      kind="ExternalOutput").ap()
        d["dbg_pt"] = nc.dram_tensor("dbg_pt", [P, c.HP, 2, 256], BF16,
                                     kind="ExternalOutput").ap()

    with tile.TileContext(nc) as tc:
        _emit(tc, c, d)
    nc.compile()
    return nc


# --------------------------------------------------------------------------
# host side
# --------------------------------------------------------------------------
def shard_inputs(inputs, c: Cfg = CFG):
    f32 = lambda a: np.ascontiguousarray(np.asarray(a, np.float32))
    x = f32(inputs["x"])
    g1, b1n = f32(inputs["ln1_g"]), f32(inputs["ln1_b"])
    g2, b2n = f32(inputs["ln2_g"]), f32(inputs["ln2_b"])
    Wq, Wk, Wv, Wo = (f32(inputs[k]) for k in ("Wq", "Wk", "Wv", "Wo"))
    W1, W2 = f32(inputs["W1"]), f32(inputs["W2"])
    bo, b1, b2 = f32(inputs["bo"]), f32(inputs["b1"]), f32(inputs["b2"])

    q8 = lambda w: np.ascontiguousarray((w * SW)).astype(NPF8)
    bf = lambda w: np.ascontiguousarray(w).astype(NPBF16)
    chunks = lambda v, k: np.ascontiguousarray(v.reshape(k, P).T)

    def split8(w):
        ws = np.ascontiguousarray(w * SW)
        a = ws.astype(NPF8)
        b = (ws - a.astype(np.float32)).astype(NPF8)
        return a, b

    w1a, w1b = split8(g2[:, None] * W1)
    w2a, w2b = split8(W2)
    com = {
        "wq8": q8(g1[:, None] * Wq),
        "wk8": q8(g1[:, None] * Wk),
        "wv8": q8(g1[:, None] * Wv),
        "wo8": q8(Wo),
        "w1a": w1a, "w1b": w1b,
        "w2a": w2a, "w2b": w2b,
        "mb2dr": np.broadcast_to(
            (b2 * SX).astype(NPF8), (P, 2, c.E)
        ).reshape(P, 2 * c.E).copy(),
        "bq": chunks(b1n @ Wq, c.KE),
        "bk": chunks(b1n @ Wk, c.KE),
        "bv32": chunks((b1n @ Wv) * SX, c.KE),
        "boc": chunks(bo, c.KE),
        "mb1": chunks(b1 + b2n @ W1, c.KF),
        "ident": np.eye(P, dtype=np.float32).astype(NPBF16),
    }

    p_idx = np.arange(P)[:, None]
    lq_idx = np.arange(P)[None, :]
    maps = []
    for core in range(c.NC):
        b, j = core // c.CPB, core % c.CPB
        phases = [(j + 1) % 4, (j + 2) % 4, (j + 3) % 4, j]
        ctx = np.empty((c.E, c.T), np.float32)
        for i, ph in enumerate(phases):
            ctx[:, 512 * i : 512 * (i + 1)] = x[b, ph::4, :].T
        band = np.zeros((P, c.NQC, 2, P), np.float32)
        for i, ph in enumerate(phases):
            delta = 1 if ph > j else 0
            m_ = (p_idx <= lq_idx - delta).astype(np.float32)
            band[:, i, 0, :] = m_
            band[:, i, 1, :] = m_
        m = dict(com)
        m["xt"] = bf(ctx)
        m["band"] = band.reshape(P, c.NQC * 2 * P).astype(NPBF16)
        maps.append(m)
    return maps


def assemble(results, c: Cfg = CFG):
    out = np.empty((c.B, c.T, c.E), np.float32)
    for core in range(c.NC):
        b, j = core // c.CPB, core % c.CPB
        out[b, j::4, :] = results[core]["out_t"].T
    return out


_NC_CACHE = {}


def _get_nc(c: Cfg = CFG):
    if c not in _NC_CACHE:
        _NC_CACHE[c] = build_program(c)
    return _NC_CACHE[c]


LAST_RESULT = None


def kernel(**inputs):
    global LAST_RESULT
    c = CFG
    nc = _get_nc(c)
    maps = shard_inputs(inputs, c)
    res = bass_utils.run_bass_kernel_spmd(nc, maps, core_ids=list(range(c.NC)))
    LAST_RESULT = res
    return assemble(res.results, c)


# revision 70
# speedup vs baseline: 2.0033x; 1.0887x over previous
"""Trainium2 Bass kernel for a pre-norm transformer block (dense_transformer).

Computation (per reference):
    x = x + Attn(LN1(x));  x = x + MLP(LN2(x))
with causal multi-head attention (H=16 heads, D=64) and a 4E ReLU MLP.

Sharding: 8 cores = 2 batches x 4 query PHASES.  Core (b, j) owns the 512
query tokens {4r + j}.  The context (all 2048 tokens) is column-PERMUTED
per core so the core's own phase sits last: position 512*i + r holds token
4r + phase_i with phase order (j+1, j+2, j+3, j) mod 4.  With this striping
the causal block structure is identical on every core (SPMD uniform): query
tile m attends context tiles t with t%4 <= m, so upper score tiles are
skipped for real FLOP savings; the diagonal band mask is a per-core input.

Precision: QKV and output projections run in fp8e4 DoubleRow (2x PE perf,
K=256 per instruction) with power-of-2 scales folded into the weights and
descale factors folded into existing copies; LN gains/biases are folded
into the weights host-side.  Scores/attnV/MLP stay bf16 (error budget).

attnV runs "query-major": out[q, d] with a ones column appended to V, so
softmax row-sums accumulate for free in PSUM column 64; normalization is a
per-partition scalar multiply, then a PE transpose returns to feature-major
for the fp8 out-projection.
"""

from dataclasses import dataclass

import numpy as np
import ml_dtypes

import concourse.bass as bass  # noqa: F401
import concourse.mybir as mybir
import concourse.tile as tile
from concourse import bacc
from concourse import bass_utils

F32 = mybir.dt.float32
BF16 = mybir.dt.bfloat16
F8 = mybir.dt.float8e4
AF = mybir.ActivationFunctionType
OP = mybir.AluOpType
DR = mybir.MatmulPerfMode.DoubleRow
NPBF16 = ml_dtypes.bfloat16
NPF8 = ml_dtypes.float8_e4m3

P = 128
SX = 32.0       # fp8 scale on activations
SW = 256.0      # fp8 scale on weights
DS = 1.0 / (SX * SW)


@dataclass(frozen=True)
class Cfg:
    B: int = 2
    T: int = 2048
    E: int = 1024
    H: int = 16
    D: int = 64
    NC: int = 8
    eps: float = 1e-5

    @property
    def CPB(self):
        return self.NC // self.B

    @property
    def Tq(self):
        return self.T // self.CPB

    @property
    def KE(self):
        return self.E // P

    @property
    def TK(self):
        return self.T // P

    @property
    def HP(self):
        return self.H // 2

    @property
    def F(self):
        return 4 * self.E

    @property
    def KF(self):
        return self.F // P

    @property
    def TCH(self):
        return 512

    @property
    def NQC(self):
        return self.T // self.TCH

    @property
    def NM(self):
        return self.Tq // P  # query tiles per core

    def check(self):
        assert self.D == 64 and self.E == self.H * self.D
        assert self.Tq == 512 and self.KE == 8 and self.TK == 16
        assert self.CPB == 4 and self.HP == 8 and self.KF == 32


CFG = Cfg()
DEBUG = False


class Pools:
    def __init__(self, tc, prefix=""):
        self.tc = tc
        self.prefix = prefix
        self.live = {}

    def open(self, key, bufs, space=None, side=None):
        kw = dict(name=self.prefix + key, bufs=bufs)
        if space:
            kw["space"] = space
        if side:
            kw["side"] = side
        cm = self.tc.tile_pool(**kw)
        pool = cm.__enter__()
        self.live[key] = cm
        return pool

    def close(self, *keys):
        for key in keys:
            self.live.pop(key).__exit__(None, None, None)

    def close_all(self):
        for key in reversed(list(self.live)):
            self.close(key)


def _emit(tc, c: Cfg, d):
    nc = tc.nc
    E, T, Tq, H, D = c.E, c.T, c.Tq, c.H, c.D
    KE, TK, HP, KF, NM = c.KE, c.TK, c.HP, c.KF, c.NM
    TCH, NQC = c.TCH, c.NQC
    SCL = 1.0 / float(np.sqrt(D))

    pp = Pools(tc)

    # ---------------- constants ----------------
    const = pp.open("const", 1)
    ones_bf = const.tile([P, 1], BF16, name="ones_bf")
    nc.vector.memset(ones_bf[:], 1.0)
    ones_f1 = const.tile([1, P], F32, name="ones_f1")
    nc.vector.memset(ones_f1[:], 1.0)
    ident = const.tile([P, P], BF16, name="ident")
    band = const.tile([P, NQC, 2, P], BF16, name="band")
    gbt = {}
    for nm, cols in [("bq", KE), ("bk", KE), ("bv32", KE), ("boc", KE),
                     ("mb1", KF)]:
        gbt[nm] = const.tile([P, cols], F32, name=nm + "_t")
        nc.sync.dma_start(gbt[nm][:], d[nm])

    # ---------------- weights (fp8, resident) ----------------
    p_w8 = pp.open("w8", 1)
    wk8 = p_w8.tile([P, KE, E], F8, name="wk8")
    wo8 = p_w8.tile([P, KE, E], F8, name="wo8")

    # ---------------- warmup (PE p-state ramp; no DMA dependency) ----------
    p_wsb = pp.open("warm_sb", 1, side="right")
    ps_wm = pp.open("warm_ps", 1, "PSUM")
    wsb = p_wsb.tile([P, TCH], BF16, name="wsb")
    nc.vector.memset(wsb[:], 0.0)
    wmp = ps_wm.tile([1, TCH], F32, name="wmp")
    for _w in range(8):
        nc.tensor.matmul(wmp[:], ones_bf[:], wsb[:], start=True, stop=True)
    pp.close("warm_ps", "warm_sb")

    # ---------------- long-lived activations ----------------
    # left stack: const, w8, xqp | w8b, xtp, LN pools (freed) | w2p, phase3/4
    # right stack: aop | xnp, ktp, qtp, vsp (freed after attention), attn pools
    p_xq = pp.open("xqp", 1)
    xq = [p_xq.tile([P, Tq], F32, name=f"xq{e}") for e in range(KE)]
    p_ao = pp.open("aop", 1, side="right")
    aop8 = p_ao.tile([P, HP, Tq], F8, name="aop8")
    p_xn = pp.open("xnp", 1, side="right")
    xn8 = p_xn.tile([P, KE, T], F8, name="xn8")
    p_kt = pp.open("ktp", 1, side="right")
    kt = [p_kt.tile([P, T], BF16, name=f"kt{j}") for j in range(HP)]
    p_qt = pp.open("qtp", 1, side="right")
    qt = [p_qt.tile([P, Tq], BF16, name=f"qt{j}") for j in range(HP)]
    p_vs = pp.open("vsp", 1, side="right")
    vsb = [p_vs.tile([P, H * (D + 1)], BF16, name=f"vsb{t}") for t in range(TK)]

    # ======================================================================
    # Phase 1: x load + LN1 + QKV (chunk-interleaved)
    # ======================================================================
    p_w8b = pp.open("w8b", 1)
    wq8 = p_w8b.tile([P, KE, E], F8, name="wq8")
    wv8 = p_w8b.tile([P, KE, E], F8, name="wv8")

    p_xt = pp.open("xtp", 1)
    xt = [p_xt.tile([P, T], BF16, name=f"xt{e}") for e in range(KE)]
    # DMA priority order: xt (stats start on it), then K/V weights (used from
    # chunk 0), then Q (chunk 3) and O/ident/band (attention phase).
    for e in range(KE):
        nc.sync.dma_start(xt[e][:], d["xt"][e * P : (e + 1) * P, :])
    for nm, t_ in [("wv8", wv8), ("wk8", wk8), ("wq8", wq8), ("wo8", wo8)]:
        nc.sync.dma_start(t_[:], d[nm].rearrange("(e p) m -> p e m", p=P))
    nc.sync.dma_start(ident[:], d["ident"])
    nc.sync.dma_start(band[:], d["band"].rearrange("p (i s q) -> p i s q",
                                                   i=NQC, s=2))

    # ones columns of V (col 64 of each head slot)
    for t in range(TK):
        nc.vector.memset(vsb[t][:, :].rearrange("p (h d) -> p h d", d=D + 1)[:, :, D], 1.0)

    p_tmp = pp.open("ln_tmp", 3)
    p_rows = pp.open("ln_rows", 1)
    ps_st = pp.open("ln_st", 2, "PSUM")
    ps_bc = pp.open("ln_bc", 1, "PSUM")
    ps_qkv = pp.open("qkv_ps", 2, "PSUM")

    def k_proj(j, psum_pool, cis, nm="qkv", shp=None, eng="act"):
        for ci in cis:
            cs = slice(ci * TCH, (ci + 1) * TCH)
            pst = psum_pool.tile(shp or [P, TCH], F32, name=nm)
            ps = pst[:, 0, :] if shp else pst[:]
            for g in range(KE // 2):
                nc.tensor.matmul(
                    ps, wk8[:, 2 * g : 2 * g + 2, j * P : (j + 1) * P],
                    xn8[:, 2 * g : 2 * g + 2, cs],
                    start=(g == 0), stop=(g == KE // 2 - 1), perf_mode=DR,
                )
            if eng == "act":
                nc.scalar.activation(
                    kt[j][:, cs], ps, AF.Identity,
                    bias=gbt["bk"][:, j : j + 1], scale=DS,
                )
            else:
                nc.vector.tensor_scalar(
                    kt[j][:, cs], ps, DS, gbt["bk"][:, j : j + 1],
                    OP.mult, OP.add,
                )

    def v_proj_chunk(ci):
        for tt in range(4 * ci, 4 * ci + 4):
            for hf in range(2):
                ps = ps_qkv.tile([P, KE, D], F32, name="qkv")
                for g in range(KE // 2):
                    nc.tensor.matmul(
                        ps[:], xn8[:, 2 * g : 2 * g + 2, tt * P : (tt + 1) * P],
                        wv8[:, 2 * g : 2 * g + 2, hf * 512 : hf * 512 + 512],
                        start=(g == 0), stop=(g == KE // 2 - 1), perf_mode=DR,
                    )
                dst = vsb[tt][:, hf * 8 * (D + 1) :].rearrange(
                    "p (h d) -> p h d", d=D + 1
                )[:, 0:8, 0:D]
                if tt % 2 == 0:
                    nc.scalar.activation(dst, ps[:], AF.Copy, scale=DS)
                else:
                    nc.vector.tensor_scalar_mul(dst, ps[:], DS)

    def q_proj(j):
        ps = ps_qkv.tile([P, Tq], F32, name="qkv")
        for g in range(KE // 2):
            nc.tensor.matmul(
                ps[:], wq8[:, 2 * g : 2 * g + 2, j * P : (j + 1) * P],
                xn8[:, 2 * g : 2 * g + 2, T - Tq :],
                start=(g == 0), stop=(g == KE // 2 - 1), perf_mode=DR,
            )
        nc.vector.tensor_scalar(
            qt[j][:], ps[:], DS, gbt["bq"][:, j : j + 1], OP.mult, OP.add
        )

    def ln_stats(ci):
        """Stats matmuls + row math + broadcast + SBUF stage for chunk ci.
        Returns (mub_sb, rsb_sb)."""
        cs = slice(ci * TCH, (ci + 1) * TCH)
        s1 = ps_st.tile([1, TCH], F32, name="s1")
        s2 = ps_st.tile([1, TCH], F32, name="s2")
        for e in range(KE):
            x2 = p_tmp.tile([P, TCH], BF16, name="x2bf")
            nc.scalar.square(x2[:], xt[e][:, cs])
            nc.tensor.matmul(s1[:], ones_bf[:], xt[e][:, cs],
                             start=(e == 0), stop=(e == KE - 1))
            nc.tensor.matmul(s2[:], ones_bf[:], x2[:],
                             start=(e == 0), stop=(e == KE - 1))
        mu = p_rows.tile([1, TCH], F32, name="mu")
        nc.vector.tensor_scalar_mul(mu[:], s1[:], 1.0 / E)
        ve = p_rows.tile([1, TCH], F32, name="ve")
        nc.vector.tensor_scalar(ve[:], s2[:], 1.0 / E, c.eps, OP.mult, OP.add)
        mu2 = p_rows.tile([1, TCH], F32, name="mu2")
        nc.vector.tensor_tensor(mu2[:], mu[:], mu[:], OP.mult)
        vee = p_rows.tile([1, TCH], F32, name="vee")
        nc.vector.tensor_tensor(vee[:], ve[:], mu2[:], OP.subtract)
        # rstd*SX: sqrt(vee/SX^2) on ACT (stays in the sqrt table set),
        # then fast reciprocal on DVE -> SX/sqrt(vee)
        sq = p_rows.tile([1, TCH], F32, name="sq")
        nc.scalar.activation(sq[:], vee[:], AF.Sqrt, scale=1.0 / (SX * SX))
        rstd32 = p_rows.tile([1, TCH], F32, name="rstd32")
        nc.vector.reciprocal_approx_fast(rstd32[:], sq[:])

        mub = ps_bc.tile([P, TCH], F32, name="mub")
        nc.tensor.matmul(mub[:], ones_f1[:], mu[:], start=True, stop=True)
        rsb = ps_bc.tile([P, TCH], F32, name="rsb")
        nc.tensor.matmul(rsb[:], ones_f1[:], rstd32[:], start=True, stop=True)
        mub_sb = p_tmp.tile([P, TCH], BF16, name="mub_sb")
        nc.vector.tensor_copy(mub_sb[:], mub[:])
        rsb_sb = p_tmp.tile([P, TCH], BF16, name="rsb_sb")
        nc.vector.tensor_copy(rsb_sb[:], rsb[:])
        return mub_sb, rsb_sb

    # software-pipelined: stats(ci+1) emitted before projections(ci) so the
    # PE has stat matmuls to chew while DVE/Pool normalize chunk ci
    stg = ln_stats(0)
    for ci in range(NQC):
        cs = slice(ci * TCH, (ci + 1) * TCH)
        mub_sb, rsb_sb = stg
        for e in range(KE):
            t1 = p_tmp.tile([P, TCH], BF16, name="t1")
            sub_eng = nc.gpsimd if e % 8 < 5 else nc.vector
            sub_eng.tensor_tensor(t1[:], xt[e][:, cs], mub_sb[:], OP.subtract)
            nc.vector.tensor_tensor(xn8[:, e, cs], t1[:], rsb_sb[:], OP.mult)
        if ci + 1 < NQC:
            stg = ln_stats(ci + 1)

        # residual extraction for the query chunk (last chunk): xq = x + bo
        if ci == NQC - 1:
            for e in range(KE):
                nc.vector.tensor_scalar(
                    xq[e][:], xt[e][:, T - Tq :], gbt["boc"][:, e : e + 1],
                    None, OP.add,
                )

        # interleaved QKV for this chunk
        k_proj(0, ps_qkv, [ci])
        k_proj(1, ps_qkv, [ci])
        v_proj_chunk(ci)
        if ci == NQC - 1:
            for j in range(HP):
                q_proj(j)

    pp.close("qkv_ps", "ln_bc", "ln_st", "ln_rows", "ln_tmp", "xtp", "w8b")

    if DEBUG:
        nc.sync.dma_start(d["dbg_xn"], xn8[:])
        for j in range(HP):
            nc.sync.dma_start(d["dbg_qt"][:, j * Tq : (j + 1) * Tq], qt[j][:])
        for t in range(TK):
            nc.sync.dma_start(
                d["dbg_vs"][:, t * 1040 : (t + 1) * 1040], vsb[t][:]
            )

    # w2 resident load (xt freed now; lands during attention)
    p_w2 = pp.open("w2p", 1)
    w2a = p_w2.tile([P, KF, E], F8, name="w2a")
    w2b = p_w2.tile([P, KF, E], F8, name="w2b")
    mb2dr = p_w2.tile([P, 2, E], F8, name="mb2dr")
    ones8 = p_w2.tile([P, 2, Tq], F8, name="ones8")
    nc.vector.memset(ones8[:], 1.0)
    nc.sync.dma_start(mb2dr[:], d["mb2dr"].rearrange("p (k m) -> p k m", k=2))
    for nm, t_ in (("w2a", w2a), ("w2b", w2b)):
        for fq in range(4):
            nc.sync.dma_start(
                t_[:, 8 * fq : 8 * fq + 8, :],
                d[nm].rearrange("(f p) m -> p f m", p=P)[:, 8 * fq : 8 * fq + 8, :],
            )

    # ======================================================================
    # Phase 2: attention, head-pair at a time, query-major attnV
    # ======================================================================
    ss_p = pp.open("ss_ps", 2, "PSUM")
    oh_p = pp.open("oh_ps", 1, "PSUM")
    kv2_p = pp.open("kv2_ps", 1, "PSUM")
    tp_p = pp.open("tp_ps", 1, "PSUM")
    p_pr = pp.open("prp", 3, side="right")
    p_tail = pp.open("tailp", 2, side="right")

    for j in range(HP):
        ohq = oh_p.tile([P, 2, 512], F32, name="ohq")  # per s: 4*65 used
        for t in range(TK):
            u = t % 4
            i = t // 4
            # filler: K projection for pair j+2, one chunk per phase block,
            # slotted into the t-loop so PE bubbles absorb the psum ping-pong
            if u == 2 and j + 2 < HP:
                k_proj(j + 2, kv2_p, [i], eng="dve")
            ss = ss_p.tile([P, 2, Tq], F32, name="ss")
            for s in (0, 1):
                nc.tensor.matmul(
                    ss[:, s, u * P : Tq],
                    kt[j][s * 64 : (s + 1) * 64, t * P : (t + 1) * P],
                    qt[j][s * 64 : (s + 1) * 64, u * P : Tq],
                    start=True, stop=True, tile_position=(s * 64, 0),
                )
            pr = p_pr.tile([P, 2, Tq], BF16, name="pr")
            nc.scalar.activation(pr[:, :, u * P : Tq], ss[:, :, u * P : Tq],
                                 AF.Exp, scale=SCL)
            nc.gpsimd.tensor_tensor(
                pr[:, :, u * P : (u + 1) * P], pr[:, :, u * P : (u + 1) * P],
                band[:, i, :, :], OP.mult,
            )
            # diagonal block (m==u) last: it depends on the band multiply,
            # the off-diagonal blocks only on the exp.  At t==0 the bank-
            # resetting m==0 matmul must stay first.
            m_order = range(NM) if t == 0 else list(range(u + 1, NM)) + [u]
            for m in m_order:
                for s in (0, 1):
                    h = 2 * j + s
                    # start=True resets the whole PSUM bank: only the first
                    # matmul into each bank (m==0 at t==0) may set it; later
                    # groups accumulate onto the bank-wide zero.
                    nc.tensor.matmul(
                        ohq[:, s, m * 65 : m * 65 + 65],
                        pr[:, s, m * P : (m + 1) * P],
                        vsb[t][:, h * (D + 1) : (h + 1) * (D + 1)],
                        start=(t == 0 and m == 0), stop=(t == 12 + m),
                        skip_group_check=True,
                    )
        # tail: row-sums -> 1/rs -> normalize -> transpose -> fp8 quantize
        rs_sb = p_tail.tile([P, 2, NM], F32, name="rs_sb")
        for s in (0, 1):
            for m in range(NM):
                nc.vector.tensor_copy(rs_sb[:, s, m : m + 1],
                                      ohq[:, s, m * 65 + 64 : m * 65 + 65])
        irs = p_tail.tile([P, 2, NM], F32, name="irs")
        nc.vector.reciprocal(irs[:], rs_sb[:])
        ptb = p_tail.tile([P, 2, 256], BF16, name="ptb")
        for s in (0, 1):
            for m in range(NM):
                nc.vector.tensor_scalar_mul(
                    ptb[:, s, m * 64 : (m + 1) * 64],
                    ohq[:, s, m * 65 : m * 65 + 64],
                    irs[:, s, m : m + 1],
                )
        for m in range(NM):
            tp = tp_p.tile([P, P], BF16, name="tp")
            nc.tensor.transpose(tp[0:64, :], ptb[:, 0, m * 64 : (m + 1) * 64],
                                ident[:], tile_position=(0, 0))
            nc.tensor.transpose(tp[64:128, :], ptb[:, 1, m * 64 : (m + 1) * 64],
                                ident[:], tile_position=(0, 64))
            nc.vector.tensor_scalar(
                aop8[:, j, m * P : (m + 1) * P], tp[:], SX,
                gbt["bv32"][:, j : j + 1], OP.mult, OP.add,
            )
        if DEBUG:
            nc.sync.dma_start(d["dbg_rs"][:, j, :, :], rs_sb[:])
            nc.sync.dma_start(d["dbg_irs"][:, j, :, :], irs[:])
            nc.sync.dma_start(d["dbg_pt"][:, j, :, :], ptb[:])

    if DEBUG:
        nc.sync.dma_start(d["dbg_ao"], aop8[:])
        for j in range(HP):
            nc.sync.dma_start(d["dbg_kt"][:, j * T : (j + 1) * T], kt[j][:])
    pp.close("tailp", "prp", "tp_ps", "kv2_ps", "oh_ps", "ss_ps")
    pp.close("vsp", "qtp", "ktp", "xnp")

    # ======================================================================
    # Phase 3: out-projection (fp8) + residual; LayerNorm2
    # ======================================================================
    p_xr = pp.open("xrp", 1)
    p_x2 = pp.open("xn2p", 1)
    ps_ao = pp.open("ao_ps", 2, "PSUM")
    xres = [p_xr.tile([P, Tq], F32, name=f"xres{e}") for e in range(KE)]
    x2h = p_x2.tile([P, KE, Tq], F8, name="x2h")
    x2l = p_x2.tile([P, KE, Tq], F8, name="x2l")

    for e in range(KE):
        ps = ps_ao.tile([P, Tq], F32, name="aops")
        for g in range(KE // 2):
            nc.tensor.matmul(
                ps[:], wo8[:, 2 * g : 2 * g + 2, e * P : (e + 1) * P],
                aop8[:, 2 * g : 2 * g + 2, :],
                start=(g == 0), stop=(g == KE // 2 - 1), perf_mode=DR,
            )
        nc.vector.scalar_tensor_tensor(
            xres[e][:], ps[:], DS, xq[e][:], OP.mult, OP.add
        )
    pp.close("ao_ps", "aop")

    p_tmp = pp.open("ln2_tmp", 3)
    p_rows = pp.open("ln2_rows", 1)
    ps_st = pp.open("ln2_st", 1, "PSUM")
    ps_bc = pp.open("ln2_bc", 1, "PSUM")
    s1 = ps_st.tile([1, Tq], F32, name="s1b")
    s2 = ps_st.tile([1, Tq], F32, name="s2b")
    for e in range(KE):
        xbf = p_tmp.tile([P, Tq], BF16, name="xbf2")
        nc.scalar.activation(xbf[:], xres[e][:], AF.Copy)
        x2 = p_tmp.tile([P, Tq], BF16, name="x2bf2")
        nc.gpsimd.tensor_tensor(x2[:], xbf[:], xbf[:], OP.mult)
        nc.tensor.matmul(s1[:], ones_bf[:], xbf[:], start=(e == 0), stop=(e == KE - 1))
        nc.tensor.matmul(s2[:], ones_bf[:], x2[:], start=(e == 0), stop=(e == KE - 1))
    mu = p_rows.tile([1, Tq], F32, name="mu_2")
    nc.vector.tensor_scalar_mul(mu[:], s1[:], 1.0 / E)
    ve = p_rows.tile([1, Tq], F32, name="ve_2")
    nc.vector.tensor_scalar(ve[:], s2[:], 1.0 / E, c.eps, OP.mult, OP.add)
    mu2 = p_rows.tile([1, Tq], F32, name="mu2_2")
    nc.vector.tensor_tensor(mu2[:], mu[:], mu[:], OP.mult)
    vee = p_rows.tile([1, Tq], F32, name="vee_2")
    nc.vector.tensor_tensor(vee[:], ve[:], mu2[:], OP.subtract)
    sq2 = p_rows.tile([1, Tq], F32, name="sq_2")
    nc.scalar.activation(sq2[:], vee[:], AF.Sqrt, scale=1.0 / (SX * SX))
    rstd = p_rows.tile([1, Tq], F32, name="rstd_2")
    nc.vector.reciprocal_approx_fast(rstd[:], sq2[:])
    mub = ps_bc.tile([P, Tq], F32, name="mub2")
    nc.tensor.matmul(mub[:], ones_f1[:], mu[:], start=True, stop=True)
    rsb = ps_bc.tile([P, Tq], F32, name="rsb2")
    nc.tensor.matmul(rsb[:], ones_f1[:], rstd[:], start=True, stop=True)
    mub_sb = p_tmp.tile([P, Tq], BF16, name="mub2_sb")
    nc.vector.tensor_copy(mub_sb[:], mub[:])
    rsb_sb = p_tmp.tile([P, Tq], BF16, name="rsb2_sb")
    nc.vector.tensor_copy(rsb_sb[:], rsb[:])
    # xn2 (scaled by SX) split into fp8 head + fp8 residual for DoubleRow MLP1
    for e in range(KE):
        t1 = p_tmp.tile([P, Tq], BF16, name="t1b")
        nc.gpsimd.tensor_tensor(t1[:], xres[e][:], mub_sb[:], OP.subtract)
        xn2bf = p_tmp.tile([P, Tq], BF16, name="xn2bf")
        nc.vector.tensor_tensor(xn2bf[:], t1[:], rsb_sb[:], OP.mult)
        nc.scalar.activation(x2h[:, e, :], xn2bf[:], AF.Copy)
        nc.vector.tensor_tensor(x2l[:, e, :], xn2bf[:], x2h[:, e, :], OP.subtract)
    pp.close("ln2_rows", "ln2_tmp", "ln2_bc", "ln2_st")

    # ======================================================================
    # Phase 4: MLP in split-fp8 DoubleRow: X~H+L, W~A+B (fp8 residuals);
    # X@W ~ H@A + H@B + L@A at 0.75x the bf16 PE cost, ~bf16 accuracy.
    # ======================================================================
    EH = 6
    p_h1 = pp.open("h1p", 1, side="right")
    p_hbf = pp.open("h1bfp", 3, side="right")
    p_out = pp.open("outp", 2)
    p_w1 = pp.open("w1s", 3)
    ps_h1 = pp.open("h1_ps", 2, "PSUM")
    ps_h2a = pp.open("h2a_ps", 1, "PSUM")

    h1h = p_h1.tile([P, KF, Tq], F8, name="h1h")
    h1l = p_h1.tile([P, KF, Tq], F8, name="h1l")
    h2a = [ps_h2a.tile([P, Tq], F32, name=f"h2a{e}") for e in range(EH)]

    def mlp2_acc(g, es, h2ps, first, last):
        for i, e in enumerate(es):
            for wt, ht in ((w2a, h1h), (w2b, h1h), (w2a, h1l)):
                fst = first and wt is w2a and ht is h1h
                lst = last and wt is w2a and ht is h1l
                nc.tensor.matmul(
                    h2ps[i][:], wt[:, 2 * g : 2 * g + 2, e * P : (e + 1) * P],
                    ht[:, 2 * g : 2 * g + 2, :],
                    start=fst, stop=False, perf_mode=DR, skip_group_check=True,
                )
            if last:
                # += b2*8192 via ones rhs, then stop the group
                nc.tensor.matmul(
                    h2ps[i][:], mb2dr[:, :, e * P : (e + 1) * P], ones8[:],
                    start=False, stop=True, perf_mode=DR, skip_group_check=True,
                )

    for f in range(KF):
        w1af = p_w1.tile([P, KE, P], F8, name="w1a")
        w1bf = p_w1.tile([P, KE, P], F8, name="w1b")
        nc.sync.dma_start(
            w1af[:],
            d["w1a"].rearrange("(e p) m -> p e m", p=P)[:, :, f * P : (f + 1) * P],
        )
        nc.sync.dma_start(
            w1bf[:],
            d["w1b"].rearrange("(e p) m -> p e m", p=P)[:, :, f * P : (f + 1) * P],
        )
        ps = ps_h1.tile([P, Tq], F32, name="h1ps")
        for g in range(KE // 2):
            for wt, xt_ in ((w1af, x2h), (w1bf, x2h), (w1af, x2l)):
                nc.tensor.matmul(
                    ps[:], wt[:, 2 * g : 2 * g + 2, :],
                    xt_[:, 2 * g : 2 * g + 2, :],
                    start=(g == 0 and wt is w1af and xt_ is x2h),
                    stop=(g == KE // 2 - 1 and wt is w1af and xt_ is x2l),
                    perf_mode=DR,
                )
        h1bf = p_hbf.tile([P, Tq], BF16, name="h1bf")
        nc.scalar.activation(
            h1bf[:], ps[:], AF.Relu, bias=gbt["mb1"][:, f : f + 1], scale=DS
        )
        nc.vector.tensor_scalar_mul(h1h[:, f, :], h1bf[:], SX)
        nc.vector.scalar_tensor_tensor(
            h1l[:, f, :], h1bf[:], SX, h1h[:, f, :], OP.mult, OP.subtract
        )
        if f % 2 == 1:
            mlp2_acc(f // 2, range(EH), h2a, first=(f == 1), last=(f == KF - 1))
    for e in range(EH):
        of = p_out.tile([P, Tq], F32, name="outf")
        nc.vector.scalar_tensor_tensor(
            of[:], h2a[e][:], DS, xres[e][:], OP.mult, OP.add
        )
        nc.sync.dma_start(d["out_t"][e * P : (e + 1) * P, :], of[:])
    pp.close("h2a_ps", "h1_ps")

    ps_h2b = pp.open("h2b_ps", 1, "PSUM")
    h2b = [ps_h2b.tile([P, Tq], F32, name=f"h2b{i}") for i in range(KE - EH)]
    for g in range(KF // 2):
        mlp2_acc(g, range(EH, KE), h2b, first=(g == 0), last=(g == KF // 2 - 1))
    for i, e in enumerate(range(EH, KE)):
        of = p_out.tile([P, Tq], F32, name="outf")
        nc.vector.scalar_tensor_tensor(
            of[:], h2b[i][:], DS, xres[e][:], OP.mult, OP.add
        )
        nc.sync.dma_start(d["out_t"][e * P : (e + 1) * P, :], of[:])

    pp.close_all()


def build_program(c: Cfg = CFG):
    c.check()
    nc = bacc.Bacc(
        "TRN2",
        target_bir_lowering=False,
        debug=False,
        enable_asserts=False,
        num_devices=c.NC,
    )
    d = {}
    d["xt"] = nc.dram_tensor("xt", [c.E, c.T], BF16, kind="ExternalInput").ap()
    for nm in ("wq8", "wk8", "wv8", "wo8"):
        d[nm] = nc.dram_tensor(nm, [c.E, c.E], F8, kind="ExternalInput").ap()
    d["w1a"] = nc.dram_tensor("w1a", [c.E, c.F], F8, kind="ExternalInput").ap()
    d["w1b"] = nc.dram_tensor("w1b", [c.E, c.F], F8, kind="ExternalInput").ap()
    d["w2a"] = nc.dram_tensor("w2a", [c.F, c.E], F8, kind="ExternalInput").ap()
    d["w2b"] = nc.dram_tensor("w2b", [c.F, c.E], F8, kind="ExternalInput").ap()
    d["mb2dr"] = nc.dram_tensor("mb2dr", [P, 2 * c.E], F8, kind="ExternalInput").ap()
    for nm, cols in [("bq", c.KE), ("bk", c.KE), ("bv32", c.KE), ("boc", c.KE),
                     ("mb1", c.KF)]:
        d[nm] = nc.dram_tensor(nm, [P, cols], F32, kind="ExternalInput").ap()
    d["band"] = nc.dram_tensor("band", [P, c.NQC * 2 * P], BF16,
                               kind="ExternalInput").ap()
    d["ident"] = nc.dram_tensor("ident", [P, P], BF16, kind="ExternalInput").ap()
    d["out_t"] = nc.dram_tensor("out_t", [c.E, c.Tq], F32, kind="ExternalOutput").ap()
    if DEBUG:
        d["dbg_xn"] = nc.dram_tensor("dbg_xn", [P, c.KE, c.T], F8,
                                     kind="ExternalOutput").ap()
        d["dbg_qt"] = nc.dram_tensor("dbg_qt", [P, c.HP * c.Tq], BF16,
                                     kind="ExternalOutput").ap()
        d["dbg_kt"] = nc.dram_tensor("dbg_kt", [P, c.HP * c.T], BF16,
                                     kind="ExternalOutput").ap()
        d["dbg_vs"] = nc.dram_tensor("dbg_vs", [P, c.TK * 1040], BF16,
                                     kind="ExternalOutput").ap()
        d["dbg_ao"] = nc.dram_tensor("dbg_ao", [P, c.HP, c.Tq], F8,
                                     kind="ExternalOutput").ap()
        d["dbg_rs"] = nc.dram_tensor("dbg_rs", [P, c.HP, 2, 4], F32,
                                     kind="ExternalOutput").ap()
        d["dbg_irs"] = nc.dram_tensor("dbg_irs", [P, c.HP, 2, 4], F32,
                                      kind="ExternalOutput").ap()
        d["dbg_pt"] = nc.dram_tensor("dbg_pt", [P, c.HP, 2, 256], BF16,
                                     kind="ExternalOutput").ap()

    with tile.TileContext(nc) as tc:
        _emit(tc, c, d)
    nc.compile()
    return nc


# --------------------------------------------------------------------------
# host side
# --------------------------------------------------------------------------
def shard_inputs(inputs, c: Cfg = CFG):
    f32 = lambda a: np.ascontiguousarray(np.asarray(a, np.float32))
    x = f32(inputs["x"])
    g1, b1n = f32(inputs["ln1_g"]), f32(inputs["ln1_b"])
    g2, b2n = f32(inputs["ln2_g"]), f32(inputs["ln2_b"])
    Wq, Wk, Wv, Wo = (f32(inputs[k]) for k in ("Wq", "Wk", "Wv", "Wo"))
    W1, W2 = f32(inputs["W1"]), f32(inputs["W2"])
    bo, b1, b2 = f32(inputs["bo"]), f32(inputs["b1"]), f32(inputs["b2"])

    q8 = lambda w: np.ascontiguousarray((w * SW)).astype(NPF8)
    bf = lambda w: np.ascontiguousarray(w).astype(NPBF16)
    chunks = lambda v, k: np.ascontiguousarray(v.reshape(k, P).T)

    def split8(w):
        ws = np.ascontiguousarray(w * SW)
        a = ws.astype(NPF8)
        b = (ws - a.astype(np.float32)).astype(NPF8)
        return a, b

    w1a, w1b = split8(g2[:, None] * W1)
    w2a, w2b = split8(W2)
    com = {
        "wq8": q8(g1[:, None] * Wq),
        "wk8": q8(g1[:, None] * Wk),
        "wv8": q8(g1[:, None] * Wv),
        "wo8": q8(Wo),
        "w1a": w1a, "w1b": w1b,
        "w2a": w2a, "w2b": w2b,
        "mb2dr": np.broadcast_to(
            (b2 * SX).astype(NPF8), (P, 2, c.E)
        ).reshape(P, 2 * c.E).copy(),
        "bq": chunks(b1n @ Wq, c.KE),
        "bk": chunks(b1n @ Wk, c.KE),
        "bv32": chunks((b1n @ Wv) * SX, c.KE),
        "boc": chunks(bo, c.KE),
        "mb1": chunks(b1 + b2n @ W1, c.KF),
        "ident": np.eye(P, dtype=np.float32).astype(NPBF16),
    }

    p_idx = np.arange(P)[:, None]
    lq_idx = np.arange(P)[None, :]
    maps = []
    for core in range(c.NC):
        b, j = core // c.CPB, core % c.CPB
        phases = [(j + 1) % 4, (j + 2) % 4, (j + 3) % 4, j]
        ctx = np.empty((c.E, c.T), np.float32)
        for i, ph in enumerate(phases):
            ctx[:, 512 * i : 512 * (i + 1)] = x[b, ph::4, :].T
        band = np.zeros((P, c.NQC, 2, P), np.float32)
        for i, ph in enumerate(phases):
            delta = 1 if ph > j else 0
            m_ = (p_idx <= lq_idx - delta).astype(np.float32)
            band[:, i, 0, :] = m_
            band[:, i, 1, :] = m_
        m = dict(com)
        m["xt"] = bf(ctx)
        m["band"] = band.reshape(P, c.NQC * 2 * P).astype(NPBF16)
        maps.append(m)
    return maps


def assemble(results, c: Cfg = CFG):
    out = np.empty((c.B, c.T, c.E), np.float32)
    for core in range(c.NC):
        b, j = core // c.CPB, core % c.CPB
        out[b, j::4, :] = results[core]["out_t"].T
    return out


_NC_CACHE = {}


def _get_nc(c: Cfg = CFG):
    if c not in _NC_CACHE:
        _NC_CACHE[c] = build_program(c)
    return _NC_CACHE[c]


LAST_RESULT = None


def kernel(**inputs):
    global LAST_RESULT
    c = CFG
    nc = _get_nc(c)
    maps = shard_inputs(inputs, c)
    res = bass_utils.run_bass_kernel_spmd(nc, maps, core_ids=list(range(c.NC)))
    LAST_RESULT = res
    return assemble(res.results, c)
